# revision 22
# baseline (speedup 1.0000x reference)
"""CEM sampling kernel for Trainium2, 8-core SPMD (population sharded).

Per core (512 of 4096 members), one fused program:

  Window (overlapped with the 42MB obs+noise HBM stream, ~117us):
   - DTW min-plus DP entirely on DVE (the scan/min ops exist only
     there): two packed pair-chains [t0|sep|t1] and [t2|sep|t3], DP
     state in fp16 (2x-mode mins; the scan's carry is internally fp32
     and the f32 cost rows are never rounded, so only the stored row
     values quantize).  ~1.1us/row.
   - Actions: ACT computes bf16 act = means + stds*noise per action
     dim, Pool clips in bf16, PE transposes [t,p] blocks to a
     population-major bf16 layout, ACT copies PSUM->SBUF and squares.
  Tail (~35us): AllGather dists; top-K via the gpsimd kth_largest
     library op on the [128,32] negated global dists (exact K-th
     threshold, replaces rank compares and broadcasts); weights; the
     weighted mean / E[x^2] reductions as 64 bf16 PE matmuls (with a
     p-state warmup) accumulating in PSUM; AllReduce; closing stats.
"""

import os
import sys

for _p in ("/opt/trn_rl_repo", "/root/.axon_site/_ro/trn_rl_repo"):
    if _p not in sys.path:
        sys.path.insert(0, _p)

import numpy as np

import concourse.bass as bass
import concourse.bacc as bacc
import concourse.bass_isa as bass_isa
import concourse.tile as tile
from concourse import mybir
from concourse import bass_utils
from concourse.masks import make_identity

F32 = mybir.dt.float32
FP16 = mybir.dt.float16
BF16 = mybir.dt.bfloat16
ALU = mybir.AluOpType
ACTF = mybir.ActivationFunctionType

P, T, A = 4096, 128, 32
NCORES = 8
PL = P // NCORES          # 512 population per core
NT = PL // 128            # 4 tiles of 128 on the partition dim
K = int(P * 0.1)          # 409
TEMP, MOM, MIN_STD = 0.5, 0.1, 0.05
INFDP = 30000.0           # fp16-safe stand-in for +inf in the DP
CSCALE = 0.125            # cost prescale: positive scaling preserves the
                          # min-plus argmin and shrinks the fp16 quantum of
                          # the DP values ~4-8x; TEMP rescales to match
TEMP_EFF = TEMP / CSCALE
RCH = int(os.environ.get("CEM_RCH", "8"))   # DP rows per streamed chunk
NCHUNK = T // RCH
CBUFS = int(os.environ.get("CEM_CBUFS", "3"))
WARM = int(os.environ.get("CEM_WARM", "8"))  # PE p-state warmup matmuls
DPDT = FP16 if os.environ.get("CEM_DPDT", "fp16") == "fp16" else F32
GROUP = [list(range(NCORES))]

# packed cost-row layout: [t0(128) sep t1(128) | t2(128) sep t3(128)]
CW = 257                  # cost width of one pair-chain
CWF = 514
SEP1, SEP2 = 128, 385
DMAP = {0: 0, 1: 129, 2: 257, 3: 386}  # pop tile -> flat cost column

_CACHE = {}


def _build(stage=9, single=False):
    nc = bacc.Bacc(
        "TRN2",
        target_bir_lowering=False,
        debug=False,
        num_devices=1 if single else NCORES,
    )
    obs_d = nc.dram_tensor("obs", [PL, T, T], F32, kind="ExternalInput")
    means_d = nc.dram_tensor("means", [T, 1, A], F32, kind="ExternalInput")
    stds_d = nc.dram_tensor("stds", [T, 1, A], F32, kind="ExternalInput")
    noise_d = nc.dram_tensor("noise", [T, PL, A], F32, kind="ExternalInput")
    out_d = nc.dram_tensor("out", [2, T, 1, A], F32, kind="ExternalOutput")

    with tile.TileContext(nc) as tc:
        with (
            tc.tile_pool(name="main", bufs=1) as mp,
            tc.tile_pool(name="dram", bufs=1, space="DRAM") as dp,
        ):
            # ---- small persistent tiles
            means_t = mp.tile([T, A], F32)
            stds_t = mp.tile([T, A], F32)
            nc.sync.dma_start(means_t[:], means_d[:, 0, :])
            nc.sync.dma_start(stds_t[:], stds_d[:, 0, :])
            ident = mp.tile([128, 128], BF16)
            make_identity(nc, ident[:])
            # preload the ACT function tables used in the tail
            warmt = mp.tile([128, 1], F32)
            nc.scalar.activation(warmt[:], means_t[:, 0:1], ACTF.Exp)
            nc.scalar.sqrt(warmt[:], warmt[:])

            # actions (bf16), noise staging quarters, transposed layouts
            actb = mp.tile([T, PL, A], BF16)
            utile = mp.tile([128, 2 * PL * A // 4], F32)  # [128, 8192]
            nhq = [
                utile[:, 0:4096].rearrange("t (p a) -> t p a", a=A),
                utile[:, 4096:8192].rearrange("t (p a) -> t p a", a=A),
            ]
            actT = mp.tile([128, NT, T, A], BF16)
            act2T = mp.tile([128, NT, T, A], BF16)

            # ---- DTW state: two packed pair-chains, ping-pong, DPDT
            h01a = mp.tile([128, CW + 1], DPDT)
            h01b = mp.tile([128, CW + 1], DPDT)
            h23a = mp.tile([128, CW + 1], DPDT)
            h23b = mp.tile([128, CW + 1], DPDT)
            ub01 = mp.tile([128, CW], DPDT)
            ub23 = mp.tile([128, CW], DPDT)
            for t_ in (h01a, h01b, h23a, h23b):
                nc.vector.memset(t_[:], INFDP)
            # D[0][0] = 0 for each tile (pair cols 0 and 129)
            nc.vector.memset(h01a[:, 0:1], 0.0)
            nc.vector.memset(h01a[:, 129:130], 0.0)
            nc.vector.memset(h23a[:, 0:1], 0.0)
            nc.vector.memset(h23a[:, 129:130], 0.0)

            down = mp.tile([128, NT], F32)
            ch01 = (h01a, h01b)
            ch23 = (h23a, h23b)

            def dtw_row(i, cb, r):
                crow = cb[:, r]
                A1, B1 = ch01[i % 2], ch01[(i + 1) % 2]
                A2, B2 = ch23[i % 2], ch23[(i + 1) % 2]
                nc.vector.tensor_tensor(
                    ub01[:], A1[:, 0:CW], A1[:, 1 : CW + 1], op=ALU.min
                )
                nc.vector.tensor_tensor(
                    ub23[:], A2[:, 0:CW], A2[:, 1 : CW + 1], op=ALU.min
                )
                nc.vector.tensor_tensor_scan(
                    B1[:, 1 : CW + 1], ub01[:], crow[:, 0:CW], INFDP,
                    op0=ALU.min, op1=ALU.add,
                )
                nc.vector.tensor_tensor_scan(
                    B2[:, 1 : CW + 1], ub23[:], crow[:, CW:CWF], INFDP,
                    op0=ALU.min, op1=ALU.add,
                )
                if i == 0:
                    # D[i>0][0] = INF at the never-rewritten left columns
                    nc.vector.memset(h01a[:, 0:1], INFDP)
                    nc.vector.memset(h23a[:, 0:1], INFDP)

            # ---- actions pipeline pieces (emitted interleaved with DTW)
            def noise_dma(q):
                nc.sync.dma_start(
                    nhq[q % 2][:], noise_d[:, q * 128 : (q + 1) * 128, :]
                )

            def affine(q):
                for a in range(A):
                    nc.scalar.activation(
                        actb[:, q * 128 : (q + 1) * 128, a],
                        nhq[q % 2][:, :, a],
                        ACTF.Identity,
                        bias=means_t[:, a : a + 1],
                        scale=stds_t[:, a : a + 1],
                    )

            def clip(k):
                v = actb[:, k * 128 : (k + 1) * 128, :].rearrange(
                    "t p a -> t (p a)"
                )
                nc.gpsimd.tensor_scalar(
                    v, v, 1.0, -1.0, op0=ALU.min, op1=ALU.max
                )

            def transposes(tpp, k):
                for a in range(A):
                    pt = tpp.tile([128, 128], BF16, tag="tp")
                    nc.tensor.transpose(
                        pt[:],
                        actb[:, k * 128 : (k + 1) * 128, a],
                        ident[:],
                    )
                    nc.scalar.activation(
                        actT[:, k, :, a], pt[:], ACTF.Copy
                    )

            def square(k):
                nc.scalar.activation(
                    act2T[:, k].rearrange("t a b -> t (a b)"),
                    actT[:, k].rearrange("t a b -> t (a b)"),
                    ACTF.Square,
                )

            # ---- window: obs chunks + DTW rows + action stages
            with tc.tile_pool(name="cwin", bufs=CBUFS) as cp, \
                 tc.tile_pool(name="psum_tp", bufs=4, space="PSUM") as tpp:
                cbs = []

                def chunk_dma(c):
                    cb = cp.tile([128, RCH, CWF], F32, tag="cw")
                    for k in range(NT):
                        o = DMAP[k]
                        nc.sync.dma_start(
                            cb[:, :, o : o + T],
                            obs_d[k * 128 : (k + 1) * 128,
                                  c * RCH : (c + 1) * RCH, :],
                        )
                    # separators + cost prescale (Pool is otherwise idle
                    # in the window; the scale is folded into TEMP_EFF)
                    nc.gpsimd.memset(cb[:, :, SEP1 : SEP1 + 1], INFDP)
                    nc.gpsimd.memset(cb[:, :, SEP2 : SEP2 + 1], INFDP)
                    cbf = cb[:].rearrange("t r w -> t (r w)")
                    nc.gpsimd.tensor_scalar(
                        cbf, cbf, CSCALE, None, op0=ALU.mult
                    )
                    return cb

                cbs.append(chunk_dma(0))
                noise_dma(0)
                for c in range(1, min(CBUFS, NCHUNK)):
                    cbs.append(chunk_dma(c))

                acts = {
                    0: [lambda: affine(0), lambda: noise_dma(1)],
                    1: [lambda: affine(1), lambda: noise_dma(2)],
                    2: [lambda: clip(0)],
                    3: [lambda: affine(2), lambda: noise_dma(3),
                        lambda: clip(1)],
                    4: [lambda: transposes(tpp, 0)],
                    5: [lambda: affine(3), lambda: clip(2)],
                    6: [lambda: transposes(tpp, 1), lambda: square(0)],
                    7: [lambda: clip(3)],
                    8: [lambda: transposes(tpp, 2), lambda: square(1)],
                    9: [lambda: transposes(tpp, 3)],
                    10: [lambda: square(2)],
                    11: [lambda: square(3)],
                }

                for c in range(NCHUNK):
                    cb = cbs[c]
                    for r in range(RCH):
                        dtw_row(c * RCH + r, cb, r)
                    if c + CBUFS < NCHUNK:
                        cbs.append(chunk_dma(c + CBUFS))
                    if stage >= 1:
                        for th in acts.get(c, []):
                            th()

            # own dists from the final (even-side) buffers, fp16 -> f32
            nc.scalar.activation(down[:, 0:1], h01a[:, 128:129], ACTF.Copy)
            nc.scalar.activation(down[:, 1:2], h01a[:, 257:258], ACTF.Copy)
            nc.scalar.activation(down[:, 2:3], h23a[:, 128:129], ACTF.Copy)
            nc.scalar.activation(down[:, 3:4], h23a[:, 257:258], ACTF.Copy)

            if stage >= 2:
                # ---- AllGather dists (tiny)
                ld = dp.tile([PL], F32)
                gd = dp.tile([P], F32)
                # member order in gd is irrelevant (kth/threshold are
                # order-free), so write ld partition-major: 4x fewer descs
                nc.sync.dma_start(ld.rearrange("(p k) -> p k", k=NT), down[:])
                if single:
                    _, lsrc = bass.broadcast_tensor_aps(
                        gd.rearrange("(r f) -> r f", r=NCORES),
                        ld.rearrange("(o f) -> o f", o=1),
                    )
                    nc.sync.dma_start(
                        gd.rearrange("(r f) -> r f", r=NCORES), lsrc
                    )
                else:
                    nc.gpsimd.collective_compute(
                        "AllGather",
                        ALU.bypass,
                        replica_groups=GROUP,
                        ins=[ld.opt()],
                        outs=[gd.opt()],
                    )

            if stage >= 3:
                # ---- top-K threshold via gpsimd kth_largest on -dists
                gdsq = mp.tile([128, P // 128], F32)
                nc.sync.dma_start(
                    gdsq[:], gd.rearrange("(p f) -> p f", p=128)
                )
                ngd = mp.tile([128, P // 128], F32)
                nc.vector.tensor_scalar(
                    ngd[:], gdsq[:], -1.0, None, op0=ALU.mult
                )
                kth = mp.tile([128, 2], F32)
                nc.gpsimd.kth_largest(
                    kth[:], ngd[:], P // 128, K + 3,
                    quantile=1.0 - (K - 0.5) / (P - 1),
                )
                # kth col1 = desc[k_adj+1] = -s[K] ; mask = d < s[K]
                thb = mp.tile([128, 2], F32)
                nc.gpsimd.partition_broadcast(thb[:], kth[0:1, :])
                thneg = mp.tile([128, 1], F32)
                nc.vector.tensor_scalar(
                    thneg[:], thb[:, 1:2], -1.0, None, op0=ALU.mult
                )
                # softmax shift: any global constant cancels exactly; use
                # gd[0] (safe unless the dist spread nears 176/TEMP).
                dref = mp.tile([128, 1], F32)
                nc.gpsimd.partition_broadcast(dref[:], gdsq[0:1, 0:1])
                biast = mp.tile([128, 1], F32)
                nc.gpsimd.tensor_scalar(
                    biast[:], dref[:], TEMP_EFF, None, op0=ALU.mult
                )
                mask4 = mp.tile([128, NT], F32)
                nc.vector.tensor_scalar(
                    mask4[:], down[:], thneg[:, 0:1], None, op0=ALU.is_lt
                )
                e4 = mp.tile([128, NT], F32)
                nc.scalar.activation(
                    e4[:], down[:], ACTF.Exp, bias=biast[:, 0:1],
                    scale=-TEMP_EFF,
                )
                w4 = mp.tile([128, NT], F32)
                nc.vector.tensor_tensor(w4[:], e4[:], mask4[:], op=ALU.mult)
                wb = mp.tile([128, NT], BF16)
                nc.scalar.activation(wb[:], w4[:], ACTF.Copy)
                # sum of weights across members (free then partitions)
                slocal = mp.tile([128, 1], F32)
                nc.vector.tensor_reduce(
                    slocal[:], w4[:], axis=mybir.AxisListType.X, op=ALU.add
                )
                swr = mp.tile([128, 1], F32)
                nc.gpsimd.partition_all_reduce(
                    swr[:], slocal[:], 128, bass_isa.ReduceOp.add
                )
                # dnb: bf16 dists, ready at DTW end -- gates the PE warmup
                dnb = mp.tile([128, NT], BF16)
                nc.scalar.activation(dnb[:], down[:], ACTF.Copy)

            if stage >= 4:
                # ---- weighted sums as PE matmuls accumulating over tiles
                NTOT = 2 * T * A + 1
                arin = dp.tile([NTOT], F32)
                arout = dp.tile([NTOT], F32)
                nc.sync.dma_start(
                    arin[2 * T * A : NTOT].rearrange("(o f) -> o f", o=1),
                    swr[0:1, 0:1],
                )
                _pst_cm = tc.tile_pool(name="psum_st", bufs=1, space="PSUM")
                pst = _pst_cm.__enter__()
                sts = []
                for c in range(8):
                    st = pst.tile([128, 512], F32, tag=f"st{c}")
                    sts.append(st)
                # PE p-state warmup: junk matmuls gated on the dists; their
                # outputs are reset by the first start=True real matmul.
                for wi in range(WARM):
                    nc.tensor.matmul(
                        sts[wi % 8][0:1, :],
                        dnb[:, 0:1],
                        actT[:, wi % NT, (wi % 8) * 16 : (wi % 8) * 16 + 16, :],
                        start=True, stop=True, skip_group_check=True,
                    )
                # staging rows alias dead actb (32-aligned partitions)
                arsc = actb[:].rearrange("t p a -> t (p a)").bitcast(F32)
                arsb_m = arsc[0:1, 0 : T * A]
                arsb_s = arsc[32:33, 0 : T * A]
                for c in range(8):
                    for k in range(NT):
                        nc.tensor.matmul(
                            sts[c][0:1, :],
                            wb[:, k : k + 1],
                            actT[:, k, c * 16 : (c + 1) * 16, :],
                            start=(k == 0), stop=(k == NT - 1),
                        )
                    nc.scalar.activation(
                        arsb_m[:, c * 512 : (c + 1) * 512],
                        sts[c][0:1, :], ACTF.Copy,
                    )
                for c in range(8):
                    for k in range(NT):
                        nc.tensor.matmul(
                            sts[c][32:33, :],
                            wb[:, k : k + 1],
                            act2T[:, k, c * 16 : (c + 1) * 16, :],
                            start=(k == 0), stop=(k == NT - 1),
                        )
                    nc.vector.tensor_copy(
                        arsb_s[:, c * 512 : (c + 1) * 512],
                        sts[c][32:33, :],
                    )

                nc.sync.dma_start(
                    arin[0 : T * A].rearrange("(o f) -> o f", o=1), arsb_m[:]
                )
                nc.sync.dma_start(
                    arin[T * A : 2 * T * A].rearrange("(o f) -> o f", o=1),
                    arsb_s[:],
                )
                if single:
                    nc.sync.dma_start(arout[:], arin[:])
                else:
                    nc.gpsimd.collective_compute(
                        "AllReduce",
                        ALU.add,
                        replica_groups=GROUP,
                        ins=[arin.opt()],
                        outs=[arout.opt()],
                    )
                _pst_cm.__exit__(None, None, None)

            if stage >= 5:
                # ---- final statistics
                rn12 = mp.tile([128, 2, A], F32)
                nc.sync.dma_start(
                    rn12[:],
                    arout[0 : 2 * T * A].rearrange(
                        "(q t a) -> t q a", q=2, t=T
                    ),
                )
                rs = mp.tile([128, 1], F32)
                _, rssrc = bass.broadcast_tensor_aps(
                    rs[:],
                    arout[2 * T * A : NTOT].rearrange("(o f) -> o f", o=1),
                )
                nc.sync.dma_start(rs[:], rssrc)
                rn1 = rn12[:, 0]
                rn2 = rn12[:, 1]
                inv = mp.tile([128, 1], F32)
                nc.vector.reciprocal(inv[:], rs[:])
                mh = mp.tile([128, A], F32)
                nc.vector.tensor_scalar(
                    mh[:], rn1, inv[:, 0:1], None, op0=ALU.mult
                )
                q = mp.tile([128, A], F32)
                nc.vector.tensor_scalar(
                    q[:], rn2, inv[:, 0:1], None, op0=ALU.mult
                )
                msq = mp.tile([128, A], F32)
                nc.vector.tensor_tensor(msq[:], mh[:], mh[:], op=ALU.mult)
                var = mp.tile([128, A], F32)
                nc.vector.tensor_tensor(var[:], q[:], msq[:], op=ALU.subtract)
                nc.vector.tensor_scalar(var[:], var[:], 0.0, None, op0=ALU.max)
                stdv = mp.tile([128, A], F32)
                nc.scalar.sqrt(stdv[:], var[:])
                nc.vector.tensor_scalar(
                    stdv[:], stdv[:], MIN_STD, 1.0, op0=ALU.max, op1=ALU.min
                )
                ostk = mp.tile([128, 2, A], F32)
                nc.vector.tensor_scalar(
                    mh[:], mh[:], 1.0 - MOM, None, op0=ALU.mult
                )
                nc.vector.scalar_tensor_tensor(
                    ostk[:, 0], means_t[:], MOM, mh[:], op0=ALU.mult,
                    op1=ALU.add,
                )
                nc.vector.tensor_copy(ostk[:, 1], stdv[:])
                nc.sync.dma_start(
                    out_d.rearrange("q t o a -> t (q o) a"), ostk[:]
                )
            else:
                # bisect debug output
                dbg = mp.tile([128, A], F32)
                nc.vector.memset(dbg[:], 0.0)
                nc.vector.tensor_copy(dbg[:, 0:NT], down[:])
                if stage >= 3:
                    nc.vector.tensor_copy(dbg[:, 4 : 4 + NT], w4[:])
                    nc.vector.tensor_copy(dbg[:, 8:9], thneg[:])
                    nc.vector.tensor_copy(dbg[:, 9:10], swr[:])
                if stage == 2:
                    gdbg = mp.tile([128, A], F32)
                    nc.sync.dma_start(
                        gdbg[:],
                        gd[0 : 128 * A].rearrange("(p a) -> p a", a=A),
                    )
                    nc.vector.tensor_copy(dbg[:, 4:8], gdbg[:, 0:4])
                nc.sync.dma_start(out_d[0, :, 0, :], dbg[:])
                nc.sync.dma_start(out_d[1, :, 0, :], dbg[:])

    nc.compile()
    return nc


def _get_nc(stage=None, single=None):
    if stage is None:
        stage = int(os.environ.get("CEM_STAGE", "9"))
    if single is None:
        single = bool(int(os.environ.get("CEM_SINGLE", "0")))
    key = ("nc", stage, single)
    if key not in _CACHE:
        _CACHE[key] = _build(stage, single)
    return _CACHE[key]


def kernel(**inputs):
    obs = np.ascontiguousarray(np.asarray(inputs["obs_diffs"], np.float32))
    means = np.ascontiguousarray(np.asarray(inputs["means"], np.float32))
    stds = np.ascontiguousarray(np.asarray(inputs["stds"], np.float32))
    noise = np.ascontiguousarray(np.asarray(inputs["noise"], np.float32))

    nc = _get_nc(stage=9, single=False)
    in_maps = []
    for c in range(NCORES):
        in_maps.append(
            {
                "obs": obs[c * PL : (c + 1) * PL],
                "means": means,
                "stds": stds,
                "noise": np.ascontiguousarray(noise[:, c * PL : (c + 1) * PL, :]),
            }
        )
    res = bass_utils.run_bass_kernel_spmd(
        nc, in_maps, core_ids=list(range(NCORES))
    )
    out = np.asarray(res.results[0]["out"], np.float32)
    return out.reshape(2, T, 1, A)


# revision 25
# speedup vs baseline: 1.0572x; 1.0572x over previous
"""CEM sampling kernel for Trainium2, 8-core SPMD (population sharded).

Per core (512 of 4096 members), one fused program:

  Window (overlapped with the 42MB obs+noise HBM stream, ~117us):
   - DTW min-plus DP entirely on DVE (the scan/min ops exist only
     there): two packed pair-chains [t0|sep|t1] and [t2|sep|t3], DP
     state in fp16 (2x-mode mins; the scan's carry is internally fp32
     and the f32 cost rows are never rounded, so only the stored row
     values quantize).  ~1.1us/row.
   - Actions: ACT computes bf16 act = means + stds*noise per action
     dim, Pool clips in bf16, PE transposes [t,p] blocks to a
     population-major bf16 layout, ACT copies PSUM->SBUF and squares.
  Tail (~35us): AllGather dists; top-K via the gpsimd kth_largest
     library op on the [128,32] negated global dists (exact K-th
     threshold, replaces rank compares and broadcasts); weights; the
     weighted mean / E[x^2] reductions as 64 bf16 PE matmuls (with a
     p-state warmup) accumulating in PSUM; AllReduce; closing stats.
"""

import os
import sys

for _p in ("/opt/trn_rl_repo", "/root/.axon_site/_ro/trn_rl_repo"):
    if _p not in sys.path:
        sys.path.insert(0, _p)

import numpy as np

import concourse.bass as bass
import concourse.bacc as bacc
import concourse.bass_isa as bass_isa
import concourse.tile as tile
from concourse import mybir
from concourse import bass_utils
from concourse.masks import make_identity

F32 = mybir.dt.float32
FP16 = mybir.dt.float16
BF16 = mybir.dt.bfloat16
ALU = mybir.AluOpType
ACTF = mybir.ActivationFunctionType

P, T, A = 4096, 128, 32
NCORES = 8
PL = P // NCORES          # 512 population per core
NT = PL // 128            # 4 tiles of 128 on the partition dim
K = int(P * 0.1)          # 409
TEMP, MOM, MIN_STD = 0.5, 0.1, 0.05
INFDP = 30000.0           # fp16-safe stand-in for +inf in the DP
RCH = int(os.environ.get("CEM_RCH", "8"))   # DP rows per streamed chunk
NCHUNK = T // RCH
CBUFS = int(os.environ.get("CEM_CBUFS", "3"))
WARM = int(os.environ.get("CEM_WARM", "8"))  # PE p-state warmup matmuls
DPDT = FP16 if os.environ.get("CEM_DPDT", "fp16") == "fp16" else F32
R16 = int(os.environ.get("CEM_R16", "80"))  # rows in fp16 before f32
if DPDT == F32:
    R16 = 0
GROUP = [list(range(NCORES))]

# packed cost-row layout: [t0(128) sep t1(128) | t2(128) sep t3(128)]
CW = 257                  # cost width of one pair-chain
CWF = 514
SEP1, SEP2 = 128, 385
DMAP = {0: 0, 1: 129, 2: 257, 3: 386}  # pop tile -> flat cost column

_CACHE = {}


def _build(stage=9, single=False):
    nc = bacc.Bacc(
        "TRN2",
        target_bir_lowering=False,
        debug=False,
        num_devices=1 if single else NCORES,
    )
    obs_d = nc.dram_tensor("obs", [PL, T, T], F32, kind="ExternalInput")
    means_d = nc.dram_tensor("means", [T, 1, A], F32, kind="ExternalInput")
    stds_d = nc.dram_tensor("stds", [T, 1, A], F32, kind="ExternalInput")
    noise_d = nc.dram_tensor("noise", [T, PL, A], F32, kind="ExternalInput")
    out_d = nc.dram_tensor("out", [2, T, 1, A], F32, kind="ExternalOutput")

    with tile.TileContext(nc) as tc:
        with (
            tc.tile_pool(name="main", bufs=1) as mp,
            tc.tile_pool(name="dram", bufs=1, space="DRAM") as dp,
        ):
            # ---- small persistent tiles
            means_t = mp.tile([T, A], F32)
            stds_t = mp.tile([T, A], F32)
            nc.sync.dma_start(means_t[:], means_d[:, 0, :])
            nc.sync.dma_start(stds_t[:], stds_d[:, 0, :])
            ident = mp.tile([128, 128], BF16)
            make_identity(nc, ident[:])
            # preload the ACT function tables used in the tail
            warmt = mp.tile([128, 1], F32)
            nc.scalar.activation(warmt[:], means_t[:, 0:1], ACTF.Exp)
            nc.scalar.sqrt(warmt[:], warmt[:])

            # actions (bf16), noise staging quarters, transposed layouts
            actb = mp.tile([T, PL, A], BF16)
            utile = mp.tile([128, 2 * PL * A // 4], F32)  # [128, 8192]
            nhq = [
                utile[:, 0:4096].rearrange("t (p a) -> t p a", a=A),
                utile[:, 4096:8192].rearrange("t (p a) -> t p a", a=A),
            ]
            actT = mp.tile([128, NT, T, A], BF16)
            act2T = mp.tile([128, NT, T, A], BF16)

            # ---- DTW state: two packed pair-chains, ping-pong.  Rows
            # < R16 keep the DP values in fp16 (2x-mode mins); the last
            # rows -- where the absolute values and hence fp16 quanta are
            # largest -- run in f32 so the accumulated rounding stays small.
            h01a = mp.tile([128, CW + 1], FP16)
            h01b = mp.tile([128, CW + 1], FP16)
            h23a = mp.tile([128, CW + 1], FP16)
            h23b = mp.tile([128, CW + 1], FP16)
            f01a = mp.tile([128, CW + 1], F32)
            f01b = mp.tile([128, CW + 1], F32)
            f23a = mp.tile([128, CW + 1], F32)
            f23b = mp.tile([128, CW + 1], F32)
            ub01 = mp.tile([128, CW], FP16)
            ub23 = mp.tile([128, CW], FP16)
            uf01 = mp.tile([128, CW], F32)
            uf23 = mp.tile([128, CW], F32)
            for t_ in (h01a, h01b, h23a, h23b, f01a, f01b, f23a, f23b):
                nc.vector.memset(t_[:], INFDP)
            # D[0][0] = 0 for each tile (pair cols 0 and 129)
            nc.vector.memset(h01a[:, 0:1], 0.0)
            nc.vector.memset(h01a[:, 129:130], 0.0)
            nc.vector.memset(h23a[:, 0:1], 0.0)
            nc.vector.memset(h23a[:, 129:130], 0.0)
            down = mp.tile([128, NT], F32)
            ch01 = (h01a, h01b)
            ch23 = (h23a, h23b)
            cf01 = (f01a, f01b)
            cf23 = (f23a, f23b)

            def dtw_row(i, cb, r):
                crow = cb[:, r]
                # row i reads the side written at i-1: fp16 through row R16,
                # f32 after; the switch row reads fp16 and writes f32.  The
                # f32 pair's col 0 is INFDP from init and never rewritten.
                A1 = (ch01 if i <= R16 else cf01)[i % 2]
                A2 = (ch23 if i <= R16 else cf23)[i % 2]
                if i < R16:
                    B1, B2, u1, u2 = (
                        ch01[(i + 1) % 2], ch23[(i + 1) % 2], ub01, ub23)
                else:
                    B1, B2, u1, u2 = (
                        cf01[(i + 1) % 2], cf23[(i + 1) % 2], uf01, uf23)
                nc.vector.tensor_tensor(
                    u1[:], A1[:, 0:CW], A1[:, 1 : CW + 1], op=ALU.min
                )
                nc.vector.tensor_tensor(
                    u2[:], A2[:, 0:CW], A2[:, 1 : CW + 1], op=ALU.min
                )
                nc.vector.tensor_tensor_scan(
                    B1[:, 1 : CW + 1], u1[:], crow[:, 0:CW], INFDP,
                    op0=ALU.min, op1=ALU.add,
                )
                nc.vector.tensor_tensor_scan(
                    B2[:, 1 : CW + 1], u2[:], crow[:, CW:CWF], INFDP,
                    op0=ALU.min, op1=ALU.add,
                )
                if i == 0:
                    # D[i>0][0] = INF at the never-rewritten left columns
                    nc.vector.memset(h01a[:, 0:1], INFDP)
                    nc.vector.memset(h23a[:, 0:1], INFDP)

            # ---- actions pipeline pieces (emitted interleaved with DTW)
            def noise_dma(q):
                nc.sync.dma_start(
                    nhq[q % 2][:], noise_d[:, q * 128 : (q + 1) * 128, :]
                )

            def affine(q):
                for a in range(A):
                    nc.scalar.activation(
                        actb[:, q * 128 : (q + 1) * 128, a],
                        nhq[q % 2][:, :, a],
                        ACTF.Identity,
                        bias=means_t[:, a : a + 1],
                        scale=stds_t[:, a : a + 1],
                    )

            def clip(k):
                v = actb[:, k * 128 : (k + 1) * 128, :].rearrange(
                    "t p a -> t (p a)"
                )
                nc.gpsimd.tensor_scalar(
                    v, v, 1.0, -1.0, op0=ALU.min, op1=ALU.max
                )

            def transposes(tpp, k):
                for a in range(A):
                    pt = tpp.tile([128, 128], BF16, tag="tp")
                    nc.tensor.transpose(
                        pt[:],
                        actb[:, k * 128 : (k + 1) * 128, a],
                        ident[:],
                    )
                    nc.scalar.activation(
                        actT[:, k, :, a], pt[:], ACTF.Copy
                    )

            def square(k):
                nc.scalar.activation(
                    act2T[:, k].rearrange("t a b -> t (a b)"),
                    actT[:, k].rearrange("t a b -> t (a b)"),
                    ACTF.Square,
                )

            # ---- window: obs chunks + DTW rows + action stages
            with tc.tile_pool(name="cwin", bufs=CBUFS) as cp, \
                 tc.tile_pool(name="psum_tp", bufs=4, space="PSUM") as tpp:
                cbs = []

                def chunk_dma(c):
                    cb = cp.tile([128, RCH, CWF], F32, tag="cw")
                    for k in range(NT):
                        o = DMAP[k]
                        nc.sync.dma_start(
                            cb[:, :, o : o + T],
                            obs_d[k * 128 : (k + 1) * 128,
                                  c * RCH : (c + 1) * RCH, :],
                        )
                    # refresh both INF separators each generation
                    nc.gpsimd.memset(cb[:, :, SEP1 : SEP1 + 1], INFDP)
                    nc.gpsimd.memset(cb[:, :, SEP2 : SEP2 + 1], INFDP)
                    return cb

                cbs.append(chunk_dma(0))
                noise_dma(0)
                for c in range(1, min(CBUFS, NCHUNK)):
                    cbs.append(chunk_dma(c))

                acts = {
                    0: [lambda: affine(0), lambda: noise_dma(1)],
                    1: [lambda: affine(1), lambda: noise_dma(2)],
                    2: [lambda: clip(0)],
                    3: [lambda: affine(2), lambda: noise_dma(3),
                        lambda: clip(1)],
                    4: [lambda: transposes(tpp, 0)],
                    5: [lambda: affine(3), lambda: clip(2)],
                    6: [lambda: transposes(tpp, 1), lambda: square(0)],
                    7: [lambda: clip(3)],
                    8: [lambda: transposes(tpp, 2), lambda: square(1)],
                    9: [lambda: transposes(tpp, 3)],
                    10: [lambda: square(2)],
                    11: [lambda: square(3)],
                }

                for c in range(NCHUNK):
                    cb = cbs[c]
                    for r in range(RCH):
                        dtw_row(c * RCH + r, cb, r)
                    if c + CBUFS < NCHUNK:
                        cbs.append(chunk_dma(c + CBUFS))
                    if stage >= 1:
                        for th in acts.get(c, []):
                            th()

            # own dists from the final (even-side) f32 buffers
            nc.scalar.activation(down[:, 0:1], f01a[:, 128:129], ACTF.Copy)
            nc.scalar.activation(down[:, 1:2], f01a[:, 257:258], ACTF.Copy)
            nc.scalar.activation(down[:, 2:3], f23a[:, 128:129], ACTF.Copy)
            nc.scalar.activation(down[:, 3:4], f23a[:, 257:258], ACTF.Copy)

            if stage >= 2:
                # ---- AllGather dists (tiny)
                ld = dp.tile([PL], F32)
                gd = dp.tile([P], F32)
                # member order in gd is irrelevant (kth/threshold are
                # order-free), so write ld partition-major: 4x fewer descs
                nc.sync.dma_start(ld.rearrange("(p k) -> p k", k=NT), down[:])
                if single:
                    _, lsrc = bass.broadcast_tensor_aps(
                        gd.rearrange("(r f) -> r f", r=NCORES),
                        ld.rearrange("(o f) -> o f", o=1),
                    )
                    nc.sync.dma_start(
                        gd.rearrange("(r f) -> r f", r=NCORES), lsrc
                    )
                else:
                    nc.gpsimd.collective_compute(
                        "AllGather",
                        ALU.bypass,
                        replica_groups=GROUP,
                        ins=[ld.opt()],
                        outs=[gd.opt()],
                    )

            if stage >= 3:
                # ---- top-K threshold via gpsimd kth_largest on -dists
                gdsq = mp.tile([128, P // 128], F32)
                nc.sync.dma_start(
                    gdsq[:], gd.rearrange("(p f) -> p f", p=128)
                )
                ngd = mp.tile([128, P // 128], F32)
                nc.vector.tensor_scalar(
                    ngd[:], gdsq[:], -1.0, None, op0=ALU.mult
                )
                kth = mp.tile([128, 2], F32)
                nc.gpsimd.kth_largest(
                    kth[:], ngd[:], P // 128, K + 3,
                    quantile=1.0 - (K - 0.5) / (P - 1),
                )
                # kth col1 = desc[k_adj+1] = -s[K] ; mask = d < s[K]
                thb = mp.tile([128, 2], F32)
                nc.gpsimd.partition_broadcast(thb[:], kth[0:1, :])
                thneg = mp.tile([128, 1], F32)
                nc.vector.tensor_scalar(
                    thneg[:], thb[:, 1:2], -1.0, None, op0=ALU.mult
                )
                # softmax shift: any global constant cancels exactly; use
                # gd[0] (safe unless the dist spread nears 176/TEMP).
                dref = mp.tile([128, 1], F32)
                nc.gpsimd.partition_broadcast(dref[:], gdsq[0:1, 0:1])
                biast = mp.tile([128, 1], F32)
                nc.gpsimd.tensor_scalar(
                    biast[:], dref[:], TEMP, None, op0=ALU.mult
                )
                mask4 = mp.tile([128, NT], F32)
                nc.vector.tensor_scalar(
                    mask4[:], down[:], thneg[:, 0:1], None, op0=ALU.is_lt
                )
                e4 = mp.tile([128, NT], F32)
                nc.scalar.activation(
                    e4[:], down[:], ACTF.Exp, bias=biast[:, 0:1], scale=-TEMP
                )
                w4 = mp.tile([128, NT], F32)
                nc.vector.tensor_tensor(w4[:], e4[:], mask4[:], op=ALU.mult)
                wb = mp.tile([128, NT], BF16)
                nc.scalar.activation(wb[:], w4[:], ACTF.Copy)
                # sum of weights across members (free then partitions)
                slocal = mp.tile([128, 1], F32)
                nc.vector.tensor_reduce(
                    slocal[:], w4[:], axis=mybir.AxisListType.X, op=ALU.add
                )
                swr = mp.tile([128, 1], F32)
                nc.gpsimd.partition_all_reduce(
                    swr[:], slocal[:], 128, bass_isa.ReduceOp.add
                )
                # dnb: bf16 dists, ready at DTW end -- gates the PE warmup
                dnb = mp.tile([128, NT], BF16)
                nc.scalar.activation(dnb[:], down[:], ACTF.Copy)

            if stage >= 4:
                # ---- weighted sums as PE matmuls accumulating over tiles
                NTOT = 2 * T * A + 1
                arin = dp.tile([NTOT], F32)
                arout = dp.tile([NTOT], F32)
                nc.sync.dma_start(
                    arin[2 * T * A : NTOT].rearrange("(o f) -> o f", o=1),
                    swr[0:1, 0:1],
                )
                _pst_cm = tc.tile_pool(name="psum_st", bufs=1, space="PSUM")
                pst = _pst_cm.__enter__()
                sts = []
                for c in range(8):
                    st = pst.tile([128, 512], F32, tag=f"st{c}")
                    sts.append(st)
                # PE p-state warmup: junk matmuls gated on the dists; their
                # outputs are reset by the first start=True real matmul.
                for wi in range(WARM):
                    nc.tensor.matmul(
                        sts[wi % 8][0:1, :],
                        dnb[:, 0:1],
                        actT[:, wi % NT, (wi % 8) * 16 : (wi % 8) * 16 + 16, :],
                        start=True, stop=True, skip_group_check=True,
                    )
                # staging rows alias dead actb (32-aligned partitions)
                arsc = actb[:].rearrange("t p a -> t (p a)").bitcast(F32)
                arsb_m = arsc[0:1, 0 : T * A]
                arsb_s = arsc[32:33, 0 : T * A]
                for c in range(8):
                    for k in range(NT):
                        nc.tensor.matmul(
                            sts[c][0:1, :],
                            wb[:, k : k + 1],
                            actT[:, k, c * 16 : (c + 1) * 16, :],
                            start=(k == 0), stop=(k == NT - 1),
                        )
                    nc.scalar.activation(
                        arsb_m[:, c * 512 : (c + 1) * 512],
                        sts[c][0:1, :], ACTF.Copy,
                    )
                for c in range(8):
                    for k in range(NT):
                        nc.tensor.matmul(
                            sts[c][32:33, :],
                            wb[:, k : k + 1],
                            act2T[:, k, c * 16 : (c + 1) * 16, :],
                            start=(k == 0), stop=(k == NT - 1),
                        )
                    nc.vector.tensor_copy(
                        arsb_s[:, c * 512 : (c + 1) * 512],
                        sts[c][32:33, :],
                    )

                nc.sync.dma_start(
                    arin[0 : T * A].rearrange("(o f) -> o f", o=1), arsb_m[:]
                )
                nc.sync.dma_start(
                    arin[T * A : 2 * T * A].rearrange("(o f) -> o f", o=1),
                    arsb_s[:],
                )
                if single:
                    nc.sync.dma_start(arout[:], arin[:])
                else:
                    nc.gpsimd.collective_compute(
                        "AllReduce",
                        ALU.add,
                        replica_groups=GROUP,
                        ins=[arin.opt()],
                        outs=[arout.opt()],
                    )
                _pst_cm.__exit__(None, None, None)

            if stage >= 5:
                # ---- final statistics
                rn12 = mp.tile([128, 2, A], F32)
                nc.sync.dma_start(
                    rn12[:],
                    arout[0 : 2 * T * A].rearrange(
                        "(q t a) -> t q a", q=2, t=T
                    ),
                )
                rs = mp.tile([128, 1], F32)
                _, rssrc = bass.broadcast_tensor_aps(
                    rs[:],
                    arout[2 * T * A : NTOT].rearrange("(o f) -> o f", o=1),
                )
                nc.sync.dma_start(rs[:], rssrc)
                rn1 = rn12[:, 0]
                rn2 = rn12[:, 1]
                inv = mp.tile([128, 1], F32)
                nc.vector.reciprocal(inv[:], rs[:])
                mh = mp.tile([128, A], F32)
                nc.vector.tensor_scalar(
                    mh[:], rn1, inv[:, 0:1], None, op0=ALU.mult
                )
                q = mp.tile([128, A], F32)
                nc.vector.tensor_scalar(
                    q[:], rn2, inv[:, 0:1], None, op0=ALU.mult
                )
                msq = mp.tile([128, A], F32)
                nc.vector.tensor_tensor(msq[:], mh[:], mh[:], op=ALU.mult)
                var = mp.tile([128, A], F32)
                nc.vector.tensor_tensor(var[:], q[:], msq[:], op=ALU.subtract)
                nc.vector.tensor_scalar(var[:], var[:], 0.0, None, op0=ALU.max)
                stdv = mp.tile([128, A], F32)
                ostk = mp.tile([128, 2, A], F32)
                nc.scalar.sqrt(stdv[:], var[:])
                nc.vector.tensor_scalar(
                    ostk[:, 1], stdv[:], MIN_STD, 1.0, op0=ALU.max, op1=ALU.min
                )
                nc.vector.tensor_scalar(
                    mh[:], mh[:], 1.0 - MOM, None, op0=ALU.mult
                )
                nc.vector.scalar_tensor_tensor(
                    ostk[:, 0], means_t[:], MOM, mh[:], op0=ALU.mult,
                    op1=ALU.add,
                )
                nc.sync.dma_start(
                    out_d.rearrange("q t o a -> t (q o) a"), ostk[:]
                )
            else:
                # bisect debug output
                dbg = mp.tile([128, A], F32)
                nc.vector.memset(dbg[:], 0.0)
                nc.vector.tensor_copy(dbg[:, 0:NT], down[:])
                if stage >= 3:
                    nc.vector.tensor_copy(dbg[:, 4 : 4 + NT], w4[:])
                    nc.vector.tensor_copy(dbg[:, 8:9], thneg[:])
                    nc.vector.tensor_copy(dbg[:, 9:10], swr[:])
                if stage == 2:
                    gdbg = mp.tile([128, A], F32)
                    nc.sync.dma_start(
                        gdbg[:],
                        gd[0 : 128 * A].rearrange("(p a) -> p a", a=A),
                    )
                    nc.vector.tensor_copy(dbg[:, 4:8], gdbg[:, 0:4])
                nc.sync.dma_start(out_d[0, :, 0, :], dbg[:])
                nc.sync.dma_start(out_d[1, :, 0, :], dbg[:])

    nc.compile()
    return nc


def _get_nc(stage=None, single=None):
    if stage is None:
        stage = int(os.environ.get("CEM_STAGE", "9"))
    if single is None:
        single = bool(int(os.environ.get("CEM_SINGLE", "0")))
    key = ("nc", stage, single)
    if key not in _CACHE:
        _CACHE[key] = _build(stage, single)
    return _CACHE[key]


def kernel(**inputs):
    obs = np.ascontiguousarray(np.asarray(inputs["obs_diffs"], np.float32))
    means = np.ascontiguousarray(np.asarray(inputs["means"], np.float32))
    stds = np.ascontiguousarray(np.asarray(inputs["stds"], np.float32))
    noise = np.ascontiguousarray(np.asarray(inputs["noise"], np.float32))

    nc = _get_nc(stage=9, single=False)
    in_maps = []
    for c in range(NCORES):
        in_maps.append(
            {
                "obs": obs[c * PL : (c + 1) * PL],
                "means": means,
                "stds": stds,
                "noise": np.ascontiguousarray(noise[:, c * PL : (c + 1) * PL, :]),
            }
        )
    res = bass_utils.run_bass_kernel_spmd(
        nc, in_maps, core_ids=list(range(NCORES))
    )
    out = np.asarray(res.results[0]["out"], np.float32)
    return out.reshape(2, T, 1, A)


# revision 31
# speedup vs baseline: 1.0611x; 1.0037x over previous
"""CEM sampling kernel for Trainium2, 8-core SPMD (population sharded).

Per core (512 of 4096 members), one fused program:

  Window (overlapped with the 42MB obs+noise HBM stream, ~117us):
   - DTW min-plus DP entirely on DVE (the scan/min ops exist only
     there): two packed pair-chains [t0|sep|t1] and [t2|sep|t3], DP
     state in fp16 (2x-mode mins; the scan's carry is internally fp32
     and the f32 cost rows are never rounded, so only the stored row
     values quantize).  ~1.1us/row.
   - Actions: ACT computes bf16 act = means + stds*noise per action
     dim, Pool clips in bf16, PE transposes [t,p] blocks to a
     population-major bf16 layout, ACT copies PSUM->SBUF and squares.
  Tail (~35us): AllGather dists; top-K via the gpsimd kth_largest
     library op on the [128,32] negated global dists (exact K-th
     threshold, replaces rank compares and broadcasts); weights; the
     weighted mean / E[x^2] reductions as 64 bf16 PE matmuls (with a
     p-state warmup) accumulating in PSUM; AllReduce; closing stats.
"""

import os
import sys

for _p in ("/opt/trn_rl_repo", "/root/.axon_site/_ro/trn_rl_repo"):
    if _p not in sys.path:
        sys.path.insert(0, _p)

import numpy as np

import concourse.bass as bass
import concourse.bacc as bacc
import concourse.bass_isa as bass_isa
import concourse.tile as tile
from concourse import mybir
from concourse import bass_utils
from concourse.masks import make_identity

F32 = mybir.dt.float32
FP16 = mybir.dt.float16
BF16 = mybir.dt.bfloat16
ALU = mybir.AluOpType
ACTF = mybir.ActivationFunctionType

P, T, A = 4096, 128, 32
NCORES = 8
PL = P // NCORES          # 512 population per core
NT = PL // 128            # 4 tiles of 128 on the partition dim
K = int(P * 0.1)          # 409
TEMP, MOM, MIN_STD = 0.5, 0.1, 0.05
INFDP = 30000.0           # fp16-safe stand-in for +inf in the DP
RCH = int(os.environ.get("CEM_RCH", "8"))   # DP rows per streamed chunk
CROWS = [RCH] * (T // RCH)                  # chunk row counts
COFF = [sum(CROWS[:i]) for i in range(len(CROWS))]
NCHUNK = len(CROWS)
CBUFS = int(os.environ.get("CEM_CBUFS", "4"))
WARM = int(os.environ.get("CEM_WARM", "8"))  # PE p-state warmup matmuls
DPDT = FP16 if os.environ.get("CEM_DPDT", "fp16") == "fp16" else F32
R16 = int(os.environ.get("CEM_R16", "64"))  # rows in fp16 before f32
if DPDT == F32:
    R16 = 0
GROUP = [list(range(NCORES))]

# packed cost-row layout: [t0(128) sep t1(128) | t2(128) sep t3(128)]
CW = 257                  # cost width of one pair-chain
CWF = 514
SEP1, SEP2 = 128, 385
DMAP = {0: 0, 1: 129, 2: 257, 3: 386}  # pop tile -> flat cost column

_CACHE = {}


def _build(stage=9, single=False):
    nc = bacc.Bacc(
        "TRN2",
        target_bir_lowering=False,
        debug=False,
        num_devices=1 if single else NCORES,
    )
    obs_d = nc.dram_tensor("obs", [PL, T, T], F32, kind="ExternalInput")
    means_d = nc.dram_tensor("means", [T, 1, A], F32, kind="ExternalInput")
    stds_d = nc.dram_tensor("stds", [T, 1, A], F32, kind="ExternalInput")
    noise_d = nc.dram_tensor("noise", [T, PL, A], F32, kind="ExternalInput")
    out_d = nc.dram_tensor("out", [2, T, 1, A], F32, kind="ExternalOutput")

    with tile.TileContext(nc) as tc:
        with (
            tc.tile_pool(name="main", bufs=1) as mp,
            tc.tile_pool(name="dram", bufs=1, space="DRAM") as dp,
        ):
            # ---- small persistent tiles
            means_t = mp.tile([T, A], F32)
            stds_t = mp.tile([T, A], F32)
            nc.sync.dma_start(means_t[:], means_d[:, 0, :])
            nc.sync.dma_start(stds_t[:], stds_d[:, 0, :])
            ident = mp.tile([128, 128], BF16)
            make_identity(nc, ident[:])
            # preload the ACT function tables used in the tail
            warmt = mp.tile([128, 1], F32)
            nc.scalar.activation(warmt[:], means_t[:, 0:1], ACTF.Exp)
            nc.scalar.sqrt(warmt[:], warmt[:])

            # actions (bf16), noise staging quarters, transposed layouts
            actb = mp.tile([T, PL, A], BF16)
            utile = mp.tile([128, 2 * PL * A // 4], F32)  # [128, 8192]
            nhq = [
                utile[:, 0:4096].rearrange("t (p a) -> t p a", a=A),
                utile[:, 4096:8192].rearrange("t (p a) -> t p a", a=A),
            ]
            actT = mp.tile([128, NT, T, A], BF16)
            act2T = mp.tile([128, NT, T, A], BF16)

            # ---- DTW state: two packed pair-chains, ping-pong.  Rows
            # < R16 keep the DP values in fp16 (2x-mode mins); the last
            # rows -- where the absolute values and hence fp16 quanta are
            # largest -- run in f32 so the accumulated rounding stays small.
            h01a = mp.tile([128, CW + 1], FP16)
            h01b = mp.tile([128, CW + 1], FP16)
            h23a = mp.tile([128, CW + 1], FP16)
            h23b = mp.tile([128, CW + 1], FP16)
            f01a = mp.tile([128, CW + 1], F32)
            f01b = mp.tile([128, CW + 1], F32)
            f23a = mp.tile([128, CW + 1], F32)
            f23b = mp.tile([128, CW + 1], F32)
            ub01 = mp.tile([128, CW], FP16)
            ub23 = mp.tile([128, CW], FP16)
            uf01 = mp.tile([128, CW], F32)
            uf23 = mp.tile([128, CW], F32)
            for t_ in (h01a, h01b, h23a, h23b, f01a, f01b, f23a, f23b):
                nc.vector.memset(t_[:], INFDP)
            # D[0][0] = 0 for each tile (pair cols 0 and 129)
            nc.vector.memset(h01a[:, 0:1], 0.0)
            nc.vector.memset(h01a[:, 129:130], 0.0)
            nc.vector.memset(h23a[:, 0:1], 0.0)
            nc.vector.memset(h23a[:, 129:130], 0.0)
            down = mp.tile([128, NT], F32)
            ch01 = (h01a, h01b)
            ch23 = (h23a, h23b)
            cf01 = (f01a, f01b)
            cf23 = (f23a, f23b)

            def dtw_row(i, cb, r):
                crow = cb[:, r]
                # row i reads the side written at i-1: fp16 through row R16,
                # f32 after; the switch row reads fp16 and writes f32.  The
                # f32 pair's col 0 is INFDP from init and never rewritten.
                A1 = (ch01 if i <= R16 else cf01)[i % 2]
                A2 = (ch23 if i <= R16 else cf23)[i % 2]
                if i < R16:
                    B1, B2, u1, u2 = (
                        ch01[(i + 1) % 2], ch23[(i + 1) % 2], ub01, ub23)
                else:
                    B1, B2, u1, u2 = (
                        cf01[(i + 1) % 2], cf23[(i + 1) % 2], uf01, uf23)
                nc.vector.tensor_tensor(
                    u1[:], A1[:, 0:CW], A1[:, 1 : CW + 1], op=ALU.min
                )
                nc.vector.tensor_tensor(
                    u2[:], A2[:, 0:CW], A2[:, 1 : CW + 1], op=ALU.min
                )
                nc.vector.tensor_tensor_scan(
                    B1[:, 1 : CW + 1], u1[:], crow[:, 0:CW], INFDP,
                    op0=ALU.min, op1=ALU.add,
                )
                nc.vector.tensor_tensor_scan(
                    B2[:, 1 : CW + 1], u2[:], crow[:, CW:CWF], INFDP,
                    op0=ALU.min, op1=ALU.add,
                )
                if i == 0:
                    # D[i>0][0] = INF at the never-rewritten left columns
                    nc.vector.memset(h01a[:, 0:1], INFDP)
                    nc.vector.memset(h23a[:, 0:1], INFDP)

            # ---- actions pipeline pieces (emitted interleaved with DTW)
            def noise_dma(q):
                nc.sync.dma_start(
                    nhq[q % 2][:], noise_d[:, q * 128 : (q + 1) * 128, :]
                )

            def affine(q):
                for a in range(A):
                    nc.scalar.activation(
                        actb[:, q * 128 : (q + 1) * 128, a],
                        nhq[q % 2][:, :, a],
                        ACTF.Identity,
                        bias=means_t[:, a : a + 1],
                        scale=stds_t[:, a : a + 1],
                    )

            def clip(k):
                v = actb[:, k * 128 : (k + 1) * 128, :].rearrange(
                    "t p a -> t (p a)"
                )
                nc.gpsimd.tensor_scalar(
                    v, v, 1.0, -1.0, op0=ALU.min, op1=ALU.max
                )

            def transposes(tpp, k):
                for a in range(A):
                    pt = tpp.tile([128, 128], BF16, tag="tp")
                    nc.tensor.transpose(
                        pt[:],
                        actb[:, k * 128 : (k + 1) * 128, a],
                        ident[:],
                    )
                    nc.scalar.activation(
                        actT[:, k, :, a], pt[:], ACTF.Copy
                    )

            def square(k):
                nc.scalar.activation(
                    act2T[:, k].rearrange("t a b -> t (a b)"),
                    actT[:, k].rearrange("t a b -> t (a b)"),
                    ACTF.Square,
                )

            # ---- window: obs chunks + DTW rows + action stages
            with tc.tile_pool(name="cwin", bufs=CBUFS) as cp, \
                 tc.tile_pool(name="psum_tp", bufs=4, space="PSUM") as tpp:
                cbs = []

                def chunk_dma(c):
                    rows = CROWS[c]
                    cb = cp.tile([128, RCH, CWF], F32, tag="cw")
                    for k in range(NT):
                        o = DMAP[k]
                        nc.sync.dma_start(
                            cb[:, 0:rows, o : o + T],
                            obs_d[k * 128 : (k + 1) * 128,
                                  COFF[c] : COFF[c] + rows, :],
                        )
                    # refresh both INF separators each generation
                    nc.gpsimd.memset(cb[:, 0:rows, SEP1 : SEP1 + 1], INFDP)
                    nc.gpsimd.memset(cb[:, 0:rows, SEP2 : SEP2 + 1], INFDP)
                    return cb

                cbs.append(chunk_dma(0))
                noise_dma(0)
                for c in range(1, min(CBUFS, NCHUNK)):
                    cbs.append(chunk_dma(c))

                acts = {
                    0: [lambda: affine(0), lambda: noise_dma(1)],
                    1: [lambda: affine(1), lambda: noise_dma(2)],
                    2: [lambda: clip(0)],
                    3: [lambda: affine(2), lambda: noise_dma(3),
                        lambda: clip(1)],
                    4: [lambda: transposes(tpp, 0)],
                    5: [lambda: affine(3), lambda: clip(2)],
                    6: [lambda: transposes(tpp, 1), lambda: square(0)],
                    7: [lambda: clip(3)],
                    8: [lambda: transposes(tpp, 2), lambda: square(1)],
                    9: [lambda: transposes(tpp, 3)],
                    10: [lambda: square(2)],
                    11: [lambda: square(3)],
                }

                next_key = 0
                for c in range(NCHUNK):
                    cb = cbs[c]
                    for r in range(CROWS[c]):
                        dtw_row(COFF[c] + r, cb, r)
                    if c + CBUFS < NCHUNK:
                        cbs.append(chunk_dma(c + CBUFS))
                    if stage >= 1:
                        # acts keyed by 8-row octiles of emitted DP rows
                        done = COFF[c] + CROWS[c]
                        while next_key * 8 + 8 <= done:
                            for th in acts.get(next_key, []):
                                th()
                            next_key += 1

            # own dists from the final (even-side) f32 buffers
            nc.scalar.activation(down[:, 0:1], f01a[:, 128:129], ACTF.Copy)
            nc.scalar.activation(down[:, 1:2], f01a[:, 257:258], ACTF.Copy)
            nc.scalar.activation(down[:, 2:3], f23a[:, 128:129], ACTF.Copy)
            nc.scalar.activation(down[:, 3:4], f23a[:, 257:258], ACTF.Copy)

            if stage >= 2:
                # ---- AllGather dists (tiny)
                ld = dp.tile([PL], F32)
                gd = dp.tile([P], F32)
                # member order in gd is irrelevant (kth/threshold are
                # order-free), so write ld partition-major: fewer descs
                nc.sync.dma_start(ld.rearrange("(p k) -> p k", k=NT), down[:])
                if single:
                    _, lsrc = bass.broadcast_tensor_aps(
                        gd.rearrange("(r f) -> r f", r=NCORES),
                        ld.rearrange("(o f) -> o f", o=1),
                    )
                    nc.sync.dma_start(
                        gd.rearrange("(r f) -> r f", r=NCORES), lsrc
                    )
                else:
                    nc.gpsimd.collective_compute(
                        "AllGather",
                        ALU.bypass,
                        replica_groups=GROUP,
                        ins=[ld.opt()],
                        outs=[gd.opt()],
                    )

            if stage >= 3:
                # ---- top-K threshold via gpsimd kth_largest on -dists
                gdsq = mp.tile([128, P // 128], F32)
                nc.sync.dma_start(
                    gdsq[:], gd.rearrange("(p f) -> p f", p=128)
                )
                ngd = mp.tile([128, P // 128], F32)
                nc.vector.tensor_scalar(
                    ngd[:], gdsq[:], -1.0, None, op0=ALU.mult
                )
                kth = mp.tile([128, 2], F32)
                nc.gpsimd.kth_largest(
                    kth[:], ngd[:], P // 128, K + 3,
                    quantile=1.0 - (K - 0.5) / (P - 1),
                )
                # kth col1 = desc[k_adj+1] = -s[K] ; mask = d < s[K]
                thb = mp.tile([128, 2], F32)
                nc.gpsimd.partition_broadcast(thb[:], kth[0:1, :])
                thneg = mp.tile([128, 1], F32)
                nc.vector.tensor_scalar(
                    thneg[:], thb[:, 1:2], -1.0, None, op0=ALU.mult
                )
                # softmax shift: any global constant cancels exactly; use
                # gd[0] (safe unless the dist spread nears 176/TEMP).
                dref = mp.tile([128, 1], F32)
                nc.gpsimd.partition_broadcast(dref[:], gdsq[0:1, 0:1])
                biast = mp.tile([128, 1], F32)
                nc.gpsimd.tensor_scalar(
                    biast[:], dref[:], TEMP, None, op0=ALU.mult
                )
                mask4 = mp.tile([128, NT], F32)
                nc.vector.tensor_scalar(
                    mask4[:], down[:], thneg[:, 0:1], None, op0=ALU.is_lt
                )
                e4 = mp.tile([128, NT], F32)
                nc.scalar.activation(
                    e4[:], down[:], ACTF.Exp, bias=biast[:, 0:1], scale=-TEMP
                )
                w4 = mp.tile([128, NT], F32)
                nc.vector.tensor_tensor(w4[:], e4[:], mask4[:], op=ALU.mult)
                wb = mp.tile([128, NT], BF16)
                nc.scalar.activation(wb[:], w4[:], ACTF.Copy)
                # sum of weights across members (free then partitions)
                slocal = mp.tile([128, 1], F32)
                nc.vector.tensor_reduce(
                    slocal[:], w4[:], axis=mybir.AxisListType.X, op=ALU.add
                )
                swr = mp.tile([128, 1], F32)
                nc.gpsimd.partition_all_reduce(
                    swr[:], slocal[:], 128, bass_isa.ReduceOp.add
                )
                # dnb: bf16 dists, ready at DTW end -- gates the PE warmup
                dnb = mp.tile([128, NT], BF16)
                nc.scalar.activation(dnb[:], down[:], ACTF.Copy)

            if stage >= 4:
                # ---- weighted sums as PE matmuls accumulating over tiles
                NTOT = 2 * T * A + 1
                arin = dp.tile([NTOT], F32)
                arout = dp.tile([NTOT], F32)
                nc.sync.dma_start(
                    arin[2 * T * A : NTOT].rearrange("(o f) -> o f", o=1),
                    swr[0:1, 0:1],
                )
                _pst_cm = tc.tile_pool(name="psum_st", bufs=1, space="PSUM")
                pst = _pst_cm.__enter__()
                sts = []
                for c in range(8):
                    st = pst.tile([128, 512], F32, tag=f"st{c}")
                    sts.append(st)
                # PE p-state warmup: junk matmuls gated on the dists; their
                # outputs are reset by the first start=True real matmul.
                for wi in range(WARM):
                    nc.tensor.matmul(
                        sts[wi % 8][0:1, :],
                        dnb[:, 0:1],
                        actT[:, wi % NT, (wi % 8) * 16 : (wi % 8) * 16 + 16, :],
                        start=True, stop=True, skip_group_check=True,
                    )
                # staging rows alias dead actb (32-aligned partitions)
                arsc = actb[:].rearrange("t p a -> t (p a)").bitcast(F32)
                arsb_m = arsc[0:1, 0 : T * A]
                arsb_s = arsc[32:33, 0 : T * A]
                for c in range(8):
                    for k in range(NT):
                        nc.tensor.matmul(
                            sts[c][0:1, :],
                            wb[:, k : k + 1],
                            actT[:, k, c * 16 : (c + 1) * 16, :],
                            start=(k == 0), stop=(k == NT - 1),
                        )
                    nc.scalar.activation(
                        arsb_m[:, c * 512 : (c + 1) * 512],
                        sts[c][0:1, :], ACTF.Copy,
                    )
                for c in range(8):
                    for k in range(NT):
                        nc.tensor.matmul(
                            sts[c][32:33, :],
                            wb[:, k : k + 1],
                            act2T[:, k, c * 16 : (c + 1) * 16, :],
                            start=(k == 0), stop=(k == NT - 1),
                        )
                    nc.vector.tensor_copy(
                        arsb_s[:, c * 512 : (c + 1) * 512],
                        sts[c][32:33, :],
                    )

                nc.sync.dma_start(
                    arin[0 : T * A].rearrange("(o f) -> o f", o=1), arsb_m[:]
                )
                nc.sync.dma_start(
                    arin[T * A : 2 * T * A].rearrange("(o f) -> o f", o=1),
                    arsb_s[:],
                )
                if single:
                    nc.sync.dma_start(arout[:], arin[:])
                else:
                    nc.gpsimd.collective_compute(
                        "AllReduce",
                        ALU.add,
                        replica_groups=GROUP,
                        ins=[arin.opt()],
                        outs=[arout.opt()],
                    )
                _pst_cm.__exit__(None, None, None)

            if stage >= 5:
                # ---- final statistics
                rn12 = mp.tile([128, 2, A], F32)
                nc.sync.dma_start(
                    rn12[:],
                    arout[0 : 2 * T * A].rearrange(
                        "(q t a) -> t q a", q=2, t=T
                    ),
                )
                rs = mp.tile([128, 1], F32)
                _, rssrc = bass.broadcast_tensor_aps(
                    rs[:],
                    arout[2 * T * A : NTOT].rearrange("(o f) -> o f", o=1),
                )
                nc.sync.dma_start(rs[:], rssrc)
                rn1 = rn12[:, 0]
                rn2 = rn12[:, 1]
                inv = mp.tile([128, 1], F32)
                nc.vector.reciprocal(inv[:], rs[:])
                mh = mp.tile([128, A], F32)
                nc.vector.tensor_scalar(
                    mh[:], rn1, inv[:, 0:1], None, op0=ALU.mult
                )
                q = mp.tile([128, A], F32)
                nc.vector.tensor_scalar(
                    q[:], rn2, inv[:, 0:1], None, op0=ALU.mult
                )
                msq = mp.tile([128, A], F32)
                nc.vector.tensor_tensor(msq[:], mh[:], mh[:], op=ALU.mult)
                var = mp.tile([128, A], F32)
                nc.vector.tensor_tensor(var[:], q[:], msq[:], op=ALU.subtract)
                nc.vector.tensor_scalar(var[:], var[:], 0.0, None, op0=ALU.max)
                stdv = mp.tile([128, A], F32)
                ostk = mp.tile([128, 2, A], F32)
                nc.scalar.sqrt(stdv[:], var[:])
                nc.vector.tensor_scalar(
                    ostk[:, 1], stdv[:], MIN_STD, 1.0, op0=ALU.max, op1=ALU.min
                )
                nc.vector.tensor_scalar(
                    mh[:], mh[:], 1.0 - MOM, None, op0=ALU.mult
                )
                nc.vector.scalar_tensor_tensor(
                    ostk[:, 0], means_t[:], MOM, mh[:], op0=ALU.mult,
                    op1=ALU.add,
                )
                nc.sync.dma_start(
                    out_d.rearrange("q t o a -> t (q o) a"), ostk[:]
                )
            else:
                # bisect debug output
                dbg = mp.tile([128, A], F32)
                nc.vector.memset(dbg[:], 0.0)
                nc.vector.tensor_copy(dbg[:, 0:NT], down[:])
                if stage >= 3:
                    nc.vector.tensor_copy(dbg[:, 4 : 4 + NT], w4[:])
                    nc.vector.tensor_copy(dbg[:, 8:9], thneg[:])
                    nc.vector.tensor_copy(dbg[:, 9:10], swr[:])
                if stage == 2:
                    gdbg = mp.tile([128, A], F32)
                    nc.sync.dma_start(
                        gdbg[:],
                        gd[0 : 128 * A].rearrange("(p a) -> p a", a=A),
                    )
                    nc.vector.tensor_copy(dbg[:, 4:8], gdbg[:, 0:4])
                nc.sync.dma_start(out_d[0, :, 0, :], dbg[:])
                nc.sync.dma_start(out_d[1, :, 0, :], dbg[:])

    nc.compile()
    return nc


def _get_nc(stage=None, single=None):
    if stage is None:
        stage = int(os.environ.get("CEM_STAGE", "9"))
    if single is None:
        single = bool(int(os.environ.get("CEM_SINGLE", "0")))
    key = ("nc", stage, single)
    if key not in _CACHE:
        _CACHE[key] = _build(stage, single)
    return _CACHE[key]


def kernel(**inputs):
    obs = np.ascontiguousarray(np.asarray(inputs["obs_diffs"], np.float32))
    means = np.ascontiguousarray(np.asarray(inputs["means"], np.float32))
    stds = np.ascontiguousarray(np.asarray(inputs["stds"], np.float32))
    noise = np.ascontiguousarray(np.asarray(inputs["noise"], np.float32))

    nc = _get_nc(stage=9, single=False)
    in_maps = []
    for c in range(NCORES):
        in_maps.append(
            {
                "obs": obs[c * PL : (c + 1) * PL],
                "means": means,
                "stds": stds,
                "noise": np.ascontiguousarray(noise[:, c * PL : (c + 1) * PL, :]),
            }
        )
    res = bass_utils.run_bass_kernel_spmd(
        nc, in_maps, core_ids=list(range(NCORES))
    )
    out = np.asarray(res.results[0]["out"], np.float32)
    return out.reshape(2, T, 1, A)


# revision 35
# speedup vs baseline: 1.0726x; 1.0108x over previous
"""CEM sampling kernel for Trainium2, 8-core SPMD (population sharded).

Per core (512 of 4096 members), one fused program:

  Window (overlapped with the 42MB obs+noise HBM stream, ~117us):
   - DTW min-plus DP entirely on DVE (the scan/min ops exist only
     there): two packed pair-chains [t0|sep|t1] and [t2|sep|t3], DP
     state in fp16 (2x-mode mins; the scan's carry is internally fp32
     and the f32 cost rows are never rounded, so only the stored row
     values quantize).  ~1.1us/row.
   - Actions: ACT computes bf16 act = means + stds*noise per action
     dim, Pool clips in bf16, PE transposes [t,p] blocks to a
     population-major bf16 layout, ACT copies PSUM->SBUF and squares.
  Tail (~35us): AllGather dists; top-K via the gpsimd kth_largest
     library op on the [128,32] negated global dists (exact K-th
     threshold, replaces rank compares and broadcasts); weights; the
     weighted mean / E[x^2] reductions as 64 bf16 PE matmuls (with a
     p-state warmup) accumulating in PSUM; AllReduce; closing stats.
"""

import os
import sys

for _p in ("/opt/trn_rl_repo", "/root/.axon_site/_ro/trn_rl_repo"):
    if _p not in sys.path:
        sys.path.insert(0, _p)

import numpy as np

import concourse.bass as bass
import concourse.bacc as bacc
import concourse.bass_isa as bass_isa
import concourse.tile as tile
from concourse import mybir
from concourse import bass_utils
from concourse.masks import make_identity

F32 = mybir.dt.float32
FP16 = mybir.dt.float16
BF16 = mybir.dt.bfloat16
ALU = mybir.AluOpType
ACTF = mybir.ActivationFunctionType

P, T, A = 4096, 128, 32
NCORES = 8
PL = P // NCORES          # 512 population per core
NT = PL // 128            # 4 tiles of 128 on the partition dim
K = int(P * 0.1)          # 409
TEMP, MOM, MIN_STD = 0.5, 0.1, 0.05
INFDP = 30000.0           # fp16-safe stand-in for +inf in the DP
RCH = int(os.environ.get("CEM_RCH", "8"))   # DP rows per streamed chunk
_C0 = int(os.environ.get("CEM_C0", "4"))    # optional small first chunk
CROWS = ([_C0, RCH - _C0] if _C0 else []) + [RCH] * ((T - (RCH if _C0 else 0)) // RCH)
COFF = [sum(CROWS[:i]) for i in range(len(CROWS))]
NCHUNK = len(CROWS)
CBUFS = int(os.environ.get("CEM_CBUFS", "4"))
WARM = int(os.environ.get("CEM_WARM", "8"))  # PE p-state warmup matmuls
WARM2 = int(os.environ.get("CEM_WARM2", "5"))  # late warmups gated on gdsq
DPDT = FP16 if os.environ.get("CEM_DPDT", "fp16") == "fp16" else F32
R16 = int(os.environ.get("CEM_R16", "64"))  # rows in fp16 before f32
if DPDT == F32:
    R16 = 0
GROUP = [list(range(NCORES))]

# packed cost-row layout: [t0(128) sep t1(128) | t2(128) sep t3(128)]
CW = 257                  # cost width of one pair-chain
CWF = 514
SEP1, SEP2 = 128, 385
DMAP = {0: 0, 1: 129, 2: 257, 3: 386}  # pop tile -> flat cost column

_CACHE = {}


def _build(stage=9, single=False):
    nc = bacc.Bacc(
        "TRN2",
        target_bir_lowering=False,
        debug=False,
        num_devices=1 if single else NCORES,
    )
    obs_d = nc.dram_tensor("obs", [PL, T, T], F32, kind="ExternalInput")
    means_d = nc.dram_tensor("means", [T, 1, A], F32, kind="ExternalInput")
    stds_d = nc.dram_tensor("stds", [T, 1, A], F32, kind="ExternalInput")
    noise_d = nc.dram_tensor("noise", [T, PL, A], F32, kind="ExternalInput")
    out_d = nc.dram_tensor("out", [2, T, 1, A], F32, kind="ExternalOutput")

    with tile.TileContext(nc) as tc:
        with (
            tc.tile_pool(name="main", bufs=1) as mp,
            tc.tile_pool(name="dram", bufs=1, space="DRAM") as dp,
        ):
            # ---- small persistent tiles
            means_t = mp.tile([T, A], F32)
            stds_t = mp.tile([T, A], F32)
            nc.sync.dma_start(means_t[:], means_d[:, 0, :])
            nc.sync.dma_start(stds_t[:], stds_d[:, 0, :])
            ident = mp.tile([128, 128], BF16)
            make_identity(nc, ident[:])
            # preload the ACT function tables used in the tail
            warmt = mp.tile([128, 1], F32)
            nc.scalar.activation(warmt[:], means_t[:, 0:1], ACTF.Exp)
            nc.scalar.sqrt(warmt[:], warmt[:])

            # actions (bf16), noise staging quarters, transposed layouts
            actb = mp.tile([T, PL, A], BF16)
            utile = mp.tile([128, 2 * PL * A // 4], F32)  # [128, 8192]
            nhq = [
                utile[:, 0:4096].rearrange("t (p a) -> t p a", a=A),
                utile[:, 4096:8192].rearrange("t (p a) -> t p a", a=A),
            ]
            actT = mp.tile([128, NT, T, A], BF16)
            act2T = mp.tile([128, NT, T, A], BF16)

            # ---- DTW state: two packed pair-chains, ping-pong.  Rows
            # < R16 keep the DP values in fp16 (2x-mode mins); the last
            # rows -- where the absolute values and hence fp16 quanta are
            # largest -- run in f32 so the accumulated rounding stays small.
            h01a = mp.tile([128, CW + 1], FP16)
            h01b = mp.tile([128, CW + 1], FP16)
            h23a = mp.tile([128, CW + 1], FP16)
            h23b = mp.tile([128, CW + 1], FP16)
            f01a = mp.tile([128, CW + 1], F32)
            f01b = mp.tile([128, CW + 1], F32)
            f23a = mp.tile([128, CW + 1], F32)
            f23b = mp.tile([128, CW + 1], F32)
            ub01 = mp.tile([128, CW], FP16)
            ub23 = mp.tile([128, CW], FP16)
            uf01 = mp.tile([128, CW], F32)
            uf23 = mp.tile([128, CW], F32)
            for t_ in (h01a, h01b, h23a, h23b, f01a, f01b, f23a, f23b):
                nc.vector.memset(t_[:], INFDP)
            # D[0][0] = 0 for each tile (pair cols 0 and 129)
            nc.vector.memset(h01a[:, 0:1], 0.0)
            nc.vector.memset(h01a[:, 129:130], 0.0)
            nc.vector.memset(h23a[:, 0:1], 0.0)
            nc.vector.memset(h23a[:, 129:130], 0.0)
            down = mp.tile([128, NT], F32)
            ch01 = (h01a, h01b)
            ch23 = (h23a, h23b)
            cf01 = (f01a, f01b)
            cf23 = (f23a, f23b)

            def dtw_row(i, cb, r):
                crow = cb[:, r]
                # row i reads the side written at i-1: fp16 through row R16,
                # f32 after; the switch row reads fp16 and writes f32.  The
                # f32 pair's col 0 is INFDP from init and never rewritten.
                A1 = (ch01 if i <= R16 else cf01)[i % 2]
                A2 = (ch23 if i <= R16 else cf23)[i % 2]
                if i < R16:
                    B1, B2, u1, u2 = (
                        ch01[(i + 1) % 2], ch23[(i + 1) % 2], ub01, ub23)
                else:
                    B1, B2, u1, u2 = (
                        cf01[(i + 1) % 2], cf23[(i + 1) % 2], uf01, uf23)
                nc.vector.tensor_tensor(
                    u1[:], A1[:, 0:CW], A1[:, 1 : CW + 1], op=ALU.min
                )
                nc.vector.tensor_tensor(
                    u2[:], A2[:, 0:CW], A2[:, 1 : CW + 1], op=ALU.min
                )
                nc.vector.tensor_tensor_scan(
                    B1[:, 1 : CW + 1], u1[:], crow[:, 0:CW], INFDP,
                    op0=ALU.min, op1=ALU.add,
                )
                nc.vector.tensor_tensor_scan(
                    B2[:, 1 : CW + 1], u2[:], crow[:, CW:CWF], INFDP,
                    op0=ALU.min, op1=ALU.add,
                )
                if i == 0:
                    # D[i>0][0] = INF at the never-rewritten left columns
                    nc.vector.memset(h01a[:, 0:1], INFDP)
                    nc.vector.memset(h23a[:, 0:1], INFDP)

            # ---- actions pipeline pieces (emitted interleaved with DTW)
            def noise_dma(q):
                nc.sync.dma_start(
                    nhq[q % 2][:], noise_d[:, q * 128 : (q + 1) * 128, :]
                )

            def affine(q):
                for a in range(A):
                    nc.scalar.activation(
                        actb[:, q * 128 : (q + 1) * 128, a],
                        nhq[q % 2][:, :, a],
                        ACTF.Identity,
                        bias=means_t[:, a : a + 1],
                        scale=stds_t[:, a : a + 1],
                    )

            def clip(k):
                v = actb[:, k * 128 : (k + 1) * 128, :].rearrange(
                    "t p a -> t (p a)"
                )
                nc.gpsimd.tensor_scalar(
                    v, v, 1.0, -1.0, op0=ALU.min, op1=ALU.max
                )

            def transposes(tpp, k):
                for a in range(A):
                    pt = tpp.tile([128, 128], BF16, tag="tp")
                    nc.tensor.transpose(
                        pt[:],
                        actb[:, k * 128 : (k + 1) * 128, a],
                        ident[:],
                    )
                    nc.scalar.activation(
                        actT[:, k, :, a], pt[:], ACTF.Copy
                    )

            def square(k):
                nc.scalar.activation(
                    act2T[:, k].rearrange("t a b -> t (a b)"),
                    actT[:, k].rearrange("t a b -> t (a b)"),
                    ACTF.Square,
                )

            # ---- window: obs chunks + DTW rows + action stages
            with tc.tile_pool(name="cwin", bufs=CBUFS) as cp, \
                 tc.tile_pool(name="psum_tp", bufs=4, space="PSUM") as tpp:
                cbs = []

                def chunk_dma(c):
                    rows = CROWS[c]
                    cb = cp.tile([128, RCH, CWF], F32, tag="cw")
                    for k in range(NT):
                        o = DMAP[k]
                        nc.sync.dma_start(
                            cb[:, 0:rows, o : o + T],
                            obs_d[k * 128 : (k + 1) * 128,
                                  COFF[c] : COFF[c] + rows, :],
                        )
                    # refresh both INF separators each generation
                    nc.gpsimd.memset(cb[:, 0:rows, SEP1 : SEP1 + 1], INFDP)
                    nc.gpsimd.memset(cb[:, 0:rows, SEP2 : SEP2 + 1], INFDP)
                    return cb

                # prime obs chunks ahead of the first noise quarter so
                # the DTW never starves during pipeline fill
                for c in range(min(3, CBUFS, NCHUNK)):
                    cbs.append(chunk_dma(c))
                noise_dma(0)
                for c in range(3, min(CBUFS, NCHUNK)):
                    cbs.append(chunk_dma(c))

                acts = {
                    0: [lambda: affine(0), lambda: noise_dma(1)],
                    1: [lambda: affine(1), lambda: noise_dma(2)],
                    2: [lambda: clip(0)],
                    3: [lambda: affine(2), lambda: noise_dma(3),
                        lambda: clip(1)],
                    4: [lambda: transposes(tpp, 0)],
                    5: [lambda: affine(3), lambda: clip(2)],
                    6: [lambda: transposes(tpp, 1), lambda: square(0)],
                    7: [lambda: clip(3)],
                    8: [lambda: transposes(tpp, 2), lambda: square(1)],
                    9: [lambda: transposes(tpp, 3)],
                    10: [lambda: square(2)],
                    11: [lambda: square(3)],
                }

                next_key = 0
                for c in range(NCHUNK):
                    cb = cbs[c]
                    for r in range(CROWS[c]):
                        dtw_row(COFF[c] + r, cb, r)
                    if c + CBUFS < NCHUNK:
                        cbs.append(chunk_dma(c + CBUFS))
                    if stage >= 1:
                        # acts keyed by 8-row octiles of emitted DP rows
                        done = COFF[c] + CROWS[c]
                        while next_key * 8 + 8 <= done:
                            for th in acts.get(next_key, []):
                                th()
                            next_key += 1

            # own dists from the final (even-side) f32 buffers
            nc.scalar.activation(down[:, 0:1], f01a[:, 128:129], ACTF.Copy)
            nc.scalar.activation(down[:, 1:2], f01a[:, 257:258], ACTF.Copy)
            nc.scalar.activation(down[:, 2:3], f23a[:, 128:129], ACTF.Copy)
            nc.scalar.activation(down[:, 3:4], f23a[:, 257:258], ACTF.Copy)

            if stage >= 2:
                # ---- AllGather dists (tiny)
                ld = dp.tile([PL], F32)
                gd = dp.tile([P], F32)
                # member order in gd is irrelevant (kth/threshold are
                # order-free), so write ld partition-major: fewer descs
                nc.sync.dma_start(ld.rearrange("(p k) -> p k", k=NT), down[:])
                if single:
                    _, lsrc = bass.broadcast_tensor_aps(
                        gd.rearrange("(r f) -> r f", r=NCORES),
                        ld.rearrange("(o f) -> o f", o=1),
                    )
                    nc.sync.dma_start(
                        gd.rearrange("(r f) -> r f", r=NCORES), lsrc
                    )
                else:
                    nc.gpsimd.collective_compute(
                        "AllGather",
                        ALU.bypass,
                        replica_groups=GROUP,
                        ins=[ld.opt()],
                        outs=[gd.opt()],
                    )

            if stage >= 3:
                # ---- top-K threshold via gpsimd kth_largest on -dists
                gdsq = mp.tile([128, P // 128], F32)
                nc.sync.dma_start(
                    gdsq[:], gd.rearrange("(p f) -> p f", p=128)
                )
                ngd = mp.tile([128, P // 128], F32)
                nc.vector.tensor_scalar(
                    ngd[:], gdsq[:], -1.0, None, op0=ALU.mult
                )
                kth = mp.tile([128, 2], F32)
                nc.gpsimd.kth_largest(
                    kth[:], ngd[:], P // 128, K + 3,
                    quantile=1.0 - (K - 0.5) / (P - 1),
                )
                # kth col1 = desc[k_adj+1] = -s[K] ; mask = d < s[K]
                thb = mp.tile([128, 2], F32)
                nc.gpsimd.partition_broadcast(thb[:], kth[0:1, :])
                thneg = mp.tile([128, 1], F32)
                nc.vector.tensor_scalar(
                    thneg[:], thb[:, 1:2], -1.0, None, op0=ALU.mult
                )
                # softmax shift: any global constant cancels exactly; use
                # gd[0] (safe unless the dist spread nears 176/TEMP).
                dref = mp.tile([128, 1], F32)
                nc.gpsimd.partition_broadcast(dref[:], gdsq[0:1, 0:1])
                biast = mp.tile([128, 1], F32)
                nc.gpsimd.tensor_scalar(
                    biast[:], dref[:], TEMP, None, op0=ALU.mult
                )
                mask4 = mp.tile([128, NT], F32)
                nc.vector.tensor_scalar(
                    mask4[:], down[:], thneg[:, 0:1], None, op0=ALU.is_lt
                )
                e4 = mp.tile([128, NT], F32)
                nc.scalar.activation(
                    e4[:], down[:], ACTF.Exp, bias=biast[:, 0:1], scale=-TEMP
                )
                w4 = mp.tile([128, NT], F32)
                nc.vector.tensor_tensor(w4[:], e4[:], mask4[:], op=ALU.mult)
                wb = mp.tile([128, NT], BF16)
                nc.scalar.activation(wb[:], w4[:], ACTF.Copy)
                # sum of weights across members (free then partitions)
                slocal = mp.tile([128, 1], F32)
                nc.vector.tensor_reduce(
                    slocal[:], w4[:], axis=mybir.AxisListType.X, op=ALU.add
                )
                swr = mp.tile([128, 1], F32)
                nc.gpsimd.partition_all_reduce(
                    swr[:], slocal[:], 128, bass_isa.ReduceOp.add
                )
                # bf16 warmup gates: dnb ready at DTW end, gsb ready when
                # the gathered dists land (a few us before the weights)
                dnb = mp.tile([128, NT], BF16)
                nc.scalar.activation(dnb[:], down[:], ACTF.Copy)
                gsb = mp.tile([128, NT], BF16)
                nc.scalar.activation(gsb[:], gdsq[:, 0:NT], ACTF.Copy)

            if stage >= 4:
                # ---- weighted sums as PE matmuls accumulating over tiles
                NTOT = 2 * T * A + 1
                arin = dp.tile([NTOT], F32)
                arout = dp.tile([NTOT], F32)
                nc.sync.dma_start(
                    arin[2 * T * A : NTOT].rearrange("(o f) -> o f", o=1),
                    swr[0:1, 0:1],
                )
                _pst_cm = tc.tile_pool(name="psum_st", bufs=1, space="PSUM")
                pst = _pst_cm.__enter__()
                sts = []
                for c in range(8):
                    st = pst.tile([128, 512], F32, tag=f"st{c}")
                    sts.append(st)
                # PE p-state warmup: junk matmuls gated on the dists; their
                # outputs are reset by the first start=True real matmul.
                for wi in range(WARM + WARM2):
                    wsrc = dnb if wi < WARM else gsb
                    nc.tensor.matmul(
                        sts[wi % 8][0:1, :],
                        wsrc[:, 0:1],
                        actT[:, wi % NT, (wi % 8) * 16 : (wi % 8) * 16 + 16, :],
                        start=True, stop=True, skip_group_check=True,
                    )
                # staging rows alias dead actb (32-aligned partitions)
                arsc = actb[:].rearrange("t p a -> t (p a)").bitcast(F32)
                arsb_m = arsc[0:1, 0 : T * A]
                arsb_s = arsc[32:33, 0 : T * A]
                for c in range(8):
                    for k in range(NT):
                        nc.tensor.matmul(
                            sts[c][0:1, :],
                            wb[:, k : k + 1],
                            actT[:, k, c * 16 : (c + 1) * 16, :],
                            start=(k == 0), stop=(k == NT - 1),
                        )
                    nc.scalar.activation(
                        arsb_m[:, c * 512 : (c + 1) * 512],
                        sts[c][0:1, :], ACTF.Copy,
                    )
                for c in range(8):
                    for k in range(NT):
                        nc.tensor.matmul(
                            sts[c][32:33, :],
                            wb[:, k : k + 1],
                            act2T[:, k, c * 16 : (c + 1) * 16, :],
                            start=(k == 0), stop=(k == NT - 1),
                        )
                    nc.vector.tensor_copy(
                        arsb_s[:, c * 512 : (c + 1) * 512],
                        sts[c][32:33, :],
                    )

                nc.sync.dma_start(
                    arin[0 : T * A].rearrange("(o f) -> o f", o=1), arsb_m[:]
                )
                nc.sync.dma_start(
                    arin[T * A : 2 * T * A].rearrange("(o f) -> o f", o=1),
                    arsb_s[:],
                )
                if single:
                    nc.sync.dma_start(arout[:], arin[:])
                else:
                    nc.gpsimd.collective_compute(
                        "AllReduce",
                        ALU.add,
                        replica_groups=GROUP,
                        ins=[arin.opt()],
                        outs=[arout.opt()],
                    )
                _pst_cm.__exit__(None, None, None)

            if stage >= 5:
                # ---- final statistics
                rn12 = mp.tile([128, 2, A], F32)
                nc.sync.dma_start(
                    rn12[:],
                    arout[0 : 2 * T * A].rearrange(
                        "(q t a) -> t q a", q=2, t=T
                    ),
                )
                rs = mp.tile([128, 1], F32)
                _, rssrc = bass.broadcast_tensor_aps(
                    rs[:],
                    arout[2 * T * A : NTOT].rearrange("(o f) -> o f", o=1),
                )
                nc.sync.dma_start(rs[:], rssrc)
                rn1 = rn12[:, 0]
                rn2 = rn12[:, 1]
                inv = mp.tile([128, 1], F32)
                nc.vector.reciprocal(inv[:], rs[:])
                mh = mp.tile([128, A], F32)
                nc.vector.tensor_scalar(
                    mh[:], rn1, inv[:, 0:1], None, op0=ALU.mult
                )
                q = mp.tile([128, A], F32)
                nc.vector.tensor_scalar(
                    q[:], rn2, inv[:, 0:1], None, op0=ALU.mult
                )
                msq = mp.tile([128, A], F32)
                nc.vector.tensor_tensor(msq[:], mh[:], mh[:], op=ALU.mult)
                var = mp.tile([128, A], F32)
                nc.vector.tensor_tensor(var[:], q[:], msq[:], op=ALU.subtract)
                nc.vector.tensor_scalar(var[:], var[:], 0.0, None, op0=ALU.max)
                stdv = mp.tile([128, A], F32)
                ostk = mp.tile([128, 2, A], F32)
                nc.scalar.sqrt(stdv[:], var[:])
                nc.vector.tensor_scalar(
                    ostk[:, 1], stdv[:], MIN_STD, 1.0, op0=ALU.max, op1=ALU.min
                )
                nc.vector.tensor_scalar(
                    mh[:], mh[:], 1.0 - MOM, None, op0=ALU.mult
                )
                nc.vector.scalar_tensor_tensor(
                    ostk[:, 0], means_t[:], MOM, mh[:], op0=ALU.mult,
                    op1=ALU.add,
                )
                nc.sync.dma_start(
                    out_d.rearrange("q t o a -> t (q o) a"), ostk[:]
                )
            else:
                # bisect debug output
                dbg = mp.tile([128, A], F32)
                nc.vector.memset(dbg[:], 0.0)
                nc.vector.tensor_copy(dbg[:, 0:NT], down[:])
                if stage >= 3:
                    nc.vector.tensor_copy(dbg[:, 4 : 4 + NT], w4[:])
                    nc.vector.tensor_copy(dbg[:, 8:9], thneg[:])
                    nc.vector.tensor_copy(dbg[:, 9:10], swr[:])
                if stage == 2:
                    gdbg = mp.tile([128, A], F32)
                    nc.sync.dma_start(
                        gdbg[:],
                        gd[0 : 128 * A].rearrange("(p a) -> p a", a=A),
                    )
                    nc.vector.tensor_copy(dbg[:, 4:8], gdbg[:, 0:4])
                nc.sync.dma_start(out_d[0, :, 0, :], dbg[:])
                nc.sync.dma_start(out_d[1, :, 0, :], dbg[:])

    nc.compile()
    return nc


def _get_nc(stage=None, single=None):
    if stage is None:
        stage = int(os.environ.get("CEM_STAGE", "9"))
    if single is None:
        single = bool(int(os.environ.get("CEM_SINGLE", "0")))
    key = ("nc", stage, single)
    if key not in _CACHE:
        _CACHE[key] = _build(stage, single)
    return _CACHE[key]


def kernel(**inputs):
    obs = np.ascontiguousarray(np.asarray(inputs["obs_diffs"], np.float32))
    means = np.ascontiguousarray(np.asarray(inputs["means"], np.float32))
    stds = np.ascontiguousarray(np.asarray(inputs["stds"], np.float32))
    noise = np.ascontiguousarray(np.asarray(inputs["noise"], np.float32))

    nc = _get_nc(stage=9, single=False)
    in_maps = []
    for c in range(NCORES):
        in_maps.append(
            {
                "obs": obs[c * PL : (c + 1) * PL],
                "means": means,
                "stds": stds,
                "noise": np.ascontiguousarray(noise[:, c * PL : (c + 1) * PL, :]),
            }
        )
    res = bass_utils.run_bass_kernel_spmd(
        nc, in_maps, core_ids=list(range(NCORES))
    )
    out = np.asarray(res.results[0]["out"], np.float32)
    return out.reshape(2, T, 1, A)


# revision 36
# speedup vs baseline: 1.0808x; 1.0077x over previous
"""CEM sampling kernel for Trainium2, 8-core SPMD (population sharded).

Per core (512 of 4096 members), one fused program:

  Window (overlapped with the 42MB obs+noise HBM stream, ~117us):
   - DTW min-plus DP entirely on DVE (the scan/min ops exist only
     there): two packed pair-chains [t0|sep|t1] and [t2|sep|t3], DP
     state in fp16 (2x-mode mins; the scan's carry is internally fp32
     and the f32 cost rows are never rounded, so only the stored row
     values quantize).  ~1.1us/row.
   - Actions: ACT computes bf16 act = means + stds*noise per action
     dim, Pool clips in bf16, PE transposes [t,p] blocks to a
     population-major bf16 layout, ACT copies PSUM->SBUF and squares.
  Tail (~35us): AllGather dists; top-K via the gpsimd kth_largest
     library op on the [128,32] negated global dists (exact K-th
     threshold, replaces rank compares and broadcasts); weights; the
     weighted mean / E[x^2] reductions as 64 bf16 PE matmuls (with a
     p-state warmup) accumulating in PSUM; AllReduce; closing stats.
"""

import os
import sys

for _p in ("/opt/trn_rl_repo", "/root/.axon_site/_ro/trn_rl_repo"):
    if _p not in sys.path:
        sys.path.insert(0, _p)

import numpy as np

import concourse.bass as bass
import concourse.bacc as bacc
import concourse.bass_isa as bass_isa
import concourse.tile as tile
from concourse import mybir
from concourse import bass_utils
from concourse.masks import make_identity

F32 = mybir.dt.float32
FP16 = mybir.dt.float16
BF16 = mybir.dt.bfloat16
ALU = mybir.AluOpType
ACTF = mybir.ActivationFunctionType

P, T, A = 4096, 128, 32
NCORES = 8
PL = P // NCORES          # 512 population per core
NT = PL // 128            # 4 tiles of 128 on the partition dim
K = int(P * 0.1)          # 409
TEMP, MOM, MIN_STD = 0.5, 0.1, 0.05
INFDP = 30000.0           # fp16-safe stand-in for +inf in the DP
RCH = int(os.environ.get("CEM_RCH", "8"))   # DP rows per streamed chunk
_C0 = int(os.environ.get("CEM_C0", "4"))    # optional small first chunk
CROWS = ([_C0, RCH - _C0] if _C0 else []) + [RCH] * ((T - (RCH if _C0 else 0)) // RCH)
COFF = [sum(CROWS[:i]) for i in range(len(CROWS))]
NCHUNK = len(CROWS)
CBUFS = int(os.environ.get("CEM_CBUFS", "4"))
WARM = int(os.environ.get("CEM_WARM", "8"))  # PE p-state warmup matmuls
WARM2 = int(os.environ.get("CEM_WARM2", "5"))  # late warmups gated on gdsq
DPDT = FP16 if os.environ.get("CEM_DPDT", "fp16") == "fp16" else F32
R16 = int(os.environ.get("CEM_R16", "64"))  # rows in fp16 before f32
if DPDT == F32:
    R16 = 0
GROUP = [list(range(NCORES))]

# packed cost-row layout: [t0(128) sep t1(128) | t2(128) sep t3(128)]
CW = 257                  # cost width of one pair-chain
CWF = 514
SEP1, SEP2 = 128, 385
DMAP = {0: 0, 1: 129, 2: 257, 3: 386}  # pop tile -> flat cost column

_CACHE = {}


def _build(stage=9, single=False):
    nc = bacc.Bacc(
        "TRN2",
        target_bir_lowering=False,
        debug=False,
        num_devices=1 if single else NCORES,
    )
    obs_d = nc.dram_tensor("obs", [PL, T, T], F32, kind="ExternalInput")
    means_d = nc.dram_tensor("means", [T, 1, A], F32, kind="ExternalInput")
    stds_d = nc.dram_tensor("stds", [T, 1, A], F32, kind="ExternalInput")
    noise_d = nc.dram_tensor("noise", [T, PL, A], F32, kind="ExternalInput")
    out_d = nc.dram_tensor("out", [2, T, 1, A], F32, kind="ExternalOutput")

    with tile.TileContext(nc) as tc:
        with (
            tc.tile_pool(name="main", bufs=1) as mp,
            tc.tile_pool(name="dram", bufs=1, space="DRAM") as dp,
        ):
            # ---- small persistent tiles
            means_t = mp.tile([T, A], F32)
            stds_t = mp.tile([T, A], F32)
            nc.sync.dma_start(means_t[:], means_d[:, 0, :])
            nc.sync.dma_start(stds_t[:], stds_d[:, 0, :])
            ident = mp.tile([128, 128], BF16)
            make_identity(nc, ident[:])
            # preload the ACT function tables used in the tail
            warmt = mp.tile([128, 1], F32)
            nc.scalar.activation(warmt[:], means_t[:, 0:1], ACTF.Exp)
            nc.scalar.sqrt(warmt[:], warmt[:])

            # actions (bf16), noise staging quarters, transposed layouts
            actb = mp.tile([T, PL, A], BF16)
            utile = mp.tile([128, 2 * PL * A // 4], F32)  # [128, 8192]
            nhq = [
                utile[:, 0:4096].rearrange("t (p a) -> t p a", a=A),
                utile[:, 4096:8192].rearrange("t (p a) -> t p a", a=A),
            ]
            actT = mp.tile([128, NT, T, A], BF16)
            act2T = mp.tile([128, NT, T, A], BF16)

            # ---- DTW state: two packed pair-chains, ping-pong.  Rows
            # < R16 keep the DP values in fp16 (2x-mode mins); the last
            # rows -- where the absolute values and hence fp16 quanta are
            # largest -- run in f32 so the accumulated rounding stays small.
            h01a = mp.tile([128, CW + 1], FP16)
            h01b = mp.tile([128, CW + 1], FP16)
            h23a = mp.tile([128, CW + 1], FP16)
            h23b = mp.tile([128, CW + 1], FP16)
            f01a = mp.tile([128, CW + 1], F32)
            f01b = mp.tile([128, CW + 1], F32)
            f23a = mp.tile([128, CW + 1], F32)
            f23b = mp.tile([128, CW + 1], F32)
            ub01 = mp.tile([128, CW], FP16)
            ub23 = mp.tile([128, CW], FP16)
            uf01 = mp.tile([128, CW], F32)
            uf23 = mp.tile([128, CW], F32)
            for t_ in (h01a, h01b, h23a, h23b, f01a, f01b, f23a, f23b):
                nc.vector.memset(t_[:], INFDP)
            # D[0][0] = 0 for each tile (pair cols 0 and 129)
            nc.vector.memset(h01a[:, 0:1], 0.0)
            nc.vector.memset(h01a[:, 129:130], 0.0)
            nc.vector.memset(h23a[:, 0:1], 0.0)
            nc.vector.memset(h23a[:, 129:130], 0.0)
            down = mp.tile([128, NT], F32)
            ch01 = (h01a, h01b)
            ch23 = (h23a, h23b)
            cf01 = (f01a, f01b)
            cf23 = (f23a, f23b)

            def dtw_row(i, cb, r):
                crow = cb[:, r]
                # row i reads the side written at i-1: fp16 through row R16,
                # f32 after; the switch row reads fp16 and writes f32.  The
                # f32 pair's col 0 is INFDP from init and never rewritten.
                A1 = (ch01 if i <= R16 else cf01)[i % 2]
                A2 = (ch23 if i <= R16 else cf23)[i % 2]
                if i < R16:
                    B1, B2, u1, u2 = (
                        ch01[(i + 1) % 2], ch23[(i + 1) % 2], ub01, ub23)
                else:
                    B1, B2, u1, u2 = (
                        cf01[(i + 1) % 2], cf23[(i + 1) % 2], uf01, uf23)
                nc.vector.tensor_tensor(
                    u1[:], A1[:, 0:CW], A1[:, 1 : CW + 1], op=ALU.min
                )
                nc.vector.tensor_tensor(
                    u2[:], A2[:, 0:CW], A2[:, 1 : CW + 1], op=ALU.min
                )
                nc.vector.tensor_tensor_scan(
                    B1[:, 1 : CW + 1], u1[:], crow[:, 0:CW], INFDP,
                    op0=ALU.min, op1=ALU.add,
                )
                nc.vector.tensor_tensor_scan(
                    B2[:, 1 : CW + 1], u2[:], crow[:, CW:CWF], INFDP,
                    op0=ALU.min, op1=ALU.add,
                )
                if i == 0:
                    # D[i>0][0] = INF at the never-rewritten left columns
                    nc.vector.memset(h01a[:, 0:1], INFDP)
                    nc.vector.memset(h23a[:, 0:1], INFDP)

            # ---- actions pipeline pieces (emitted interleaved with DTW)
            def noise_dma(q):
                nc.sync.dma_start(
                    nhq[q % 2][:], noise_d[:, q * 128 : (q + 1) * 128, :]
                )

            def affine(q):
                for a in range(A):
                    nc.scalar.activation(
                        actb[:, q * 128 : (q + 1) * 128, a],
                        nhq[q % 2][:, :, a],
                        ACTF.Identity,
                        bias=means_t[:, a : a + 1],
                        scale=stds_t[:, a : a + 1],
                    )

            def clip(k):
                v = actb[:, k * 128 : (k + 1) * 128, :].rearrange(
                    "t p a -> t (p a)"
                )
                nc.gpsimd.tensor_scalar(
                    v, v, 1.0, -1.0, op0=ALU.min, op1=ALU.max
                )

            def transposes(tpp, k):
                for a in range(A):
                    pt = tpp.tile([128, 128], BF16, tag="tp")
                    nc.tensor.transpose(
                        pt[:],
                        actb[:, k * 128 : (k + 1) * 128, a],
                        ident[:],
                    )
                    nc.scalar.activation(
                        actT[:, k, :, a], pt[:], ACTF.Copy
                    )

            def square(k):
                nc.scalar.activation(
                    act2T[:, k].rearrange("t a b -> t (a b)"),
                    actT[:, k].rearrange("t a b -> t (a b)"),
                    ACTF.Square,
                )

            # ---- window: obs chunks + DTW rows + action stages
            with tc.tile_pool(name="cwin", bufs=CBUFS) as cp, \
                 tc.tile_pool(name="psum_tp", bufs=4, space="PSUM") as tpp:
                cbs = []

                def chunk_dma(c):
                    rows = CROWS[c]
                    cb = cp.tile([128, RCH, CWF], F32, tag="cw")
                    for k in range(NT):
                        o = DMAP[k]
                        nc.sync.dma_start(
                            cb[:, 0:rows, o : o + T],
                            obs_d[k * 128 : (k + 1) * 128,
                                  COFF[c] : COFF[c] + rows, :],
                        )
                    # refresh both INF separators each generation
                    nc.gpsimd.memset(cb[:, 0:rows, SEP1 : SEP1 + 1], INFDP)
                    nc.gpsimd.memset(cb[:, 0:rows, SEP2 : SEP2 + 1], INFDP)
                    return cb

                # prime obs chunks ahead of the first noise quarter so
                # the DTW never starves during pipeline fill
                for c in range(min(3, CBUFS, NCHUNK)):
                    cbs.append(chunk_dma(c))
                noise_dma(0)
                for c in range(3, min(CBUFS, NCHUNK)):
                    cbs.append(chunk_dma(c))

                acts = {
                    0: [lambda: affine(0)],
                    1: [lambda: noise_dma(1), lambda: affine(1)],
                    2: [lambda: noise_dma(2), lambda: clip(0)],
                    3: [lambda: affine(2), lambda: clip(1)],
                    4: [lambda: noise_dma(3),
                        lambda: transposes(tpp, 0)],
                    5: [lambda: affine(3), lambda: clip(2)],
                    6: [lambda: transposes(tpp, 1), lambda: square(0)],
                    7: [lambda: clip(3)],
                    8: [lambda: transposes(tpp, 2), lambda: square(1)],
                    9: [lambda: transposes(tpp, 3)],
                    10: [lambda: square(2)],
                    11: [lambda: square(3)],
                }

                next_key = 0
                for c in range(NCHUNK):
                    cb = cbs[c]
                    for r in range(CROWS[c]):
                        dtw_row(COFF[c] + r, cb, r)
                    if c + CBUFS < NCHUNK:
                        cbs.append(chunk_dma(c + CBUFS))
                    if stage >= 1:
                        # acts keyed by 8-row octiles of emitted DP rows
                        done = COFF[c] + CROWS[c]
                        while next_key * 8 + 8 <= done:
                            for th in acts.get(next_key, []):
                                th()
                            next_key += 1

            # own dists from the final (even-side) f32 buffers
            nc.scalar.activation(down[:, 0:1], f01a[:, 128:129], ACTF.Copy)
            nc.scalar.activation(down[:, 1:2], f01a[:, 257:258], ACTF.Copy)
            nc.scalar.activation(down[:, 2:3], f23a[:, 128:129], ACTF.Copy)
            nc.scalar.activation(down[:, 3:4], f23a[:, 257:258], ACTF.Copy)

            if stage >= 2:
                # ---- AllGather dists (tiny)
                ld = dp.tile([PL], F32)
                gd = dp.tile([P], F32)
                # member order in gd is irrelevant (kth/threshold are
                # order-free), so write ld partition-major: fewer descs
                nc.sync.dma_start(ld.rearrange("(p k) -> p k", k=NT), down[:])
                if single:
                    _, lsrc = bass.broadcast_tensor_aps(
                        gd.rearrange("(r f) -> r f", r=NCORES),
                        ld.rearrange("(o f) -> o f", o=1),
                    )
                    nc.sync.dma_start(
                        gd.rearrange("(r f) -> r f", r=NCORES), lsrc
                    )
                else:
                    nc.gpsimd.collective_compute(
                        "AllGather",
                        ALU.bypass,
                        replica_groups=GROUP,
                        ins=[ld.opt()],
                        outs=[gd.opt()],
                    )

            if stage >= 3:
                # ---- top-K threshold via gpsimd kth_largest on -dists
                gdsq = mp.tile([128, P // 128], F32)
                nc.sync.dma_start(
                    gdsq[:], gd.rearrange("(p f) -> p f", p=128)
                )
                ngd = mp.tile([128, P // 128], F32)
                nc.vector.tensor_scalar(
                    ngd[:], gdsq[:], -1.0, None, op0=ALU.mult
                )
                kth = mp.tile([128, 2], F32)
                nc.gpsimd.kth_largest(
                    kth[:], ngd[:], P // 128, K + 3,
                    quantile=1.0 - (K - 0.5) / (P - 1),
                )
                # kth col1 = desc[k_adj+1] = -s[K] ; mask = d < s[K]
                thb = mp.tile([128, 2], F32)
                nc.gpsimd.partition_broadcast(thb[:], kth[0:1, :])
                thneg = mp.tile([128, 1], F32)
                nc.vector.tensor_scalar(
                    thneg[:], thb[:, 1:2], -1.0, None, op0=ALU.mult
                )
                # softmax shift: any global constant cancels exactly; use
                # gd[0] (safe unless the dist spread nears 176/TEMP).
                dref = mp.tile([128, 1], F32)
                nc.gpsimd.partition_broadcast(dref[:], gdsq[0:1, 0:1])
                biast = mp.tile([128, 1], F32)
                nc.gpsimd.tensor_scalar(
                    biast[:], dref[:], TEMP, None, op0=ALU.mult
                )
                mask4 = mp.tile([128, NT], F32)
                nc.vector.tensor_scalar(
                    mask4[:], down[:], thneg[:, 0:1], None, op0=ALU.is_lt
                )
                e4 = mp.tile([128, NT], F32)
                nc.scalar.activation(
                    e4[:], down[:], ACTF.Exp, bias=biast[:, 0:1], scale=-TEMP
                )
                w4 = mp.tile([128, NT], F32)
                nc.vector.tensor_tensor(w4[:], e4[:], mask4[:], op=ALU.mult)
                wb = mp.tile([128, NT], BF16)
                nc.scalar.activation(wb[:], w4[:], ACTF.Copy)
                # sum of weights across members (free then partitions)
                slocal = mp.tile([128, 1], F32)
                nc.vector.tensor_reduce(
                    slocal[:], w4[:], axis=mybir.AxisListType.X, op=ALU.add
                )
                swr = mp.tile([128, 1], F32)
                nc.gpsimd.partition_all_reduce(
                    swr[:], slocal[:], 128, bass_isa.ReduceOp.add
                )
                # bf16 warmup gates: dnb ready at DTW end, gsb ready when
                # the gathered dists land (a few us before the weights)
                dnb = mp.tile([128, NT], BF16)
                nc.scalar.activation(dnb[:], down[:], ACTF.Copy)
                gsb = mp.tile([128, NT], BF16)
                nc.scalar.activation(gsb[:], gdsq[:, 0:NT], ACTF.Copy)

            if stage >= 4:
                # ---- weighted sums as PE matmuls accumulating over tiles
                NTOT = 2 * T * A + 1
                arin = dp.tile([NTOT], F32)
                arout = dp.tile([NTOT], F32)
                nc.sync.dma_start(
                    arin[2 * T * A : NTOT].rearrange("(o f) -> o f", o=1),
                    swr[0:1, 0:1],
                )
                _pst_cm = tc.tile_pool(name="psum_st", bufs=1, space="PSUM")
                pst = _pst_cm.__enter__()
                sts = []
                for c in range(8):
                    st = pst.tile([128, 512], F32, tag=f"st{c}")
                    sts.append(st)
                # PE p-state warmup: junk matmuls gated on the dists; their
                # outputs are reset by the first start=True real matmul.
                for wi in range(WARM + WARM2):
                    wsrc = dnb if wi < WARM else gsb
                    nc.tensor.matmul(
                        sts[wi % 8][0:1, :],
                        wsrc[:, 0:1],
                        actT[:, wi % NT, (wi % 8) * 16 : (wi % 8) * 16 + 16, :],
                        start=True, stop=True, skip_group_check=True,
                    )
                # staging rows alias dead actb (32-aligned partitions)
                arsc = actb[:].rearrange("t p a -> t (p a)").bitcast(F32)
                arsb_m = arsc[0:1, 0 : T * A]
                arsb_s = arsc[32:33, 0 : T * A]
                for c in range(8):
                    for k in range(NT):
                        nc.tensor.matmul(
                            sts[c][0:1, :],
                            wb[:, k : k + 1],
                            actT[:, k, c * 16 : (c + 1) * 16, :],
                            start=(k == 0), stop=(k == NT - 1),
                        )
                    nc.scalar.activation(
                        arsb_m[:, c * 512 : (c + 1) * 512],
                        sts[c][0:1, :], ACTF.Copy,
                    )
                for c in range(8):
                    for k in range(NT):
                        nc.tensor.matmul(
                            sts[c][32:33, :],
                            wb[:, k : k + 1],
                            act2T[:, k, c * 16 : (c + 1) * 16, :],
                            start=(k == 0), stop=(k == NT - 1),
                        )
                    nc.vector.tensor_copy(
                        arsb_s[:, c * 512 : (c + 1) * 512],
                        sts[c][32:33, :],
                    )

                nc.sync.dma_start(
                    arin[0 : T * A].rearrange("(o f) -> o f", o=1), arsb_m[:]
                )
                nc.sync.dma_start(
                    arin[T * A : 2 * T * A].rearrange("(o f) -> o f", o=1),
                    arsb_s[:],
                )
                if single:
                    nc.sync.dma_start(arout[:], arin[:])
                else:
                    nc.gpsimd.collective_compute(
                        "AllReduce",
                        ALU.add,
                        replica_groups=GROUP,
                        ins=[arin.opt()],
                        outs=[arout.opt()],
                    )
                _pst_cm.__exit__(None, None, None)

            if stage >= 5:
                # ---- final statistics
                rn12 = mp.tile([128, 2, A], F32)
                nc.sync.dma_start(
                    rn12[:],
                    arout[0 : 2 * T * A].rearrange(
                        "(q t a) -> t q a", q=2, t=T
                    ),
                )
                rs = mp.tile([128, 1], F32)
                _, rssrc = bass.broadcast_tensor_aps(
                    rs[:],
                    arout[2 * T * A : NTOT].rearrange("(o f) -> o f", o=1),
                )
                nc.sync.dma_start(rs[:], rssrc)
                rn1 = rn12[:, 0]
                rn2 = rn12[:, 1]
                inv = mp.tile([128, 1], F32)
                nc.vector.reciprocal(inv[:], rs[:])
                mh = mp.tile([128, A], F32)
                nc.vector.tensor_scalar(
                    mh[:], rn1, inv[:, 0:1], None, op0=ALU.mult
                )
                q = mp.tile([128, A], F32)
                nc.vector.tensor_scalar(
                    q[:], rn2, inv[:, 0:1], None, op0=ALU.mult
                )
                msq = mp.tile([128, A], F32)
                nc.vector.tensor_tensor(msq[:], mh[:], mh[:], op=ALU.mult)
                var = mp.tile([128, A], F32)
                nc.vector.tensor_tensor(var[:], q[:], msq[:], op=ALU.subtract)
                nc.vector.tensor_scalar(var[:], var[:], 0.0, None, op0=ALU.max)
                stdv = mp.tile([128, A], F32)
                ostk = mp.tile([128, 2, A], F32)
                nc.scalar.sqrt(stdv[:], var[:])
                nc.vector.tensor_scalar(
                    ostk[:, 1], stdv[:], MIN_STD, 1.0, op0=ALU.max, op1=ALU.min
                )
                nc.vector.tensor_scalar(
                    mh[:], mh[:], 1.0 - MOM, None, op0=ALU.mult
                )
                nc.vector.scalar_tensor_tensor(
                    ostk[:, 0], means_t[:], MOM, mh[:], op0=ALU.mult,
                    op1=ALU.add,
                )
                nc.sync.dma_start(
                    out_d.rearrange("q t o a -> t (q o) a"), ostk[:]
                )
            else:
                # bisect debug output
                dbg = mp.tile([128, A], F32)
                nc.vector.memset(dbg[:], 0.0)
                nc.vector.tensor_copy(dbg[:, 0:NT], down[:])
                if stage >= 3:
                    nc.vector.tensor_copy(dbg[:, 4 : 4 + NT], w4[:])
                    nc.vector.tensor_copy(dbg[:, 8:9], thneg[:])
                    nc.vector.tensor_copy(dbg[:, 9:10], swr[:])
                if stage == 2:
                    gdbg = mp.tile([128, A], F32)
                    nc.sync.dma_start(
                        gdbg[:],
                        gd[0 : 128 * A].rearrange("(p a) -> p a", a=A),
                    )
                    nc.vector.tensor_copy(dbg[:, 4:8], gdbg[:, 0:4])
                nc.sync.dma_start(out_d[0, :, 0, :], dbg[:])
                nc.sync.dma_start(out_d[1, :, 0, :], dbg[:])

    nc.compile()
    return nc


def _get_nc(stage=None, single=None):
    if stage is None:
        stage = int(os.environ.get("CEM_STAGE", "9"))
    if single is None:
        single = bool(int(os.environ.get("CEM_SINGLE", "0")))
    key = ("nc", stage, single)
    if key not in _CACHE:
        _CACHE[key] = _build(stage, single)
    return _CACHE[key]


def kernel(**inputs):
    obs = np.ascontiguousarray(np.asarray(inputs["obs_diffs"], np.float32))
    means = np.ascontiguousarray(np.asarray(inputs["means"], np.float32))
    stds = np.ascontiguousarray(np.asarray(inputs["stds"], np.float32))
    noise = np.ascontiguousarray(np.asarray(inputs["noise"], np.float32))

    nc = _get_nc(stage=9, single=False)
    in_maps = []
    for c in range(NCORES):
        in_maps.append(
            {
                "obs": obs[c * PL : (c + 1) * PL],
                "means": means,
                "stds": stds,
                "noise": np.ascontiguousarray(noise[:, c * PL : (c + 1) * PL, :]),
            }
        )
    res = bass_utils.run_bass_kernel_spmd(
        nc, in_maps, core_ids=list(range(NCORES))
    )
    out = np.asarray(res.results[0]["out"], np.float32)
    return out.reshape(2, T, 1, A)


# revision 37
# speedup vs baseline: 1.1095x; 1.0265x over previous
"""CEM sampling kernel for Trainium2, 8-core SPMD (population sharded).

Per core (512 of 4096 members), one fused program:

  Window (overlapped with the 42MB obs+noise HBM stream, ~117us):
   - DTW min-plus DP entirely on DVE (the scan/min ops exist only
     there): two packed pair-chains [t0|sep|t1] and [t2|sep|t3], DP
     state in fp16 (2x-mode mins; the scan's carry is internally fp32
     and the f32 cost rows are never rounded, so only the stored row
     values quantize).  ~1.1us/row.
   - Actions: ACT computes bf16 act = means + stds*noise per action
     dim, Pool clips in bf16, PE transposes [t,p] blocks to a
     population-major bf16 layout, ACT copies PSUM->SBUF and squares.
  Tail (~35us): AllGather dists; top-K via the gpsimd kth_largest
     library op on the [128,32] negated global dists (exact K-th
     threshold, replaces rank compares and broadcasts); weights; the
     weighted mean / E[x^2] reductions as 64 bf16 PE matmuls (with a
     p-state warmup) accumulating in PSUM; AllReduce; closing stats.
"""

import os
import sys

for _p in ("/opt/trn_rl_repo", "/root/.axon_site/_ro/trn_rl_repo"):
    if _p not in sys.path:
        sys.path.insert(0, _p)

import numpy as np

import concourse.bass as bass
import concourse.bacc as bacc
import concourse.bass_isa as bass_isa
import concourse.tile as tile
from concourse import mybir
from concourse import bass_utils
from concourse.masks import make_identity

F32 = mybir.dt.float32
FP16 = mybir.dt.float16
BF16 = mybir.dt.bfloat16
ALU = mybir.AluOpType
ACTF = mybir.ActivationFunctionType

P, T, A = 4096, 128, 32
NCORES = 8
PL = P // NCORES          # 512 population per core
NT = PL // 128            # 4 tiles of 128 on the partition dim
K = int(P * 0.1)          # 409
TEMP, MOM, MIN_STD = 0.5, 0.1, 0.05
INFDP = 30000.0           # fp16-safe stand-in for +inf in the DP
RCH = int(os.environ.get("CEM_RCH", "8"))   # DP rows per streamed chunk
_C0 = int(os.environ.get("CEM_C0", "4"))    # optional small first chunk
CROWS = ([_C0, RCH - _C0] if _C0 else []) + [RCH] * ((T - (RCH if _C0 else 0)) // RCH)
COFF = [sum(CROWS[:i]) for i in range(len(CROWS))]
NCHUNK = len(CROWS)
CBUFS = int(os.environ.get("CEM_CBUFS", "4"))
WARM = int(os.environ.get("CEM_WARM", "8"))  # PE p-state warmup matmuls
WARM2 = int(os.environ.get("CEM_WARM2", "5"))  # late warmups gated on gdsq
DPDT = FP16 if os.environ.get("CEM_DPDT", "fp16") == "fp16" else F32
R16 = int(os.environ.get("CEM_R16", "80"))  # rows in fp16 before f32
if DPDT == F32:
    R16 = 0
GROUP = [list(range(NCORES))]

# packed cost-row layout: [t0(128) sep t1(128) | t2(128) sep t3(128)]
CW = 257                  # cost width of one pair-chain
CWF = 514
SEP1, SEP2 = 128, 385
DMAP = {0: 0, 1: 129, 2: 257, 3: 386}  # pop tile -> flat cost column

_CACHE = {}


def _build(stage=9, single=False):
    nc = bacc.Bacc(
        "TRN2",
        target_bir_lowering=False,
        debug=False,
        num_devices=1 if single else NCORES,
    )
    obs_d = nc.dram_tensor("obs", [PL, T, T], F32, kind="ExternalInput")
    means_d = nc.dram_tensor("means", [T, 1, A], F32, kind="ExternalInput")
    stds_d = nc.dram_tensor("stds", [T, 1, A], F32, kind="ExternalInput")
    noise_d = nc.dram_tensor("noise", [T, PL, A], F32, kind="ExternalInput")
    out_d = nc.dram_tensor("out", [2, T, 1, A], F32, kind="ExternalOutput")

    with tile.TileContext(nc) as tc:
        with (
            tc.tile_pool(name="main", bufs=1) as mp,
            tc.tile_pool(name="dram", bufs=1, space="DRAM") as dp,
        ):
            # ---- small persistent tiles
            means_t = mp.tile([T, A], F32)
            stds_t = mp.tile([T, A], F32)
            nc.sync.dma_start(means_t[:], means_d[:, 0, :])
            nc.sync.dma_start(stds_t[:], stds_d[:, 0, :])
            ident = mp.tile([128, 128], BF16)
            make_identity(nc, ident[:])
            # preload the ACT function tables used in the tail
            warmt = mp.tile([128, 1], F32)
            nc.scalar.activation(warmt[:], means_t[:, 0:1], ACTF.Exp)
            nc.scalar.sqrt(warmt[:], warmt[:])

            # actions (bf16), noise staging quarters, transposed layouts
            actb = mp.tile([T, PL, A], BF16)
            utile = mp.tile([128, 2 * PL * A // 4], F32)  # [128, 8192]
            nhq = [
                utile[:, 0:4096].rearrange("t (p a) -> t p a", a=A),
                utile[:, 4096:8192].rearrange("t (p a) -> t p a", a=A),
            ]
            actT = mp.tile([128, NT, T, A], BF16)
            act2T = mp.tile([128, NT, T, A], BF16)

            # ---- DTW state: two packed pair-chains, ping-pong.  Rows
            # < R16 keep the DP values in fp16 (2x-mode mins); the last
            # rows -- where the absolute values and hence fp16 quanta are
            # largest -- run in f32 so the accumulated rounding stays small.
            h01a = mp.tile([128, CW + 1], FP16)
            h01b = mp.tile([128, CW + 1], FP16)
            h23a = mp.tile([128, CW + 1], FP16)
            h23b = mp.tile([128, CW + 1], FP16)
            f01a = mp.tile([128, CW + 1], F32)
            f01b = mp.tile([128, CW + 1], F32)
            f23a = mp.tile([128, CW + 1], F32)
            f23b = mp.tile([128, CW + 1], F32)
            ub01 = mp.tile([128, CW], FP16)
            ub23 = mp.tile([128, CW], FP16)
            uf01 = mp.tile([128, CW], F32)
            uf23 = mp.tile([128, CW], F32)
            for t_ in (h01a, h01b, h23a, h23b, f01a, f01b, f23a, f23b):
                nc.vector.memset(t_[:], INFDP)
            # D[0][0] = 0 for each tile (pair cols 0 and 129)
            nc.vector.memset(h01a[:, 0:1], 0.0)
            nc.vector.memset(h01a[:, 129:130], 0.0)
            nc.vector.memset(h23a[:, 0:1], 0.0)
            nc.vector.memset(h23a[:, 129:130], 0.0)
            down = mp.tile([128, NT], F32)
            ch01 = (h01a, h01b)
            ch23 = (h23a, h23b)
            cf01 = (f01a, f01b)
            cf23 = (f23a, f23b)

            def dtw_row(i, cb, r):
                crow = cb[:, r]
                # row i reads the side written at i-1: fp16 through row R16,
                # f32 after; the switch row reads fp16 and writes f32.  The
                # f32 pair's col 0 is INFDP from init and never rewritten.
                A1 = (ch01 if i <= R16 else cf01)[i % 2]
                A2 = (ch23 if i <= R16 else cf23)[i % 2]
                if i < R16:
                    B1, B2, u1, u2 = (
                        ch01[(i + 1) % 2], ch23[(i + 1) % 2], ub01, ub23)
                else:
                    B1, B2, u1, u2 = (
                        cf01[(i + 1) % 2], cf23[(i + 1) % 2], uf01, uf23)
                nc.vector.tensor_tensor(
                    u1[:], A1[:, 0:CW], A1[:, 1 : CW + 1], op=ALU.min
                )
                nc.vector.tensor_tensor(
                    u2[:], A2[:, 0:CW], A2[:, 1 : CW + 1], op=ALU.min
                )
                nc.vector.tensor_tensor_scan(
                    B1[:, 1 : CW + 1], u1[:], crow[:, 0:CW], INFDP,
                    op0=ALU.min, op1=ALU.add,
                )
                nc.vector.tensor_tensor_scan(
                    B2[:, 1 : CW + 1], u2[:], crow[:, CW:CWF], INFDP,
                    op0=ALU.min, op1=ALU.add,
                )
                if i == 0:
                    # D[i>0][0] = INF at the never-rewritten left columns
                    nc.vector.memset(h01a[:, 0:1], INFDP)
                    nc.vector.memset(h23a[:, 0:1], INFDP)

            # ---- actions pipeline pieces (emitted interleaved with DTW)
            def noise_dma(q):
                nc.sync.dma_start(
                    nhq[q % 2][:], noise_d[:, q * 128 : (q + 1) * 128, :]
                )

            def affine(q):
                for a in range(A):
                    nc.scalar.activation(
                        actb[:, q * 128 : (q + 1) * 128, a],
                        nhq[q % 2][:, :, a],
                        ACTF.Identity,
                        bias=means_t[:, a : a + 1],
                        scale=stds_t[:, a : a + 1],
                    )

            def clip(k):
                v = actb[:, k * 128 : (k + 1) * 128, :].rearrange(
                    "t p a -> t (p a)"
                )
                nc.gpsimd.tensor_scalar(
                    v, v, 1.0, -1.0, op0=ALU.min, op1=ALU.max
                )

            def transposes(tpp, k):
                for a in range(A):
                    pt = tpp.tile([128, 128], BF16, tag="tp")
                    nc.tensor.transpose(
                        pt[:],
                        actb[:, k * 128 : (k + 1) * 128, a],
                        ident[:],
                    )
                    nc.scalar.activation(
                        actT[:, k, :, a], pt[:], ACTF.Copy
                    )

            def square(k):
                nc.scalar.activation(
                    act2T[:, k].rearrange("t a b -> t (a b)"),
                    actT[:, k].rearrange("t a b -> t (a b)"),
                    ACTF.Square,
                )

            # ---- window: obs chunks + DTW rows + action stages
            with tc.tile_pool(name="cwin", bufs=CBUFS) as cp, \
                 tc.tile_pool(name="psum_tp", bufs=4, space="PSUM") as tpp:
                cbs = []

                def chunk_dma(c):
                    rows = CROWS[c]
                    cb = cp.tile([128, RCH, CWF], F32, tag="cw")
                    for k in range(NT):
                        o = DMAP[k]
                        nc.sync.dma_start(
                            cb[:, 0:rows, o : o + T],
                            obs_d[k * 128 : (k + 1) * 128,
                                  COFF[c] : COFF[c] + rows, :],
                        )
                    # refresh both INF separators each generation
                    nc.gpsimd.memset(cb[:, 0:rows, SEP1 : SEP1 + 1], INFDP)
                    nc.gpsimd.memset(cb[:, 0:rows, SEP2 : SEP2 + 1], INFDP)
                    return cb

                # prime obs chunks ahead of the first noise quarter so
                # the DTW never starves during pipeline fill
                for c in range(min(3, CBUFS, NCHUNK)):
                    cbs.append(chunk_dma(c))
                noise_dma(0)
                for c in range(3, min(CBUFS, NCHUNK)):
                    cbs.append(chunk_dma(c))

                acts = {
                    0: [lambda: affine(0)],
                    1: [lambda: noise_dma(1), lambda: affine(1)],
                    2: [lambda: noise_dma(2), lambda: clip(0)],
                    3: [lambda: affine(2), lambda: clip(1)],
                    4: [lambda: noise_dma(3),
                        lambda: transposes(tpp, 0)],
                    5: [lambda: affine(3), lambda: clip(2)],
                    6: [lambda: transposes(tpp, 1), lambda: square(0)],
                    7: [lambda: clip(3)],
                    8: [lambda: transposes(tpp, 2), lambda: square(1)],
                    9: [lambda: transposes(tpp, 3)],
                    10: [lambda: square(2)],
                    11: [lambda: square(3)],
                }

                next_key = 0
                for c in range(NCHUNK):
                    cb = cbs[c]
                    for r in range(CROWS[c]):
                        dtw_row(COFF[c] + r, cb, r)
                    if c + CBUFS < NCHUNK:
                        cbs.append(chunk_dma(c + CBUFS))
                    if stage >= 1:
                        # acts keyed by 8-row octiles of emitted DP rows
                        done = COFF[c] + CROWS[c]
                        while next_key * 8 + 8 <= done:
                            for th in acts.get(next_key, []):
                                th()
                            next_key += 1

            # own dists from the final (even-side) f32 buffers
            nc.scalar.activation(down[:, 0:1], f01a[:, 128:129], ACTF.Copy)
            nc.scalar.activation(down[:, 1:2], f01a[:, 257:258], ACTF.Copy)
            nc.scalar.activation(down[:, 2:3], f23a[:, 128:129], ACTF.Copy)
            nc.scalar.activation(down[:, 3:4], f23a[:, 257:258], ACTF.Copy)

            if stage >= 2:
                # ---- AllGather dists (tiny)
                ld = dp.tile([PL], F32)
                gd = dp.tile([P], F32)
                # member order in gd is irrelevant (kth/threshold are
                # order-free), so write ld partition-major: fewer descs
                nc.sync.dma_start(ld.rearrange("(p k) -> p k", k=NT), down[:])
                if single:
                    _, lsrc = bass.broadcast_tensor_aps(
                        gd.rearrange("(r f) -> r f", r=NCORES),
                        ld.rearrange("(o f) -> o f", o=1),
                    )
                    nc.sync.dma_start(
                        gd.rearrange("(r f) -> r f", r=NCORES), lsrc
                    )
                else:
                    nc.gpsimd.collective_compute(
                        "AllGather",
                        ALU.bypass,
                        replica_groups=GROUP,
                        ins=[ld.opt()],
                        outs=[gd.opt()],
                    )

            if stage >= 3:
                # ---- top-K threshold via gpsimd kth_largest on -dists
                gdsq = mp.tile([128, P // 128], F32)
                nc.sync.dma_start(
                    gdsq[:], gd.rearrange("(p f) -> p f", p=128)
                )
                ngd = mp.tile([128, P // 128], F32)
                nc.vector.tensor_scalar(
                    ngd[:], gdsq[:], -1.0, None, op0=ALU.mult
                )
                kth = mp.tile([128, 2], F32)
                nc.gpsimd.kth_largest(
                    kth[:], ngd[:], P // 128, K + 3,
                    quantile=1.0 - (K - 0.5) / (P - 1),
                )
                # kth col1 = desc[k_adj+1] = -s[K] ; mask = d < s[K]
                thb = mp.tile([128, 2], F32)
                nc.gpsimd.partition_broadcast(thb[:], kth[0:1, :])
                thneg = mp.tile([128, 1], F32)
                nc.vector.tensor_scalar(
                    thneg[:], thb[:, 1:2], -1.0, None, op0=ALU.mult
                )
                # softmax shift: any global constant cancels exactly; use
                # gd[0] (safe unless the dist spread nears 176/TEMP).
                dref = mp.tile([128, 1], F32)
                nc.gpsimd.partition_broadcast(dref[:], gdsq[0:1, 0:1])
                biast = mp.tile([128, 1], F32)
                nc.gpsimd.tensor_scalar(
                    biast[:], dref[:], TEMP, None, op0=ALU.mult
                )
                mask4 = mp.tile([128, NT], F32)
                nc.vector.tensor_scalar(
                    mask4[:], down[:], thneg[:, 0:1], None, op0=ALU.is_lt
                )
                e4 = mp.tile([128, NT], F32)
                nc.scalar.activation(
                    e4[:], down[:], ACTF.Exp, bias=biast[:, 0:1], scale=-TEMP
                )
                w4 = mp.tile([128, NT], F32)
                nc.vector.tensor_tensor(w4[:], e4[:], mask4[:], op=ALU.mult)
                wb = mp.tile([128, NT], BF16)
                nc.scalar.activation(wb[:], w4[:], ACTF.Copy)
                # sum of weights across members (free then partitions)
                slocal = mp.tile([128, 1], F32)
                nc.vector.tensor_reduce(
                    slocal[:], w4[:], axis=mybir.AxisListType.X, op=ALU.add
                )
                swr = mp.tile([128, 1], F32)
                nc.gpsimd.partition_all_reduce(
                    swr[:], slocal[:], 128, bass_isa.ReduceOp.add
                )
                # bf16 warmup gates: dnb ready at DTW end, gsb ready when
                # the gathered dists land (a few us before the weights)
                dnb = mp.tile([128, NT], BF16)
                nc.scalar.activation(dnb[:], down[:], ACTF.Copy)
                gsb = mp.tile([128, NT], BF16)
                nc.scalar.activation(gsb[:], gdsq[:, 0:NT], ACTF.Copy)

            if stage >= 4:
                # ---- weighted sums as PE matmuls accumulating over tiles
                NTOT = 2 * T * A + 1
                arin = dp.tile([NTOT], F32)
                arout = dp.tile([NTOT], F32)
                nc.sync.dma_start(
                    arin[2 * T * A : NTOT].rearrange("(o f) -> o f", o=1),
                    swr[0:1, 0:1],
                )
                _pst_cm = tc.tile_pool(name="psum_st", bufs=1, space="PSUM")
                pst = _pst_cm.__enter__()
                sts = []
                for c in range(8):
                    st = pst.tile([128, 512], F32, tag=f"st{c}")
                    sts.append(st)
                # PE p-state warmup: junk matmuls gated on the dists; their
                # outputs are reset by the first start=True real matmul.
                for wi in range(WARM + WARM2):
                    wsrc = dnb if wi < WARM else gsb
                    nc.tensor.matmul(
                        sts[wi % 8][0:1, :],
                        wsrc[:, 0:1],
                        actT[:, wi % NT, (wi % 8) * 16 : (wi % 8) * 16 + 16, :],
                        start=True, stop=True, skip_group_check=True,
                    )
                # staging rows alias dead actb (32-aligned partitions)
                arsc = actb[:].rearrange("t p a -> t (p a)").bitcast(F32)
                arsb_m = arsc[0:1, 0 : T * A]
                arsb_s = arsc[32:33, 0 : T * A]
                for c in range(8):
                    for k in range(NT):
                        nc.tensor.matmul(
                            sts[c][0:1, :],
                            wb[:, k : k + 1],
                            actT[:, k, c * 16 : (c + 1) * 16, :],
                            start=(k == 0), stop=(k == NT - 1),
                        )
                    nc.scalar.activation(
                        arsb_m[:, c * 512 : (c + 1) * 512],
                        sts[c][0:1, :], ACTF.Copy,
                    )
                for c in range(8):
                    for k in range(NT):
                        nc.tensor.matmul(
                            sts[c][32:33, :],
                            wb[:, k : k + 1],
                            act2T[:, k, c * 16 : (c + 1) * 16, :],
                            start=(k == 0), stop=(k == NT - 1),
                        )
                    nc.vector.tensor_copy(
                        arsb_s[:, c * 512 : (c + 1) * 512],
                        sts[c][32:33, :],
                    )

                nc.sync.dma_start(
                    arin[0 : T * A].rearrange("(o f) -> o f", o=1), arsb_m[:]
                )
                nc.sync.dma_start(
                    arin[T * A : 2 * T * A].rearrange("(o f) -> o f", o=1),
                    arsb_s[:],
                )
                if single:
                    nc.sync.dma_start(arout[:], arin[:])
                else:
                    nc.gpsimd.collective_compute(
                        "AllReduce",
                        ALU.add,
                        replica_groups=GROUP,
                        ins=[arin.opt()],
                        outs=[arout.opt()],
                    )
                _pst_cm.__exit__(None, None, None)

            if stage >= 5:
                # ---- final statistics
                rn12 = mp.tile([128, 2, A], F32)
                nc.sync.dma_start(
                    rn12[:],
                    arout[0 : 2 * T * A].rearrange(
                        "(q t a) -> t q a", q=2, t=T
                    ),
                )
                rs = mp.tile([128, 1], F32)
                _, rssrc = bass.broadcast_tensor_aps(
                    rs[:],
                    arout[2 * T * A : NTOT].rearrange("(o f) -> o f", o=1),
                )
                nc.sync.dma_start(rs[:], rssrc)
                rn1 = rn12[:, 0]
                rn2 = rn12[:, 1]
                inv = mp.tile([128, 1], F32)
                nc.vector.reciprocal(inv[:], rs[:])
                mh = mp.tile([128, A], F32)
                nc.vector.tensor_scalar(
                    mh[:], rn1, inv[:, 0:1], None, op0=ALU.mult
                )
                q = mp.tile([128, A], F32)
                nc.vector.tensor_scalar(
                    q[:], rn2, inv[:, 0:1], None, op0=ALU.mult
                )
                msq = mp.tile([128, A], F32)
                nc.vector.tensor_tensor(msq[:], mh[:], mh[:], op=ALU.mult)
                var = mp.tile([128, A], F32)
                nc.vector.tensor_tensor(var[:], q[:], msq[:], op=ALU.subtract)
                nc.vector.tensor_scalar(var[:], var[:], 0.0, None, op0=ALU.max)
                stdv = mp.tile([128, A], F32)
                ostk = mp.tile([128, 2, A], F32)
                nc.scalar.sqrt(stdv[:], var[:])
                nc.vector.tensor_scalar(
                    ostk[:, 1], stdv[:], MIN_STD, 1.0, op0=ALU.max, op1=ALU.min
                )
                nc.vector.tensor_scalar(
                    mh[:], mh[:], 1.0 - MOM, None, op0=ALU.mult
                )
                nc.vector.scalar_tensor_tensor(
                    ostk[:, 0], means_t[:], MOM, mh[:], op0=ALU.mult,
                    op1=ALU.add,
                )
                nc.sync.dma_start(
                    out_d.rearrange("q t o a -> t (q o) a"), ostk[:]
                )
            else:
                # bisect debug output
                dbg = mp.tile([128, A], F32)
                nc.vector.memset(dbg[:], 0.0)
                nc.vector.tensor_copy(dbg[:, 0:NT], down[:])
                if stage >= 3:
                    nc.vector.tensor_copy(dbg[:, 4 : 4 + NT], w4[:])
                    nc.vector.tensor_copy(dbg[:, 8:9], thneg[:])
                    nc.vector.tensor_copy(dbg[:, 9:10], swr[:])
                if stage == 2:
                    gdbg = mp.tile([128, A], F32)
                    nc.sync.dma_start(
                        gdbg[:],
                        gd[0 : 128 * A].rearrange("(p a) -> p a", a=A),
                    )
                    nc.vector.tensor_copy(dbg[:, 4:8], gdbg[:, 0:4])
                nc.sync.dma_start(out_d[0, :, 0, :], dbg[:])
                nc.sync.dma_start(out_d[1, :, 0, :], dbg[:])

    nc.compile()
    return nc


def _get_nc(stage=None, single=None):
    if stage is None:
        stage = int(os.environ.get("CEM_STAGE", "9"))
    if single is None:
        single = bool(int(os.environ.get("CEM_SINGLE", "0")))
    key = ("nc", stage, single)
    if key not in _CACHE:
        _CACHE[key] = _build(stage, single)
    return _CACHE[key]


def kernel(**inputs):
    obs = np.ascontiguousarray(np.asarray(inputs["obs_diffs"], np.float32))
    means = np.ascontiguousarray(np.asarray(inputs["means"], np.float32))
    stds = np.ascontiguousarray(np.asarray(inputs["stds"], np.float32))
    noise = np.ascontiguousarray(np.asarray(inputs["noise"], np.float32))

    nc = _get_nc(stage=9, single=False)
    in_maps = []
    for c in range(NCORES):
        in_maps.append(
            {
                "obs": obs[c * PL : (c + 1) * PL],
                "means": means,
                "stds": stds,
                "noise": np.ascontiguousarray(noise[:, c * PL : (c + 1) * PL, :]),
            }
        )
    res = bass_utils.run_bass_kernel_spmd(
        nc, in_maps, core_ids=list(range(NCORES))
    )
    out = np.asarray(res.results[0]["out"], np.float32)
    return out.reshape(2, T, 1, A)


# revision 38
# speedup vs baseline: 1.1220x; 1.0113x over previous
"""CEM sampling kernel for Trainium2, 8-core SPMD (population sharded).

Per core (512 of 4096 members), one fused program:

  Window (overlapped with the 42MB obs+noise HBM stream, ~117us):
   - DTW min-plus DP entirely on DVE (the scan/min ops exist only
     there): two packed pair-chains [t0|sep|t1] and [t2|sep|t3], DP
     state in fp16 (2x-mode mins; the scan's carry is internally fp32
     and the f32 cost rows are never rounded, so only the stored row
     values quantize).  ~1.1us/row.
   - Actions: ACT computes bf16 act = means + stds*noise per action
     dim, Pool clips in bf16, PE transposes [t,p] blocks to a
     population-major bf16 layout, ACT copies PSUM->SBUF and squares.
  Tail (~35us): AllGather dists; top-K via the gpsimd kth_largest
     library op on the [128,32] negated global dists (exact K-th
     threshold, replaces rank compares and broadcasts); weights; the
     weighted mean / E[x^2] reductions as 64 bf16 PE matmuls (with a
     p-state warmup) accumulating in PSUM; AllReduce; closing stats.
"""

import os
import sys

for _p in ("/opt/trn_rl_repo", "/root/.axon_site/_ro/trn_rl_repo"):
    if _p not in sys.path:
        sys.path.insert(0, _p)

import numpy as np

import concourse.bass as bass
import concourse.bacc as bacc
import concourse.bass_isa as bass_isa
import concourse.tile as tile
from concourse import mybir
from concourse import bass_utils
from concourse.masks import make_identity

F32 = mybir.dt.float32
FP16 = mybir.dt.float16
BF16 = mybir.dt.bfloat16
ALU = mybir.AluOpType
ACTF = mybir.ActivationFunctionType

P, T, A = 4096, 128, 32
NCORES = 8
PL = P // NCORES          # 512 population per core
NT = PL // 128            # 4 tiles of 128 on the partition dim
K = int(P * 0.1)          # 409
TEMP, MOM, MIN_STD = 0.5, 0.1, 0.05
INFDP = 30000.0           # fp16-safe stand-in for +inf in the DP
RCH = int(os.environ.get("CEM_RCH", "8"))   # DP rows per streamed chunk
_C0 = int(os.environ.get("CEM_C0", "4"))    # optional small first chunk
CROWS = ([_C0, RCH - _C0] if _C0 else []) + [RCH] * ((T - (RCH if _C0 else 0)) // RCH)
COFF = [sum(CROWS[:i]) for i in range(len(CROWS))]
NCHUNK = len(CROWS)
CBUFS = int(os.environ.get("CEM_CBUFS", "4"))
WARM = int(os.environ.get("CEM_WARM", "8"))  # PE p-state warmup matmuls
WARM2 = int(os.environ.get("CEM_WARM2", "5"))  # late warmups gated on gdsq
DPDT = FP16 if os.environ.get("CEM_DPDT", "fp16") == "fp16" else F32
R16 = int(os.environ.get("CEM_R16", "88"))  # rows in fp16 before f32
if DPDT == F32:
    R16 = 0
GROUP = [list(range(NCORES))]

# packed cost-row layout: [t0(128) sep t1(128) | t2(128) sep t3(128)]
CW = 257                  # cost width of one pair-chain
CWF = 514
SEP1, SEP2 = 128, 385
DMAP = {0: 0, 1: 129, 2: 257, 3: 386}  # pop tile -> flat cost column

_CACHE = {}


def _build(stage=9, single=False):
    nc = bacc.Bacc(
        "TRN2",
        target_bir_lowering=False,
        debug=False,
        num_devices=1 if single else NCORES,
    )
    obs_d = nc.dram_tensor("obs", [PL, T, T], F32, kind="ExternalInput")
    means_d = nc.dram_tensor("means", [T, 1, A], F32, kind="ExternalInput")
    stds_d = nc.dram_tensor("stds", [T, 1, A], F32, kind="ExternalInput")
    noise_d = nc.dram_tensor("noise", [T, PL, A], F32, kind="ExternalInput")
    out_d = nc.dram_tensor("out", [2, T, 1, A], F32, kind="ExternalOutput")

    with tile.TileContext(nc) as tc:
        with (
            tc.tile_pool(name="main", bufs=1) as mp,
            tc.tile_pool(name="dram", bufs=1, space="DRAM") as dp,
        ):
            # ---- small persistent tiles
            means_t = mp.tile([T, A], F32)
            stds_t = mp.tile([T, A], F32)
            nc.sync.dma_start(means_t[:], means_d[:, 0, :])
            nc.sync.dma_start(stds_t[:], stds_d[:, 0, :])
            ident = mp.tile([128, 128], BF16)
            make_identity(nc, ident[:])
            # preload the ACT function tables used in the tail
            warmt = mp.tile([128, 1], F32)
            nc.scalar.activation(warmt[:], means_t[:, 0:1], ACTF.Exp)
            nc.scalar.sqrt(warmt[:], warmt[:])

            # actions (bf16), noise staging quarters, transposed layouts
            actb = mp.tile([T, PL, A], BF16)
            utile = mp.tile([128, 2 * PL * A // 4], F32)  # [128, 8192]
            nhq = [
                utile[:, 0:4096].rearrange("t (p a) -> t p a", a=A),
                utile[:, 4096:8192].rearrange("t (p a) -> t p a", a=A),
            ]
            actT = mp.tile([128, NT, T, A], BF16)
            act2T = mp.tile([128, NT, T, A], BF16)

            # ---- DTW state: two packed pair-chains, ping-pong.  Rows
            # < R16 keep the DP values in fp16 (2x-mode mins); the last
            # rows -- where the absolute values and hence fp16 quanta are
            # largest -- run in f32 so the accumulated rounding stays small.
            h01a = mp.tile([128, CW + 1], FP16)
            h01b = mp.tile([128, CW + 1], FP16)
            h23a = mp.tile([128, CW + 1], FP16)
            h23b = mp.tile([128, CW + 1], FP16)
            f01a = mp.tile([128, CW + 1], F32)
            f01b = mp.tile([128, CW + 1], F32)
            f23a = mp.tile([128, CW + 1], F32)
            f23b = mp.tile([128, CW + 1], F32)
            ub01 = mp.tile([128, CW], FP16)
            ub23 = mp.tile([128, CW], FP16)
            uf01 = mp.tile([128, CW], F32)
            uf23 = mp.tile([128, CW], F32)
            for t_ in (h01a, h01b, h23a, h23b, f01a, f01b, f23a, f23b):
                nc.vector.memset(t_[:], INFDP)
            # D[0][0] = 0 for each tile (pair cols 0 and 129)
            nc.vector.memset(h01a[:, 0:1], 0.0)
            nc.vector.memset(h01a[:, 129:130], 0.0)
            nc.vector.memset(h23a[:, 0:1], 0.0)
            nc.vector.memset(h23a[:, 129:130], 0.0)
            down = mp.tile([128, NT], F32)
            ch01 = (h01a, h01b)
            ch23 = (h23a, h23b)
            cf01 = (f01a, f01b)
            cf23 = (f23a, f23b)

            def dtw_row(i, cb, r):
                crow = cb[:, r]
                # row i reads the side written at i-1: fp16 through row R16,
                # f32 after; the switch row reads fp16 and writes f32.  The
                # f32 pair's col 0 is INFDP from init and never rewritten.
                A1 = (ch01 if i <= R16 else cf01)[i % 2]
                A2 = (ch23 if i <= R16 else cf23)[i % 2]
                if i < R16:
                    B1, B2, u1, u2 = (
                        ch01[(i + 1) % 2], ch23[(i + 1) % 2], ub01, ub23)
                else:
                    B1, B2, u1, u2 = (
                        cf01[(i + 1) % 2], cf23[(i + 1) % 2], uf01, uf23)
                nc.vector.tensor_tensor(
                    u1[:], A1[:, 0:CW], A1[:, 1 : CW + 1], op=ALU.min
                )
                nc.vector.tensor_tensor(
                    u2[:], A2[:, 0:CW], A2[:, 1 : CW + 1], op=ALU.min
                )
                nc.vector.tensor_tensor_scan(
                    B1[:, 1 : CW + 1], u1[:], crow[:, 0:CW], INFDP,
                    op0=ALU.min, op1=ALU.add,
                )
                nc.vector.tensor_tensor_scan(
                    B2[:, 1 : CW + 1], u2[:], crow[:, CW:CWF], INFDP,
                    op0=ALU.min, op1=ALU.add,
                )
                if i == 0:
                    # D[i>0][0] = INF at the never-rewritten left columns
                    nc.vector.memset(h01a[:, 0:1], INFDP)
                    nc.vector.memset(h23a[:, 0:1], INFDP)

            # ---- actions pipeline pieces (emitted interleaved with DTW)
            def noise_dma(q):
                nc.sync.dma_start(
                    nhq[q % 2][:], noise_d[:, q * 128 : (q + 1) * 128, :]
                )

            def affine(q):
                for a in range(A):
                    nc.scalar.activation(
                        actb[:, q * 128 : (q + 1) * 128, a],
                        nhq[q % 2][:, :, a],
                        ACTF.Identity,
                        bias=means_t[:, a : a + 1],
                        scale=stds_t[:, a : a + 1],
                    )

            def clip(k):
                v = actb[:, k * 128 : (k + 1) * 128, :].rearrange(
                    "t p a -> t (p a)"
                )
                nc.gpsimd.tensor_scalar(
                    v, v, 1.0, -1.0, op0=ALU.min, op1=ALU.max
                )

            def transposes(tpp, k):
                for a in range(A):
                    pt = tpp.tile([128, 128], BF16, tag="tp")
                    nc.tensor.transpose(
                        pt[:],
                        actb[:, k * 128 : (k + 1) * 128, a],
                        ident[:],
                    )
                    nc.scalar.activation(
                        actT[:, k, :, a], pt[:], ACTF.Copy
                    )

            def square(k):
                nc.scalar.activation(
                    act2T[:, k].rearrange("t a b -> t (a b)"),
                    actT[:, k].rearrange("t a b -> t (a b)"),
                    ACTF.Square,
                )

            # ---- window: obs chunks + DTW rows + action stages
            with tc.tile_pool(name="cwin", bufs=CBUFS) as cp, \
                 tc.tile_pool(name="psum_tp", bufs=4, space="PSUM") as tpp:
                cbs = []

                def chunk_dma(c):
                    rows = CROWS[c]
                    cb = cp.tile([128, RCH, CWF], F32, tag="cw")
                    for k in range(NT):
                        o = DMAP[k]
                        nc.sync.dma_start(
                            cb[:, 0:rows, o : o + T],
                            obs_d[k * 128 : (k + 1) * 128,
                                  COFF[c] : COFF[c] + rows, :],
                        )
                    # refresh both INF separators each generation
                    nc.gpsimd.memset(cb[:, 0:rows, SEP1 : SEP1 + 1], INFDP)
                    nc.gpsimd.memset(cb[:, 0:rows, SEP2 : SEP2 + 1], INFDP)
                    return cb

                # prime obs chunks ahead of the first noise quarter so
                # the DTW never starves during pipeline fill
                for c in range(min(3, CBUFS, NCHUNK)):
                    cbs.append(chunk_dma(c))
                noise_dma(0)
                for c in range(3, min(CBUFS, NCHUNK)):
                    cbs.append(chunk_dma(c))

                acts = {
                    0: [lambda: affine(0)],
                    1: [lambda: noise_dma(1), lambda: affine(1)],
                    2: [lambda: noise_dma(2), lambda: clip(0)],
                    3: [lambda: affine(2), lambda: clip(1)],
                    4: [lambda: noise_dma(3),
                        lambda: transposes(tpp, 0)],
                    5: [lambda: affine(3), lambda: clip(2)],
                    6: [lambda: transposes(tpp, 1), lambda: square(0)],
                    7: [lambda: clip(3)],
                    8: [lambda: transposes(tpp, 2), lambda: square(1)],
                    9: [lambda: transposes(tpp, 3)],
                    10: [lambda: square(2)],
                    11: [lambda: square(3)],
                }

                next_key = 0
                for c in range(NCHUNK):
                    cb = cbs[c]
                    for r in range(CROWS[c]):
                        dtw_row(COFF[c] + r, cb, r)
                    if c + CBUFS < NCHUNK:
                        cbs.append(chunk_dma(c + CBUFS))
                    if stage >= 1:
                        # acts keyed by 8-row octiles of emitted DP rows
                        done = COFF[c] + CROWS[c]
                        while next_key * 8 + 8 <= done:
                            for th in acts.get(next_key, []):
                                th()
                            next_key += 1

            # own dists from the final (even-side) f32 buffers
            nc.scalar.activation(down[:, 0:1], f01a[:, 128:129], ACTF.Copy)
            nc.scalar.activation(down[:, 1:2], f01a[:, 257:258], ACTF.Copy)
            nc.scalar.activation(down[:, 2:3], f23a[:, 128:129], ACTF.Copy)
            nc.scalar.activation(down[:, 3:4], f23a[:, 257:258], ACTF.Copy)

            if stage >= 2:
                # ---- AllGather dists (tiny)
                ld = dp.tile([PL], F32)
                gd = dp.tile([P], F32)
                # member order in gd is irrelevant (kth/threshold are
                # order-free), so write ld partition-major: fewer descs
                nc.sync.dma_start(ld.rearrange("(p k) -> p k", k=NT), down[:])
                if single:
                    _, lsrc = bass.broadcast_tensor_aps(
                        gd.rearrange("(r f) -> r f", r=NCORES),
                        ld.rearrange("(o f) -> o f", o=1),
                    )
                    nc.sync.dma_start(
                        gd.rearrange("(r f) -> r f", r=NCORES), lsrc
                    )
                else:
                    nc.gpsimd.collective_compute(
                        "AllGather",
                        ALU.bypass,
                        replica_groups=GROUP,
                        ins=[ld.opt()],
                        outs=[gd.opt()],
                    )

            if stage >= 3:
                # ---- top-K threshold via gpsimd kth_largest on -dists
                gdsq = mp.tile([128, P // 128], F32)
                nc.sync.dma_start(
                    gdsq[:], gd.rearrange("(p f) -> p f", p=128)
                )
                ngd = mp.tile([128, P // 128], F32)
                nc.vector.tensor_scalar(
                    ngd[:], gdsq[:], -1.0, None, op0=ALU.mult
                )
                kth = mp.tile([128, 2], F32)
                nc.gpsimd.kth_largest(
                    kth[:], ngd[:], P // 128, K + 3,
                    quantile=1.0 - (K - 0.5) / (P - 1),
                )
                # kth col1 = desc[k_adj+1] = -s[K] ; mask = d < s[K]
                thb = mp.tile([128, 2], F32)
                nc.gpsimd.partition_broadcast(thb[:], kth[0:1, :])
                thneg = mp.tile([128, 1], F32)
                nc.vector.tensor_scalar(
                    thneg[:], thb[:, 1:2], -1.0, None, op0=ALU.mult
                )
                # softmax shift: any global constant cancels exactly; use
                # gd[0] (safe unless the dist spread nears 176/TEMP).
                dref = mp.tile([128, 1], F32)
                nc.gpsimd.partition_broadcast(dref[:], gdsq[0:1, 0:1])
                biast = mp.tile([128, 1], F32)
                nc.gpsimd.tensor_scalar(
                    biast[:], dref[:], TEMP, None, op0=ALU.mult
                )
                mask4 = mp.tile([128, NT], F32)
                nc.vector.tensor_scalar(
                    mask4[:], down[:], thneg[:, 0:1], None, op0=ALU.is_lt
                )
                e4 = mp.tile([128, NT], F32)
                nc.scalar.activation(
                    e4[:], down[:], ACTF.Exp, bias=biast[:, 0:1], scale=-TEMP
                )
                w4 = mp.tile([128, NT], F32)
                nc.vector.tensor_tensor(w4[:], e4[:], mask4[:], op=ALU.mult)
                wb = mp.tile([128, NT], BF16)
                nc.scalar.activation(wb[:], w4[:], ACTF.Copy)
                # sum of weights across members (free then partitions)
                slocal = mp.tile([128, 1], F32)
                nc.vector.tensor_reduce(
                    slocal[:], w4[:], axis=mybir.AxisListType.X, op=ALU.add
                )
                swr = mp.tile([128, 1], F32)
                nc.gpsimd.partition_all_reduce(
                    swr[:], slocal[:], 128, bass_isa.ReduceOp.add
                )
                # bf16 warmup gates: dnb ready at DTW end, gsb ready when
                # the gathered dists land (a few us before the weights)
                dnb = mp.tile([128, NT], BF16)
                nc.scalar.activation(dnb[:], down[:], ACTF.Copy)
                gsb = mp.tile([128, NT], BF16)
                nc.scalar.activation(gsb[:], gdsq[:, 0:NT], ACTF.Copy)

            if stage >= 4:
                # ---- weighted sums as PE matmuls accumulating over tiles
                NTOT = 2 * T * A + 1
                arin = dp.tile([NTOT], F32)
                arout = dp.tile([NTOT], F32)
                nc.sync.dma_start(
                    arin[2 * T * A : NTOT].rearrange("(o f) -> o f", o=1),
                    swr[0:1, 0:1],
                )
                _pst_cm = tc.tile_pool(name="psum_st", bufs=1, space="PSUM")
                pst = _pst_cm.__enter__()
                sts = []
                for c in range(8):
                    st = pst.tile([128, 512], F32, tag=f"st{c}")
                    sts.append(st)
                # PE p-state warmup: junk matmuls gated on the dists; their
                # outputs are reset by the first start=True real matmul.
                for wi in range(WARM + WARM2):
                    wsrc = dnb if wi < WARM else gsb
                    nc.tensor.matmul(
                        sts[wi % 8][0:1, :],
                        wsrc[:, 0:1],
                        actT[:, wi % NT, (wi % 8) * 16 : (wi % 8) * 16 + 16, :],
                        start=True, stop=True, skip_group_check=True,
                    )
                # staging rows alias dead actb (32-aligned partitions)
                arsc = actb[:].rearrange("t p a -> t (p a)").bitcast(F32)
                arsb_m = arsc[0:1, 0 : T * A]
                arsb_s = arsc[32:33, 0 : T * A]
                for c in range(8):
                    for k in range(NT):
                        nc.tensor.matmul(
                            sts[c][0:1, :],
                            wb[:, k : k + 1],
                            actT[:, k, c * 16 : (c + 1) * 16, :],
                            start=(k == 0), stop=(k == NT - 1),
                        )
                    nc.scalar.activation(
                        arsb_m[:, c * 512 : (c + 1) * 512],
                        sts[c][0:1, :], ACTF.Copy,
                    )
                for c in range(8):
                    for k in range(NT):
                        nc.tensor.matmul(
                            sts[c][32:33, :],
                            wb[:, k : k + 1],
                            act2T[:, k, c * 16 : (c + 1) * 16, :],
                            start=(k == 0), stop=(k == NT - 1),
                        )
                    nc.vector.tensor_copy(
                        arsb_s[:, c * 512 : (c + 1) * 512],
                        sts[c][32:33, :],
                    )

                nc.sync.dma_start(
                    arin[0 : T * A].rearrange("(o f) -> o f", o=1), arsb_m[:]
                )
                nc.sync.dma_start(
                    arin[T * A : 2 * T * A].rearrange("(o f) -> o f", o=1),
                    arsb_s[:],
                )
                if single:
                    nc.sync.dma_start(arout[:], arin[:])
                else:
                    nc.gpsimd.collective_compute(
                        "AllReduce",
                        ALU.add,
                        replica_groups=GROUP,
                        ins=[arin.opt()],
                        outs=[arout.opt()],
                    )
                _pst_cm.__exit__(None, None, None)

            if stage >= 5:
                # ---- final statistics
                rn12 = mp.tile([128, 2, A], F32)
                nc.sync.dma_start(
                    rn12[:],
                    arout[0 : 2 * T * A].rearrange(
                        "(q t a) -> t q a", q=2, t=T
                    ),
                )
                rs = mp.tile([128, 1], F32)
                _, rssrc = bass.broadcast_tensor_aps(
                    rs[:],
                    arout[2 * T * A : NTOT].rearrange("(o f) -> o f", o=1),
                )
                nc.sync.dma_start(rs[:], rssrc)
                rn1 = rn12[:, 0]
                rn2 = rn12[:, 1]
                inv = mp.tile([128, 1], F32)
                nc.vector.reciprocal(inv[:], rs[:])
                mh = mp.tile([128, A], F32)
                nc.vector.tensor_scalar(
                    mh[:], rn1, inv[:, 0:1], None, op0=ALU.mult
                )
                q = mp.tile([128, A], F32)
                nc.vector.tensor_scalar(
                    q[:], rn2, inv[:, 0:1], None, op0=ALU.mult
                )
                msq = mp.tile([128, A], F32)
                nc.vector.tensor_tensor(msq[:], mh[:], mh[:], op=ALU.mult)
                var = mp.tile([128, A], F32)
                nc.vector.tensor_tensor(var[:], q[:], msq[:], op=ALU.subtract)
                nc.vector.tensor_scalar(var[:], var[:], 0.0, None, op0=ALU.max)
                stdv = mp.tile([128, A], F32)
                ostk = mp.tile([128, 2, A], F32)
                nc.scalar.sqrt(stdv[:], var[:])
                nc.vector.tensor_scalar(
                    ostk[:, 1], stdv[:], MIN_STD, 1.0, op0=ALU.max, op1=ALU.min
                )
                nc.vector.tensor_scalar(
                    mh[:], mh[:], 1.0 - MOM, None, op0=ALU.mult
                )
                nc.vector.scalar_tensor_tensor(
                    ostk[:, 0], means_t[:], MOM, mh[:], op0=ALU.mult,
                    op1=ALU.add,
                )
                nc.sync.dma_start(
                    out_d.rearrange("q t o a -> t (q o) a"), ostk[:]
                )
            else:
                # bisect debug output
                dbg = mp.tile([128, A], F32)
                nc.vector.memset(dbg[:], 0.0)
                nc.vector.tensor_copy(dbg[:, 0:NT], down[:])
                if stage >= 3:
                    nc.vector.tensor_copy(dbg[:, 4 : 4 + NT], w4[:])
                    nc.vector.tensor_copy(dbg[:, 8:9], thneg[:])
                    nc.vector.tensor_copy(dbg[:, 9:10], swr[:])
                if stage == 2:
                    gdbg = mp.tile([128, A], F32)
                    nc.sync.dma_start(
                        gdbg[:],
                        gd[0 : 128 * A].rearrange("(p a) -> p a", a=A),
                    )
                    nc.vector.tensor_copy(dbg[:, 4:8], gdbg[:, 0:4])
                nc.sync.dma_start(out_d[0, :, 0, :], dbg[:])
                nc.sync.dma_start(out_d[1, :, 0, :], dbg[:])

    nc.compile()
    return nc


def _get_nc(stage=None, single=None):
    if stage is None:
        stage = int(os.environ.get("CEM_STAGE", "9"))
    if single is None:
        single = bool(int(os.environ.get("CEM_SINGLE", "0")))
    key = ("nc", stage, single)
    if key not in _CACHE:
        _CACHE[key] = _build(stage, single)
    return _CACHE[key]


def kernel(**inputs):
    obs = np.ascontiguousarray(np.asarray(inputs["obs_diffs"], np.float32))
    means = np.ascontiguousarray(np.asarray(inputs["means"], np.float32))
    stds = np.ascontiguousarray(np.asarray(inputs["stds"], np.float32))
    noise = np.ascontiguousarray(np.asarray(inputs["noise"], np.float32))

    nc = _get_nc(stage=9, single=False)
    in_maps = []
    for c in range(NCORES):
        in_maps.append(
            {
                "obs": obs[c * PL : (c + 1) * PL],
                "means": means,
                "stds": stds,
                "noise": np.ascontiguousarray(noise[:, c * PL : (c + 1) * PL, :]),
            }
        )
    res = bass_utils.run_bass_kernel_spmd(
        nc, in_maps, core_ids=list(range(NCORES))
    )
    out = np.asarray(res.results[0]["out"], np.float32)
    return out.reshape(2, T, 1, A)


# revision 39
# speedup vs baseline: 1.1348x; 1.0114x over previous
"""CEM sampling kernel for Trainium2, 8-core SPMD (population sharded).

Per core (512 of 4096 members), one fused program:

  Window (overlapped with the 42MB obs+noise HBM stream, ~117us):
   - DTW min-plus DP entirely on DVE (the scan/min ops exist only
     there): two packed pair-chains [t0|sep|t1] and [t2|sep|t3], DP
     state in fp16 (2x-mode mins; the scan's carry is internally fp32
     and the f32 cost rows are never rounded, so only the stored row
     values quantize).  ~1.1us/row.
   - Actions: ACT computes bf16 act = means + stds*noise per action
     dim, Pool clips in bf16, PE transposes [t,p] blocks to a
     population-major bf16 layout, ACT copies PSUM->SBUF and squares.
  Tail (~35us): AllGather dists; top-K via the gpsimd kth_largest
     library op on the [128,32] negated global dists (exact K-th
     threshold, replaces rank compares and broadcasts); weights; the
     weighted mean / E[x^2] reductions as 64 bf16 PE matmuls (with a
     p-state warmup) accumulating in PSUM; AllReduce; closing stats.
"""

import os
import sys

for _p in ("/opt/trn_rl_repo", "/root/.axon_site/_ro/trn_rl_repo"):
    if _p not in sys.path:
        sys.path.insert(0, _p)

import numpy as np

import concourse.bass as bass
import concourse.bacc as bacc
import concourse.bass_isa as bass_isa
import concourse.tile as tile
from concourse import mybir
from concourse import bass_utils
from concourse.masks import make_identity

F32 = mybir.dt.float32
FP16 = mybir.dt.float16
BF16 = mybir.dt.bfloat16
ALU = mybir.AluOpType
ACTF = mybir.ActivationFunctionType

P, T, A = 4096, 128, 32
NCORES = 8
PL = P // NCORES          # 512 population per core
NT = PL // 128            # 4 tiles of 128 on the partition dim
K = int(P * 0.1)          # 409
TEMP, MOM, MIN_STD = 0.5, 0.1, 0.05
INFDP = 30000.0           # fp16-safe stand-in for +inf in the DP
RCH = int(os.environ.get("CEM_RCH", "8"))   # DP rows per streamed chunk
_C0 = int(os.environ.get("CEM_C0", "4"))    # optional small first chunk
CROWS = ([_C0, RCH - _C0] if _C0 else []) + [RCH] * ((T - (RCH if _C0 else 0)) // RCH)
COFF = [sum(CROWS[:i]) for i in range(len(CROWS))]
NCHUNK = len(CROWS)
CBUFS = int(os.environ.get("CEM_CBUFS", "4"))
WARM = int(os.environ.get("CEM_WARM", "8"))  # PE p-state warmup matmuls
WARM2 = int(os.environ.get("CEM_WARM2", "5"))  # late warmups gated on gdsq
DPDT = FP16 if os.environ.get("CEM_DPDT", "fp16") == "fp16" else F32
R16 = int(os.environ.get("CEM_R16", "96"))  # rows in fp16 before f32
if DPDT == F32:
    R16 = 0
GROUP = [list(range(NCORES))]

# packed cost-row layout: [t0(128) sep t1(128) | t2(128) sep t3(128)]
CW = 257                  # cost width of one pair-chain
CWF = 514
SEP1, SEP2 = 128, 385
DMAP = {0: 0, 1: 129, 2: 257, 3: 386}  # pop tile -> flat cost column

_CACHE = {}


def _build(stage=9, single=False):
    nc = bacc.Bacc(
        "TRN2",
        target_bir_lowering=False,
        debug=False,
        num_devices=1 if single else NCORES,
    )
    obs_d = nc.dram_tensor("obs", [PL, T, T], F32, kind="ExternalInput")
    means_d = nc.dram_tensor("means", [T, 1, A], F32, kind="ExternalInput")
    stds_d = nc.dram_tensor("stds", [T, 1, A], F32, kind="ExternalInput")
    noise_d = nc.dram_tensor("noise", [T, PL, A], F32, kind="ExternalInput")
    out_d = nc.dram_tensor("out", [2, T, 1, A], F32, kind="ExternalOutput")

    with tile.TileContext(nc) as tc:
        with (
            tc.tile_pool(name="main", bufs=1) as mp,
            tc.tile_pool(name="dram", bufs=1, space="DRAM") as dp,
        ):
            # ---- small persistent tiles
            means_t = mp.tile([T, A], F32)
            stds_t = mp.tile([T, A], F32)
            nc.sync.dma_start(means_t[:], means_d[:, 0, :])
            nc.sync.dma_start(stds_t[:], stds_d[:, 0, :])
            ident = mp.tile([128, 128], BF16)
            make_identity(nc, ident[:])
            # preload the ACT function tables used in the tail
            warmt = mp.tile([128, 1], F32)
            nc.scalar.activation(warmt[:], means_t[:, 0:1], ACTF.Exp)
            nc.scalar.sqrt(warmt[:], warmt[:])

            # actions (bf16), noise staging quarters, transposed layouts
            actb = mp.tile([T, PL, A], BF16)
            utile = mp.tile([128, 2 * PL * A // 4], F32)  # [128, 8192]
            nhq = [
                utile[:, 0:4096].rearrange("t (p a) -> t p a", a=A),
                utile[:, 4096:8192].rearrange("t (p a) -> t p a", a=A),
            ]
            actT = mp.tile([128, NT, T, A], BF16)
            act2T = mp.tile([128, NT, T, A], BF16)

            # ---- DTW state: two packed pair-chains, ping-pong.  Rows
            # < R16 keep the DP values in fp16 (2x-mode mins); the last
            # rows -- where the absolute values and hence fp16 quanta are
            # largest -- run in f32 so the accumulated rounding stays small.
            h01a = mp.tile([128, CW + 1], FP16)
            h01b = mp.tile([128, CW + 1], FP16)
            h23a = mp.tile([128, CW + 1], FP16)
            h23b = mp.tile([128, CW + 1], FP16)
            f01a = mp.tile([128, CW + 1], F32)
            f01b = mp.tile([128, CW + 1], F32)
            f23a = mp.tile([128, CW + 1], F32)
            f23b = mp.tile([128, CW + 1], F32)
            ub01 = mp.tile([128, CW], FP16)
            ub23 = mp.tile([128, CW], FP16)
            uf01 = mp.tile([128, CW], F32)
            uf23 = mp.tile([128, CW], F32)
            for t_ in (h01a, h01b, h23a, h23b, f01a, f01b, f23a, f23b):
                nc.vector.memset(t_[:], INFDP)
            # D[0][0] = 0 for each tile (pair cols 0 and 129)
            nc.vector.memset(h01a[:, 0:1], 0.0)
            nc.vector.memset(h01a[:, 129:130], 0.0)
            nc.vector.memset(h23a[:, 0:1], 0.0)
            nc.vector.memset(h23a[:, 129:130], 0.0)
            down = mp.tile([128, NT], F32)
            ch01 = (h01a, h01b)
            ch23 = (h23a, h23b)
            cf01 = (f01a, f01b)
            cf23 = (f23a, f23b)

            def dtw_row(i, cb, r):
                crow = cb[:, r]
                # row i reads the side written at i-1: fp16 through row R16,
                # f32 after; the switch row reads fp16 and writes f32.  The
                # f32 pair's col 0 is INFDP from init and never rewritten.
                A1 = (ch01 if i <= R16 else cf01)[i % 2]
                A2 = (ch23 if i <= R16 else cf23)[i % 2]
                if i < R16:
                    B1, B2, u1, u2 = (
                        ch01[(i + 1) % 2], ch23[(i + 1) % 2], ub01, ub23)
                else:
                    B1, B2, u1, u2 = (
                        cf01[(i + 1) % 2], cf23[(i + 1) % 2], uf01, uf23)
                nc.vector.tensor_tensor(
                    u1[:], A1[:, 0:CW], A1[:, 1 : CW + 1], op=ALU.min
                )
                nc.vector.tensor_tensor(
                    u2[:], A2[:, 0:CW], A2[:, 1 : CW + 1], op=ALU.min
                )
                nc.vector.tensor_tensor_scan(
                    B1[:, 1 : CW + 1], u1[:], crow[:, 0:CW], INFDP,
                    op0=ALU.min, op1=ALU.add,
                )
                nc.vector.tensor_tensor_scan(
                    B2[:, 1 : CW + 1], u2[:], crow[:, CW:CWF], INFDP,
                    op0=ALU.min, op1=ALU.add,
                )
                if i == 0:
                    # D[i>0][0] = INF at the never-rewritten left columns
                    nc.vector.memset(h01a[:, 0:1], INFDP)
                    nc.vector.memset(h23a[:, 0:1], INFDP)

            # ---- actions pipeline pieces (emitted interleaved with DTW)
            def noise_dma(q):
                nc.sync.dma_start(
                    nhq[q % 2][:], noise_d[:, q * 128 : (q + 1) * 128, :]
                )

            def affine(q):
                for a in range(A):
                    nc.scalar.activation(
                        actb[:, q * 128 : (q + 1) * 128, a],
                        nhq[q % 2][:, :, a],
                        ACTF.Identity,
                        bias=means_t[:, a : a + 1],
                        scale=stds_t[:, a : a + 1],
                    )

            def clip(k):
                v = actb[:, k * 128 : (k + 1) * 128, :].rearrange(
                    "t p a -> t (p a)"
                )
                nc.gpsimd.tensor_scalar(
                    v, v, 1.0, -1.0, op0=ALU.min, op1=ALU.max
                )

            def transposes(tpp, k):
                for a in range(A):
                    pt = tpp.tile([128, 128], BF16, tag="tp")
                    nc.tensor.transpose(
                        pt[:],
                        actb[:, k * 128 : (k + 1) * 128, a],
                        ident[:],
                    )
                    nc.scalar.activation(
                        actT[:, k, :, a], pt[:], ACTF.Copy
                    )

            def square(k):
                nc.scalar.activation(
                    act2T[:, k].rearrange("t a b -> t (a b)"),
                    actT[:, k].rearrange("t a b -> t (a b)"),
                    ACTF.Square,
                )

            # ---- window: obs chunks + DTW rows + action stages
            with tc.tile_pool(name="cwin", bufs=CBUFS) as cp, \
                 tc.tile_pool(name="psum_tp", bufs=4, space="PSUM") as tpp:
                cbs = []

                def chunk_dma(c):
                    rows = CROWS[c]
                    cb = cp.tile([128, RCH, CWF], F32, tag="cw")
                    for k in range(NT):
                        o = DMAP[k]
                        nc.sync.dma_start(
                            cb[:, 0:rows, o : o + T],
                            obs_d[k * 128 : (k + 1) * 128,
                                  COFF[c] : COFF[c] + rows, :],
                        )
                    # refresh both INF separators each generation
                    nc.gpsimd.memset(cb[:, 0:rows, SEP1 : SEP1 + 1], INFDP)
                    nc.gpsimd.memset(cb[:, 0:rows, SEP2 : SEP2 + 1], INFDP)
                    return cb

                # prime obs chunks ahead of the first noise quarter so
                # the DTW never starves during pipeline fill
                for c in range(min(3, CBUFS, NCHUNK)):
                    cbs.append(chunk_dma(c))
                noise_dma(0)
                for c in range(3, min(CBUFS, NCHUNK)):
                    cbs.append(chunk_dma(c))

                acts = {
                    0: [lambda: affine(0)],
                    1: [lambda: noise_dma(1), lambda: affine(1)],
                    2: [lambda: noise_dma(2), lambda: clip(0)],
                    3: [lambda: affine(2), lambda: clip(1)],
                    4: [lambda: noise_dma(3),
                        lambda: transposes(tpp, 0)],
                    5: [lambda: affine(3), lambda: clip(2)],
                    6: [lambda: transposes(tpp, 1), lambda: square(0)],
                    7: [lambda: clip(3)],
                    8: [lambda: transposes(tpp, 2), lambda: square(1)],
                    9: [lambda: transposes(tpp, 3)],
                    10: [lambda: square(2)],
                    11: [lambda: square(3)],
                }

                next_key = 0
                for c in range(NCHUNK):
                    cb = cbs[c]
                    for r in range(CROWS[c]):
                        dtw_row(COFF[c] + r, cb, r)
                    if c + CBUFS < NCHUNK:
                        cbs.append(chunk_dma(c + CBUFS))
                    if stage >= 1:
                        # acts keyed by 8-row octiles of emitted DP rows
                        done = COFF[c] + CROWS[c]
                        while next_key * 8 + 8 <= done:
                            for th in acts.get(next_key, []):
                                th()
                            next_key += 1

            # own dists from the final (even-side) f32 buffers
            nc.scalar.activation(down[:, 0:1], f01a[:, 128:129], ACTF.Copy)
            nc.scalar.activation(down[:, 1:2], f01a[:, 257:258], ACTF.Copy)
            nc.scalar.activation(down[:, 2:3], f23a[:, 128:129], ACTF.Copy)
            nc.scalar.activation(down[:, 3:4], f23a[:, 257:258], ACTF.Copy)

            if stage >= 2:
                # ---- AllGather dists (tiny)
                ld = dp.tile([PL], F32)
                gd = dp.tile([P], F32)
                # member order in gd is irrelevant (kth/threshold are
                # order-free), so write ld partition-major: fewer descs
                nc.sync.dma_start(ld.rearrange("(p k) -> p k", k=NT), down[:])
                if single:
                    _, lsrc = bass.broadcast_tensor_aps(
                        gd.rearrange("(r f) -> r f", r=NCORES),
                        ld.rearrange("(o f) -> o f", o=1),
                    )
                    nc.sync.dma_start(
                        gd.rearrange("(r f) -> r f", r=NCORES), lsrc
                    )
                else:
                    nc.gpsimd.collective_compute(
                        "AllGather",
                        ALU.bypass,
                        replica_groups=GROUP,
                        ins=[ld.opt()],
                        outs=[gd.opt()],
                    )

            if stage >= 3:
                # ---- top-K threshold via gpsimd kth_largest on -dists
                gdsq = mp.tile([128, P // 128], F32)
                nc.sync.dma_start(
                    gdsq[:], gd.rearrange("(p f) -> p f", p=128)
                )
                ngd = mp.tile([128, P // 128], F32)
                nc.vector.tensor_scalar(
                    ngd[:], gdsq[:], -1.0, None, op0=ALU.mult
                )
                kth = mp.tile([128, 2], F32)
                nc.gpsimd.kth_largest(
                    kth[:], ngd[:], P // 128, K + 3,
                    quantile=1.0 - (K - 0.5) / (P - 1),
                )
                # kth col1 = desc[k_adj+1] = -s[K] ; mask = d < s[K]
                thb = mp.tile([128, 2], F32)
                nc.gpsimd.partition_broadcast(thb[:], kth[0:1, :])
                thneg = mp.tile([128, 1], F32)
                nc.vector.tensor_scalar(
                    thneg[:], thb[:, 1:2], -1.0, None, op0=ALU.mult
                )
                # softmax shift: any global constant cancels exactly; use
                # gd[0] (safe unless the dist spread nears 176/TEMP).
                dref = mp.tile([128, 1], F32)
                nc.gpsimd.partition_broadcast(dref[:], gdsq[0:1, 0:1])
                biast = mp.tile([128, 1], F32)
                nc.gpsimd.tensor_scalar(
                    biast[:], dref[:], TEMP, None, op0=ALU.mult
                )
                mask4 = mp.tile([128, NT], F32)
                nc.vector.tensor_scalar(
                    mask4[:], down[:], thneg[:, 0:1], None, op0=ALU.is_lt
                )
                e4 = mp.tile([128, NT], F32)
                nc.scalar.activation(
                    e4[:], down[:], ACTF.Exp, bias=biast[:, 0:1], scale=-TEMP
                )
                w4 = mp.tile([128, NT], F32)
                nc.vector.tensor_tensor(w4[:], e4[:], mask4[:], op=ALU.mult)
                wb = mp.tile([128, NT], BF16)
                nc.scalar.activation(wb[:], w4[:], ACTF.Copy)
                # sum of weights across members (free then partitions)
                slocal = mp.tile([128, 1], F32)
                nc.vector.tensor_reduce(
                    slocal[:], w4[:], axis=mybir.AxisListType.X, op=ALU.add
                )
                swr = mp.tile([128, 1], F32)
                nc.gpsimd.partition_all_reduce(
                    swr[:], slocal[:], 128, bass_isa.ReduceOp.add
                )
                # bf16 warmup gates: dnb ready at DTW end, gsb ready when
                # the gathered dists land (a few us before the weights)
                dnb = mp.tile([128, NT], BF16)
                nc.scalar.activation(dnb[:], down[:], ACTF.Copy)
                gsb = mp.tile([128, NT], BF16)
                nc.scalar.activation(gsb[:], gdsq[:, 0:NT], ACTF.Copy)

            if stage >= 4:
                # ---- weighted sums as PE matmuls accumulating over tiles
                NTOT = 2 * T * A + 1
                arin = dp.tile([NTOT], F32)
                arout = dp.tile([NTOT], F32)
                nc.sync.dma_start(
                    arin[2 * T * A : NTOT].rearrange("(o f) -> o f", o=1),
                    swr[0:1, 0:1],
                )
                _pst_cm = tc.tile_pool(name="psum_st", bufs=1, space="PSUM")
                pst = _pst_cm.__enter__()
                sts = []
                for c in range(8):
                    st = pst.tile([128, 512], F32, tag=f"st{c}")
                    sts.append(st)
                # PE p-state warmup: junk matmuls gated on the dists; their
                # outputs are reset by the first start=True real matmul.
                for wi in range(WARM + WARM2):
                    wsrc = dnb if wi < WARM else gsb
                    nc.tensor.matmul(
                        sts[wi % 8][0:1, :],
                        wsrc[:, 0:1],
                        actT[:, wi % NT, (wi % 8) * 16 : (wi % 8) * 16 + 16, :],
                        start=True, stop=True, skip_group_check=True,
                    )
                # staging rows alias dead actb (32-aligned partitions)
                arsc = actb[:].rearrange("t p a -> t (p a)").bitcast(F32)
                arsb_m = arsc[0:1, 0 : T * A]
                arsb_s = arsc[32:33, 0 : T * A]
                for c in range(8):
                    for k in range(NT):
                        nc.tensor.matmul(
                            sts[c][0:1, :],
                            wb[:, k : k + 1],
                            actT[:, k, c * 16 : (c + 1) * 16, :],
                            start=(k == 0), stop=(k == NT - 1),
                        )
                    nc.scalar.activation(
                        arsb_m[:, c * 512 : (c + 1) * 512],
                        sts[c][0:1, :], ACTF.Copy,
                    )
                for c in range(8):
                    for k in range(NT):
                        nc.tensor.matmul(
                            sts[c][32:33, :],
                            wb[:, k : k + 1],
                            act2T[:, k, c * 16 : (c + 1) * 16, :],
                            start=(k == 0), stop=(k == NT - 1),
                        )
                    nc.vector.tensor_copy(
                        arsb_s[:, c * 512 : (c + 1) * 512],
                        sts[c][32:33, :],
                    )

                nc.sync.dma_start(
                    arin[0 : T * A].rearrange("(o f) -> o f", o=1), arsb_m[:]
                )
                nc.sync.dma_start(
                    arin[T * A : 2 * T * A].rearrange("(o f) -> o f", o=1),
                    arsb_s[:],
                )
                if single:
                    nc.sync.dma_start(arout[:], arin[:])
                else:
                    nc.gpsimd.collective_compute(
                        "AllReduce",
                        ALU.add,
                        replica_groups=GROUP,
                        ins=[arin.opt()],
                        outs=[arout.opt()],
                    )
                _pst_cm.__exit__(None, None, None)

            if stage >= 5:
                # ---- final statistics
                rn12 = mp.tile([128, 2, A], F32)
                nc.sync.dma_start(
                    rn12[:],
                    arout[0 : 2 * T * A].rearrange(
                        "(q t a) -> t q a", q=2, t=T
                    ),
                )
                rs = mp.tile([128, 1], F32)
                _, rssrc = bass.broadcast_tensor_aps(
                    rs[:],
                    arout[2 * T * A : NTOT].rearrange("(o f) -> o f", o=1),
                )
                nc.sync.dma_start(rs[:], rssrc)
                rn1 = rn12[:, 0]
                rn2 = rn12[:, 1]
                inv = mp.tile([128, 1], F32)
                nc.vector.reciprocal(inv[:], rs[:])
                mh = mp.tile([128, A], F32)
                nc.vector.tensor_scalar(
                    mh[:], rn1, inv[:, 0:1], None, op0=ALU.mult
                )
                q = mp.tile([128, A], F32)
                nc.vector.tensor_scalar(
                    q[:], rn2, inv[:, 0:1], None, op0=ALU.mult
                )
                msq = mp.tile([128, A], F32)
                nc.vector.tensor_tensor(msq[:], mh[:], mh[:], op=ALU.mult)
                var = mp.tile([128, A], F32)
                nc.vector.tensor_tensor(var[:], q[:], msq[:], op=ALU.subtract)
                nc.vector.tensor_scalar(var[:], var[:], 0.0, None, op0=ALU.max)
                stdv = mp.tile([128, A], F32)
                ostk = mp.tile([128, 2, A], F32)
                nc.scalar.sqrt(stdv[:], var[:])
                nc.vector.tensor_scalar(
                    ostk[:, 1], stdv[:], MIN_STD, 1.0, op0=ALU.max, op1=ALU.min
                )
                nc.vector.tensor_scalar(
                    mh[:], mh[:], 1.0 - MOM, None, op0=ALU.mult
                )
                nc.vector.scalar_tensor_tensor(
                    ostk[:, 0], means_t[:], MOM, mh[:], op0=ALU.mult,
                    op1=ALU.add,
                )
                nc.sync.dma_start(
                    out_d.rearrange("q t o a -> t (q o) a"), ostk[:]
                )
            else:
                # bisect debug output
                dbg = mp.tile([128, A], F32)
                nc.vector.memset(dbg[:], 0.0)
                nc.vector.tensor_copy(dbg[:, 0:NT], down[:])
                if stage >= 3:
                    nc.vector.tensor_copy(dbg[:, 4 : 4 + NT], w4[:])
                    nc.vector.tensor_copy(dbg[:, 8:9], thneg[:])
                    nc.vector.tensor_copy(dbg[:, 9:10], swr[:])
                if stage == 2:
                    gdbg = mp.tile([128, A], F32)
                    nc.sync.dma_start(
                        gdbg[:],
                        gd[0 : 128 * A].rearrange("(p a) -> p a", a=A),
                    )
                    nc.vector.tensor_copy(dbg[:, 4:8], gdbg[:, 0:4])
                nc.sync.dma_start(out_d[0, :, 0, :], dbg[:])
                nc.sync.dma_start(out_d[1, :, 0, :], dbg[:])

    nc.compile()
    return nc


def _get_nc(stage=None, single=None):
    if stage is None:
        stage = int(os.environ.get("CEM_STAGE", "9"))
    if single is None:
        single = bool(int(os.environ.get("CEM_SINGLE", "0")))
    key = ("nc", stage, single)
    if key not in _CACHE:
        _CACHE[key] = _build(stage, single)
    return _CACHE[key]


def kernel(**inputs):
    obs = np.ascontiguousarray(np.asarray(inputs["obs_diffs"], np.float32))
    means = np.ascontiguousarray(np.asarray(inputs["means"], np.float32))
    stds = np.ascontiguousarray(np.asarray(inputs["stds"], np.float32))
    noise = np.ascontiguousarray(np.asarray(inputs["noise"], np.float32))

    nc = _get_nc(stage=9, single=False)
    in_maps = []
    for c in range(NCORES):
        in_maps.append(
            {
                "obs": obs[c * PL : (c + 1) * PL],
                "means": means,
                "stds": stds,
                "noise": np.ascontiguousarray(noise[:, c * PL : (c + 1) * PL, :]),
            }
        )
    res = bass_utils.run_bass_kernel_spmd(
        nc, in_maps, core_ids=list(range(NCORES))
    )
    out = np.asarray(res.results[0]["out"], np.float32)
    return out.reshape(2, T, 1, A)


# revision 41
# speedup vs baseline: 1.1358x; 1.0009x over previous
"""CEM sampling kernel for Trainium2, 8-core SPMD (population sharded).

Per core (512 of 4096 members), one fused program:

  Window (overlapped with the 42MB obs+noise HBM stream, ~117us):
   - DTW min-plus DP entirely on DVE (the scan/min ops exist only
     there): two packed pair-chains [t0|sep|t1] and [t2|sep|t3], DP
     state in fp16 (2x-mode mins; the scan's carry is internally fp32
     and the f32 cost rows are never rounded, so only the stored row
     values quantize).  ~1.1us/row.
   - Actions: ACT computes bf16 act = means + stds*noise per action
     dim, Pool clips in bf16, PE transposes [t,p] blocks to a
     population-major bf16 layout, ACT copies PSUM->SBUF and squares.
  Tail (~35us): AllGather dists; top-K via the gpsimd kth_largest
     library op on the [128,32] negated global dists (exact K-th
     threshold, replaces rank compares and broadcasts); weights; the
     weighted mean / E[x^2] reductions as 64 bf16 PE matmuls (with a
     p-state warmup) accumulating in PSUM; AllReduce; closing stats.
"""

import os
import sys

for _p in ("/opt/trn_rl_repo", "/root/.axon_site/_ro/trn_rl_repo"):
    if _p not in sys.path:
        sys.path.insert(0, _p)

import numpy as np

import concourse.bass as bass
import concourse.bacc as bacc
import concourse.bass_isa as bass_isa
import concourse.tile as tile
from concourse import mybir
from concourse import bass_utils
from concourse.masks import make_identity

F32 = mybir.dt.float32
FP16 = mybir.dt.float16
BF16 = mybir.dt.bfloat16
ALU = mybir.AluOpType
ACTF = mybir.ActivationFunctionType

P, T, A = 4096, 128, 32
NCORES = 8
PL = P // NCORES          # 512 population per core
NT = PL // 128            # 4 tiles of 128 on the partition dim
K = int(P * 0.1)          # 409
TEMP, MOM, MIN_STD = 0.5, 0.1, 0.05
INFDP = 30000.0           # fp16-safe stand-in for +inf in the DP
RCH = int(os.environ.get("CEM_RCH", "8"))   # DP rows per streamed chunk
_C0 = int(os.environ.get("CEM_C0", "4"))    # optional small first chunk
CROWS = ([_C0, RCH - _C0] if _C0 else []) + [RCH] * ((T - (RCH if _C0 else 0)) // RCH)
COFF = [sum(CROWS[:i]) for i in range(len(CROWS))]
NCHUNK = len(CROWS)
CBUFS = int(os.environ.get("CEM_CBUFS", "4"))
WARM = int(os.environ.get("CEM_WARM", "8"))  # PE p-state warmup matmuls
WARM2 = int(os.environ.get("CEM_WARM2", "3"))  # late warmups gated on gdsq
DPDT = FP16 if os.environ.get("CEM_DPDT", "fp16") == "fp16" else F32
R16 = int(os.environ.get("CEM_R16", "104"))  # rows in fp16 before f32
if DPDT == F32:
    R16 = 0
GROUP = [list(range(NCORES))]

# packed cost-row layout: [t0(128) sep t1(128) | t2(128) sep t3(128)]
CW = 257                  # cost width of one pair-chain
CWF = 514
SEP1, SEP2 = 128, 385
DMAP = {0: 0, 1: 129, 2: 257, 3: 386}  # pop tile -> flat cost column

_CACHE = {}


def _build(stage=9, single=False):
    nc = bacc.Bacc(
        "TRN2",
        target_bir_lowering=False,
        debug=False,
        num_devices=1 if single else NCORES,
    )
    obs_d = nc.dram_tensor("obs", [PL, T, T], F32, kind="ExternalInput")
    means_d = nc.dram_tensor("means", [T, 1, A], F32, kind="ExternalInput")
    stds_d = nc.dram_tensor("stds", [T, 1, A], F32, kind="ExternalInput")
    noise_d = nc.dram_tensor("noise", [T, PL, A], F32, kind="ExternalInput")
    out_d = nc.dram_tensor("out", [2, T, 1, A], F32, kind="ExternalOutput")

    with tile.TileContext(nc) as tc:
        with (
            tc.tile_pool(name="main", bufs=1) as mp,
            tc.tile_pool(name="dram", bufs=1, space="DRAM") as dp,
        ):
            # ---- small persistent tiles
            means_t = mp.tile([T, A], F32)
            stds_t = mp.tile([T, A], F32)
            nc.sync.dma_start(means_t[:], means_d[:, 0, :])
            nc.sync.dma_start(stds_t[:], stds_d[:, 0, :])
            ident = mp.tile([128, 128], BF16)
            make_identity(nc, ident[:])
            # preload the ACT function tables used in the tail
            warmt = mp.tile([128, 1], F32)
            nc.scalar.activation(warmt[:], means_t[:, 0:1], ACTF.Exp)
            nc.scalar.sqrt(warmt[:], warmt[:])

            # actions (bf16), noise staging quarters, transposed layouts
            actb = mp.tile([T, PL, A], BF16)
            utile = mp.tile([128, 2 * PL * A // 4], F32)  # [128, 8192]
            nhq = [
                utile[:, 0:4096].rearrange("t (p a) -> t p a", a=A),
                utile[:, 4096:8192].rearrange("t (p a) -> t p a", a=A),
            ]
            actT = mp.tile([128, NT, T, A], BF16)
            act2T = mp.tile([128, NT, T, A], BF16)

            # ---- DTW state: two packed pair-chains, ping-pong.  Rows
            # < R16 keep the DP values in fp16 (2x-mode mins); the last
            # rows -- where the absolute values and hence fp16 quanta are
            # largest -- run in f32 so the accumulated rounding stays small.
            h01a = mp.tile([128, CW + 1], FP16)
            h01b = mp.tile([128, CW + 1], FP16)
            h23a = mp.tile([128, CW + 1], FP16)
            h23b = mp.tile([128, CW + 1], FP16)
            f01a = mp.tile([128, CW + 1], F32)
            f01b = mp.tile([128, CW + 1], F32)
            f23a = mp.tile([128, CW + 1], F32)
            f23b = mp.tile([128, CW + 1], F32)
            ub01 = mp.tile([128, CW], FP16)
            ub23 = mp.tile([128, CW], FP16)
            uf01 = mp.tile([128, CW], F32)
            uf23 = mp.tile([128, CW], F32)
            for t_ in (h01a, h01b, h23a, h23b, f01a, f01b, f23a, f23b):
                nc.vector.memset(t_[:], INFDP)
            # D[0][0] = 0 for each tile (pair cols 0 and 129)
            nc.vector.memset(h01a[:, 0:1], 0.0)
            nc.vector.memset(h01a[:, 129:130], 0.0)
            nc.vector.memset(h23a[:, 0:1], 0.0)
            nc.vector.memset(h23a[:, 129:130], 0.0)
            down = mp.tile([128, NT], F32)
            ch01 = (h01a, h01b)
            ch23 = (h23a, h23b)
            cf01 = (f01a, f01b)
            cf23 = (f23a, f23b)

            def dtw_row(i, cb, r):
                crow = cb[:, r]
                # row i reads the side written at i-1: fp16 through row R16,
                # f32 after; the switch row reads fp16 and writes f32.  The
                # f32 pair's col 0 is INFDP from init and never rewritten.
                A1 = (ch01 if i <= R16 else cf01)[i % 2]
                A2 = (ch23 if i <= R16 else cf23)[i % 2]
                if i < R16:
                    B1, B2, u1, u2 = (
                        ch01[(i + 1) % 2], ch23[(i + 1) % 2], ub01, ub23)
                else:
                    B1, B2, u1, u2 = (
                        cf01[(i + 1) % 2], cf23[(i + 1) % 2], uf01, uf23)
                nc.vector.tensor_tensor(
                    u1[:], A1[:, 0:CW], A1[:, 1 : CW + 1], op=ALU.min
                )
                nc.vector.tensor_tensor(
                    u2[:], A2[:, 0:CW], A2[:, 1 : CW + 1], op=ALU.min
                )
                nc.vector.tensor_tensor_scan(
                    B1[:, 1 : CW + 1], u1[:], crow[:, 0:CW], INFDP,
                    op0=ALU.min, op1=ALU.add,
                )
                nc.vector.tensor_tensor_scan(
                    B2[:, 1 : CW + 1], u2[:], crow[:, CW:CWF], INFDP,
                    op0=ALU.min, op1=ALU.add,
                )
                if i == 0:
                    # D[i>0][0] = INF at the never-rewritten left columns
                    nc.vector.memset(h01a[:, 0:1], INFDP)
                    nc.vector.memset(h23a[:, 0:1], INFDP)

            # ---- actions pipeline pieces (emitted interleaved with DTW)
            def noise_dma(q):
                nc.sync.dma_start(
                    nhq[q % 2][:], noise_d[:, q * 128 : (q + 1) * 128, :]
                )

            def affine(q):
                for a in range(A):
                    nc.scalar.activation(
                        actb[:, q * 128 : (q + 1) * 128, a],
                        nhq[q % 2][:, :, a],
                        ACTF.Identity,
                        bias=means_t[:, a : a + 1],
                        scale=stds_t[:, a : a + 1],
                    )

            def clip(k):
                v = actb[:, k * 128 : (k + 1) * 128, :].rearrange(
                    "t p a -> t (p a)"
                )
                nc.gpsimd.tensor_scalar(
                    v, v, 1.0, -1.0, op0=ALU.min, op1=ALU.max
                )

            def transposes(tpp, k):
                for a in range(A):
                    pt = tpp.tile([128, 128], BF16, tag="tp")
                    nc.tensor.transpose(
                        pt[:],
                        actb[:, k * 128 : (k + 1) * 128, a],
                        ident[:],
                    )
                    nc.scalar.activation(
                        actT[:, k, :, a], pt[:], ACTF.Copy
                    )

            def square(k):
                nc.scalar.activation(
                    act2T[:, k].rearrange("t a b -> t (a b)"),
                    actT[:, k].rearrange("t a b -> t (a b)"),
                    ACTF.Square,
                )

            # ---- window: obs chunks + DTW rows + action stages
            with tc.tile_pool(name="cwin", bufs=CBUFS) as cp, \
                 tc.tile_pool(name="psum_tp", bufs=4, space="PSUM") as tpp:
                cbs = []

                def chunk_dma(c):
                    rows = CROWS[c]
                    cb = cp.tile([128, RCH, CWF], F32, tag="cw")
                    for k in range(NT):
                        o = DMAP[k]
                        nc.sync.dma_start(
                            cb[:, 0:rows, o : o + T],
                            obs_d[k * 128 : (k + 1) * 128,
                                  COFF[c] : COFF[c] + rows, :],
                        )
                    # refresh both INF separators each generation
                    nc.gpsimd.memset(cb[:, 0:rows, SEP1 : SEP1 + 1], INFDP)
                    nc.gpsimd.memset(cb[:, 0:rows, SEP2 : SEP2 + 1], INFDP)
                    return cb

                # prime obs chunks ahead of the first noise quarter so
                # the DTW never starves during pipeline fill
                for c in range(min(3, CBUFS, NCHUNK)):
                    cbs.append(chunk_dma(c))
                noise_dma(0)
                for c in range(3, min(CBUFS, NCHUNK)):
                    cbs.append(chunk_dma(c))

                acts = {
                    0: [lambda: affine(0)],
                    1: [lambda: noise_dma(1), lambda: affine(1)],
                    2: [lambda: noise_dma(2), lambda: clip(0)],
                    3: [lambda: affine(2), lambda: clip(1)],
                    4: [lambda: noise_dma(3),
                        lambda: transposes(tpp, 0)],
                    5: [lambda: affine(3), lambda: clip(2)],
                    6: [lambda: transposes(tpp, 1), lambda: square(0)],
                    7: [lambda: clip(3)],
                    8: [lambda: transposes(tpp, 2), lambda: square(1)],
                    9: [lambda: transposes(tpp, 3)],
                    10: [lambda: square(2)],
                    11: [lambda: square(3)],
                }

                next_key = 0
                for c in range(NCHUNK):
                    cb = cbs[c]
                    for r in range(CROWS[c]):
                        dtw_row(COFF[c] + r, cb, r)
                    if c + CBUFS < NCHUNK:
                        cbs.append(chunk_dma(c + CBUFS))
                    if stage >= 1:
                        # acts keyed by 8-row octiles of emitted DP rows
                        done = COFF[c] + CROWS[c]
                        while next_key * 8 + 8 <= done:
                            for th in acts.get(next_key, []):
                                th()
                            next_key += 1

            # own dists from the final (even-side) f32 buffers
            nc.scalar.activation(down[:, 0:1], f01a[:, 128:129], ACTF.Copy)
            nc.scalar.activation(down[:, 1:2], f01a[:, 257:258], ACTF.Copy)
            nc.scalar.activation(down[:, 2:3], f23a[:, 128:129], ACTF.Copy)
            nc.scalar.activation(down[:, 3:4], f23a[:, 257:258], ACTF.Copy)

            if stage >= 2:
                # ---- AllGather dists (tiny)
                ld = dp.tile([PL], F32)
                gd = dp.tile([P], F32)
                # member order in gd is irrelevant (kth/threshold are
                # order-free), so write ld partition-major: fewer descs
                nc.sync.dma_start(ld.rearrange("(p k) -> p k", k=NT), down[:])
                if single:
                    _, lsrc = bass.broadcast_tensor_aps(
                        gd.rearrange("(r f) -> r f", r=NCORES),
                        ld.rearrange("(o f) -> o f", o=1),
                    )
                    nc.sync.dma_start(
                        gd.rearrange("(r f) -> r f", r=NCORES), lsrc
                    )
                else:
                    nc.gpsimd.collective_compute(
                        "AllGather",
                        ALU.bypass,
                        replica_groups=GROUP,
                        ins=[ld.opt()],
                        outs=[gd.opt()],
                    )

            if stage >= 3:
                # ---- top-K threshold via gpsimd kth_largest on -dists
                gdsq = mp.tile([128, P // 128], F32)
                nc.sync.dma_start(
                    gdsq[:], gd.rearrange("(p f) -> p f", p=128)
                )
                ngd = mp.tile([128, P // 128], F32)
                nc.vector.tensor_scalar(
                    ngd[:], gdsq[:], -1.0, None, op0=ALU.mult
                )
                kth = mp.tile([128, 2], F32)
                nc.gpsimd.kth_largest(
                    kth[:], ngd[:], P // 128, K + 3,
                    quantile=1.0 - (K - 0.5) / (P - 1),
                )
                # kth col1 = desc[k_adj+1] = -s[K] ; mask = d < s[K]
                thb = mp.tile([128, 2], F32)
                nc.gpsimd.partition_broadcast(thb[:], kth[0:1, :])
                thneg = mp.tile([128, 1], F32)
                nc.vector.tensor_scalar(
                    thneg[:], thb[:, 1:2], -1.0, None, op0=ALU.mult
                )
                # softmax shift: any global constant cancels exactly; use
                # gd[0] (safe unless the dist spread nears 176/TEMP).
                dref = mp.tile([128, 1], F32)
                nc.gpsimd.partition_broadcast(dref[:], gdsq[0:1, 0:1])
                biast = mp.tile([128, 1], F32)
                nc.gpsimd.tensor_scalar(
                    biast[:], dref[:], TEMP, None, op0=ALU.mult
                )
                mask4 = mp.tile([128, NT], F32)
                nc.vector.tensor_scalar(
                    mask4[:], down[:], thneg[:, 0:1], None, op0=ALU.is_lt
                )
                e4 = mp.tile([128, NT], F32)
                nc.scalar.activation(
                    e4[:], down[:], ACTF.Exp, bias=biast[:, 0:1], scale=-TEMP
                )
                w4 = mp.tile([128, NT], F32)
                nc.vector.tensor_tensor(w4[:], e4[:], mask4[:], op=ALU.mult)
                wb = mp.tile([128, NT], BF16)
                nc.scalar.activation(wb[:], w4[:], ACTF.Copy)
                # sum of weights across members (free then partitions)
                slocal = mp.tile([128, 1], F32)
                nc.vector.tensor_reduce(
                    slocal[:], w4[:], axis=mybir.AxisListType.X, op=ALU.add
                )
                swr = mp.tile([128, 1], F32)
                nc.gpsimd.partition_all_reduce(
                    swr[:], slocal[:], 128, bass_isa.ReduceOp.add
                )
                # bf16 warmup gates: dnb ready at DTW end, gsb ready when
                # the gathered dists land (a few us before the weights)
                dnb = mp.tile([128, NT], BF16)
                nc.scalar.activation(dnb[:], down[:], ACTF.Copy)
                gsb = mp.tile([128, NT], BF16)
                nc.scalar.activation(gsb[:], gdsq[:, 0:NT], ACTF.Copy)

            if stage >= 4:
                # ---- weighted sums as PE matmuls accumulating over tiles
                NTOT = 2 * T * A + 1
                arin = dp.tile([NTOT], F32)
                arout = dp.tile([NTOT], F32)
                nc.sync.dma_start(
                    arin[2 * T * A : NTOT].rearrange("(o f) -> o f", o=1),
                    swr[0:1, 0:1],
                )
                _pst_cm = tc.tile_pool(name="psum_st", bufs=1, space="PSUM")
                pst = _pst_cm.__enter__()
                sts = []
                for c in range(8):
                    st = pst.tile([128, 512], F32, tag=f"st{c}")
                    sts.append(st)
                # PE p-state warmup: junk matmuls gated on the dists; their
                # outputs are reset by the first start=True real matmul.
                for wi in range(WARM + WARM2):
                    wsrc = dnb if wi < WARM else gsb
                    nc.tensor.matmul(
                        sts[wi % 8][0:1, :],
                        wsrc[:, 0:1],
                        actT[:, wi % NT, (wi % 8) * 16 : (wi % 8) * 16 + 16, :],
                        start=True, stop=True, skip_group_check=True,
                    )
                # staging rows alias dead actb (32-aligned partitions)
                arsc = actb[:].rearrange("t p a -> t (p a)").bitcast(F32)
                arsb_m = arsc[0:1, 0 : T * A]
                arsb_s = arsc[32:33, 0 : T * A]
                for c in range(8):
                    for k in range(NT):
                        nc.tensor.matmul(
                            sts[c][0:1, :],
                            wb[:, k : k + 1],
                            actT[:, k, c * 16 : (c + 1) * 16, :],
                            start=(k == 0), stop=(k == NT - 1),
                        )
                    nc.scalar.activation(
                        arsb_m[:, c * 512 : (c + 1) * 512],
                        sts[c][0:1, :], ACTF.Copy,
                    )
                for c in range(8):
                    for k in range(NT):
                        nc.tensor.matmul(
                            sts[c][32:33, :],
                            wb[:, k : k + 1],
                            act2T[:, k, c * 16 : (c + 1) * 16, :],
                            start=(k == 0), stop=(k == NT - 1),
                        )
                    nc.vector.tensor_copy(
                        arsb_s[:, c * 512 : (c + 1) * 512],
                        sts[c][32:33, :],
                    )

                nc.sync.dma_start(
                    arin[0 : T * A].rearrange("(o f) -> o f", o=1), arsb_m[:]
                )
                nc.sync.dma_start(
                    arin[T * A : 2 * T * A].rearrange("(o f) -> o f", o=1),
                    arsb_s[:],
                )
                if single:
                    nc.sync.dma_start(arout[:], arin[:])
                else:
                    nc.gpsimd.collective_compute(
                        "AllReduce",
                        ALU.add,
                        replica_groups=GROUP,
                        ins=[arin.opt()],
                        outs=[arout.opt()],
                    )
                _pst_cm.__exit__(None, None, None)

            if stage >= 5:
                # ---- final statistics
                rn12 = mp.tile([128, 2, A], F32)
                nc.sync.dma_start(
                    rn12[:],
                    arout[0 : 2 * T * A].rearrange(
                        "(q t a) -> t q a", q=2, t=T
                    ),
                )
                rs = mp.tile([128, 1], F32)
                _, rssrc = bass.broadcast_tensor_aps(
                    rs[:],
                    arout[2 * T * A : NTOT].rearrange("(o f) -> o f", o=1),
                )
                nc.sync.dma_start(rs[:], rssrc)
                rn1 = rn12[:, 0]
                rn2 = rn12[:, 1]
                inv = mp.tile([128, 1], F32)
                nc.vector.reciprocal(inv[:], rs[:])
                mh = mp.tile([128, A], F32)
                nc.vector.tensor_scalar(
                    mh[:], rn1, inv[:, 0:1], None, op0=ALU.mult
                )
                q = mp.tile([128, A], F32)
                nc.vector.tensor_scalar(
                    q[:], rn2, inv[:, 0:1], None, op0=ALU.mult
                )
                msq = mp.tile([128, A], F32)
                nc.vector.tensor_tensor(msq[:], mh[:], mh[:], op=ALU.mult)
                var = mp.tile([128, A], F32)
                nc.vector.tensor_tensor(var[:], q[:], msq[:], op=ALU.subtract)
                nc.vector.tensor_scalar(var[:], var[:], 0.0, None, op0=ALU.max)
                stdv = mp.tile([128, A], F32)
                ostk = mp.tile([128, 2, A], F32)
                nc.scalar.sqrt(stdv[:], var[:])
                nc.vector.tensor_scalar(
                    ostk[:, 1], stdv[:], MIN_STD, 1.0, op0=ALU.max, op1=ALU.min
                )
                nc.vector.tensor_scalar(
                    mh[:], mh[:], 1.0 - MOM, None, op0=ALU.mult
                )
                nc.vector.scalar_tensor_tensor(
                    ostk[:, 0], means_t[:], MOM, mh[:], op0=ALU.mult,
                    op1=ALU.add,
                )
                nc.sync.dma_start(
                    out_d.rearrange("q t o a -> t (q o) a"), ostk[:]
                )
            else:
                # bisect debug output
                dbg = mp.tile([128, A], F32)
                nc.vector.memset(dbg[:], 0.0)
                nc.vector.tensor_copy(dbg[:, 0:NT], down[:])
                if stage >= 3:
                    nc.vector.tensor_copy(dbg[:, 4 : 4 + NT], w4[:])
                    nc.vector.tensor_copy(dbg[:, 8:9], thneg[:])
                    nc.vector.tensor_copy(dbg[:, 9:10], swr[:])
                if stage == 2:
                    gdbg = mp.tile([128, A], F32)
                    nc.sync.dma_start(
                        gdbg[:],
                        gd[0 : 128 * A].rearrange("(p a) -> p a", a=A),
                    )
                    nc.vector.tensor_copy(dbg[:, 4:8], gdbg[:, 0:4])
                nc.sync.dma_start(out_d[0, :, 0, :], dbg[:])
                nc.sync.dma_start(out_d[1, :, 0, :], dbg[:])

    nc.compile()
    return nc


def _get_nc(stage=None, single=None):
    if stage is None:
        stage = int(os.environ.get("CEM_STAGE", "9"))
    if single is None:
        single = bool(int(os.environ.get("CEM_SINGLE", "0")))
    key = ("nc", stage, single)
    if key not in _CACHE:
        _CACHE[key] = _build(stage, single)
    return _CACHE[key]


def kernel(**inputs):
    obs = np.ascontiguousarray(np.asarray(inputs["obs_diffs"], np.float32))
    means = np.ascontiguousarray(np.asarray(inputs["means"], np.float32))
    stds = np.ascontiguousarray(np.asarray(inputs["stds"], np.float32))
    noise = np.ascontiguousarray(np.asarray(inputs["noise"], np.float32))

    nc = _get_nc(stage=9, single=False)
    in_maps = []
    for c in range(NCORES):
        in_maps.append(
            {
                "obs": obs[c * PL : (c + 1) * PL],
                "means": means,
                "stds": stds,
                "noise": np.ascontiguousarray(noise[:, c * PL : (c + 1) * PL, :]),
            }
        )
    res = bass_utils.run_bass_kernel_spmd(
        nc, in_maps, core_ids=list(range(NCORES))
    )
    out = np.asarray(res.results[0]["out"], np.float32)
    return out.reshape(2, T, 1, A)
